# revision 1
# baseline (speedup 1.0000x reference)
"""Encoder kernel library: builds the full Bass/Tile program for
nn_Encoder (watermark encoder) on 8 TRN2 cores, data-parallel over batch.

Layout conventions per core (1 image):
  activations y_k in DRAM as [64, 256, 256] (pre-BN, conv bias added)
  enc in DRAM as [h, c, w] = [256, 64, 256]
  lpn in DRAM as [3, 256, 256]  (holds -floor(clip(low,0,255)) ; conv0 weights negated/255)
"""
import numpy as np
import concourse.bass as bass
import concourse.tile as tile
from concourse import bacc, mybir
from contextlib import ExitStack

f32 = mybir.dt.float32
f32r = mybir.dt.float32r
FT = mybir.ActivationFunctionType
ALU = mybir.AluOpType

H = W = 256
HW = H * W
CH = 64
MSG = 30
WP = W + 2          # padded row length 258
R = 32              # conv band rows
NBAND = H // R      # 8

# ---------------------------------------------------------------- host consts
def host_constants():
    j = np.arange(256)
    ang = 2.0 * np.pi * np.outer(j, j) / 256.0
    C = np.cos(ang).astype(np.float32)
    S = (-np.sin(ang)).astype(np.float32)      # F = C + iS
    Ci = (C / 256.0).astype(np.float32)
    Sq = (S / 256.0).astype(np.float32)
    consts = {
        "cC": C, "cS": S, "cNS": -S, "cCi": Ci, "cSq": Sq, "cNSq": -Sq,
        "cCS": np.hstack([C, S]).astype(np.float32),
        "cIdent": np.eye(128, dtype=np.float32),
    }
    # low-pass mask, ifftshifted, in [kw, kh] layout (symmetric anyway)
    yy = np.arange(H, dtype=np.float32) - H // 2
    xx = np.arange(W, dtype=np.float32) - W // 2
    m = ((yy[:, None] ** 2 + xx[None, :] ** 2) <= float(50 * 50)).astype(np.float32)
    consts["cMask"] = np.fft.ifftshift(m).astype(np.float32)
    # watermark position constants (all in channel 0; block kh,kw in 124..132)
    cy = cx = 128
    rr, cc = [], []
    idx = 0
    for i in range(-4, 5):
        for j2 in range(-4, 5):
            if idx >= MSG:
                break
            if (i * i + j2 * j2) ** 0.5 <= 4:
                rr.append(cy + i); cc.append(cx + j2); idx += 1
    rr = np.array(rr); cc = np.array(cc)   # rr = kh (axis -2), cc = kw (axis -1)
    pk = np.zeros((MSG, 9), np.float32)    # partition (kw) indicator
    fk = np.zeros((MSG, 9), np.float32)    # free (kh) indicator
    kap = np.zeros((9, 9), np.float32)
    for k in range(MSG):
        pk[k, cc[k] - 124] = 1.0
        fk[k, rr[k] - 124] = 1.0
        kap[cc[k] - 124, rr[k] - 124] = 1.0
    consts["cPk"] = pk
    consts["cFk"] = fk
    consts["cOnemk"] = (1.0 - kap)
    return consts


# ---------------------------------------------------------------- builders
class Enc:
    def __init__(self, n_cores=8, debug_outs=()):
        self.n_cores = n_cores
        self.ntot = float(n_cores * HW)
        self.debug_outs = debug_outs
        nc = bacc.Bacc("TRN2", target_bir_lowering=False, debug=False,
                       num_devices=n_cores)
        self.nc = nc
        d = {}
        d["image"] = nc.dram_tensor("image", (3, H, W), f32, kind="ExternalInput").ap()
        d["message"] = nc.dram_tensor("message", (MSG, 1), f32, kind="ExternalInput").ap()
        d["w0"] = nc.dram_tensor("w0", (64, 3, 3, 3), f32, kind="ExternalInput").ap()
        d["b0"] = nc.dram_tensor("b0", (64, 1), f32, kind="ExternalInput").ap()
        d["g0"] = nc.dram_tensor("g0", (64, 1), f32, kind="ExternalInput").ap()
        d["be0"] = nc.dram_tensor("be0", (64, 1), f32, kind="ExternalInput").ap()
        d["ws"] = nc.dram_tensor("ws", (3, 64, 64, 3, 3), f32, kind="ExternalInput").ap()
        d["bs"] = nc.dram_tensor("bs", (3, 64, 1), f32, kind="ExternalInput").ap()
        d["gs"] = nc.dram_tensor("gs", (3, 64, 1), f32, kind="ExternalInput").ap()
        d["bes"] = nc.dram_tensor("bes", (3, 64, 1), f32, kind="ExternalInput").ap()
        d["acw"] = nc.dram_tensor("acw", (64, 97, 3, 3), f32, kind="ExternalInput").ap()
        d["acb"] = nc.dram_tensor("acb", (64, 1), f32, kind="ExternalInput").ap()
        d["acg"] = nc.dram_tensor("acg", (64, 1), f32, kind="ExternalInput").ap()
        d["acbe"] = nc.dram_tensor("acbe", (64, 1), f32, kind="ExternalInput").ap()
        d["fw"] = nc.dram_tensor("fw", (3, 64), f32, kind="ExternalInput").ap()
        d["fb"] = nc.dram_tensor("fb", (3, 1), f32, kind="ExternalInput").ap()
        for k, shp in [("cC", (256, 256)), ("cS", (256, 256)), ("cNS", (256, 256)),
                       ("cCi", (256, 256)), ("cSq", (256, 256)), ("cNSq", (256, 256)),
                       ("cCS", (256, 512)),
                       ("cIdent", (128, 128)), ("cMask", (256, 256)),
                       ("cPk", (MSG, 9)), ("cFk", (MSG, 9)), ("cOnemk", (9, 9))]:
            d[k] = nc.dram_tensor(k, shp, f32, kind="ExternalInput").ap()
        d["out"] = nc.dram_tensor("out", (3, H, W), f32, kind="ExternalOutput").ap()
        self.d = d
        self.dbg = {}

    def maybe_debug(self, name, shape):
        """Declare an extra output for stage validation."""
        if name in self.debug_outs:
            self.dbg[name] = self.nc.dram_tensor(
                "dbg_" + name, shape, f32, kind="ExternalOutput").ap()
            return self.dbg[name]
        return None

    # ------------------------------------------------------------------
    def build(self):
        nc, d = self.nc, self.d
        with tile.TileContext(nc) as tc, ExitStack() as X:
            cp = X.enter_context(tc.tile_pool(name="consts", bufs=1))
            dp = X.enter_context(tc.tile_pool(name="dram", bufs=1, space="DRAM"))
            pwX = ExitStack()
            pw = pwX.enter_context(tc.tile_pool(name="pw", bufs=2, space="PSUM"))
            wev = pwX.enter_context(tc.tile_pool(name="wev", bufs=2))

            # ---------------- DRAM scratch
            y = [dp.tile([64, H, W], f32, name=f"yact{k}") for k in range(5)]
            lpn_d = dp.tile([3, H, W], f32, name="lpn_d")
            enc_d = dp.tile([H, 64, W], f32, name="enc_d")
            cl_in = [dp.tile([64, 2], f32, name=f"clin{k}") for k in range(5)]
            cl_out = [dp.tile([64, 2], f32, name=f"clout{k}", addr_space="Shared")
                      for k in range(5)]

            # ---------------- constants into SBUF
            def cload(name, src, shape, dtype):
                t = cp.tile(shape, dtype, name=name)
                nc.sync.dma_start(t[:], src if dtype == f32 else src.bitcast(dtype))
                return t
            # DFT matrices, f32r halves for main chain, f32 for lowpass chain
            DF = {}
            for nm, wdt in [("cC", 256), ("cS", 256), ("cNS", 256), ("cCi", 256),
                            ("cSq", 256), ("cNSq", 256), ("cCS", 512)]:
                DF[nm + "_hi"] = cload(nm + "_hi", d[nm][0:128, :], [128, wdt], f32r)
                DF[nm + "_lo"] = cload(nm + "_lo", d[nm][128:256, :], [128, wdt], f32r)
                DF[nm + "_hi32"] = cload(nm + "_hi32", d[nm][0:128, :], [128, wdt], f32)
                DF[nm + "_lo32"] = cload(nm + "_lo32", d[nm][128:256, :], [128, wdt], f32)
            ident = cload("ident", d["cIdent"][:], [128, 128], f32)
            maskt = [cload(f"maskt{i}", d["cMask"][i * 128:(i + 1) * 128, :],
                           [128, 256], f32) for i in range(2)]
            pk_t = cload("pk_t", d["cPk"][:], [MSG, 9], f32)
            fk_t = cload("fk_t", d["cFk"][:], [MSG, 9], f32)
            onemk_t = cload("onemk_t", d["cOnemk"][:], [9, 9], f32)
            msg_t = cload("msg_t", d["message"][:], [MSG, 1], f32)
            ones30 = cp.tile([MSG, 1, 256], f32, name="ones30")
            nc.vector.memset(ones30[:], 1.0)
            zero128 = cp.tile([128, 1], f32, name="zero128")
            nc.vector.memset(zero128[:], 0.0)
            eps64 = cp.tile([64, 1], f32, name="eps64")
            nc.vector.memset(eps64[:], 1e-5)

            # per-layer bn param tiles
            def vload(name, src):
                t = cp.tile([64, 1], f32, name=name)
                nc.sync.dma_start(t[:], src)
                return t
            g_t = [vload("g_t0", d["g0"][:])] + \
                  [vload(f"g_t{k+1}", d["gs"][k]) for k in range(3)] + \
                  [vload("g_t4", d["acg"][:])]
            be_t = [vload("be_t0", d["be0"][:])] + \
                   [vload(f"be_t{k+1}", d["bes"][k]) for k in range(3)] + \
                   [vload("be_t4", d["acbe"][:])]
            # conv bias replicated to 128 partitions (for double-chunk evict)
            b128 = []
            for k, src in enumerate([d["b0"], d["bs"][0], d["bs"][1], d["bs"][2],
                                     d["acb"]]):
                t = cp.tile([128, 1], f32, name=f"b128_{k}")
                nc.sync.dma_start(t[0:64, :], src)
                nc.sync.dma_start(t[64:128, :], src)
                b128.append(t)
            fb_t = cp.tile([3, 1], f32, name="fb_t")
            nc.sync.dma_start(fb_t[:], d["fb"][:])

            # ---------------- weight transposes (lhsT prep)
            # conv1..3: pair lhsT [128,64] x3(dw), single lhsT [64,64] x3
            lhsT_pair, lhsT_sing = [], []
            for k in range(3):
                wsrc = cp.tile([64, 64, 9], f32, name=f"wsrc{k}")
                nc.sync.dma_start(wsrc[:], d["ws"][k].rearrange("o i a b -> o i (a b)"))
                pairs, sings = [], []
                for dw in range(3):
                    pA = cp.tile([128, 128], f32r, name=f"lhsTpA{k}{dw}")
                    pB = cp.tile([128, 128], f32r, name=f"lhsTpB{k}{dw}")
                    cx = cp.tile([64, 128], f32r, name=f"lhsTcx{k}{dw}")
                    nc.vector.memset(pA[:].bitcast(f32), 0.0)
                    nc.vector.memset(pB[:].bitcast(f32), 0.0)
                    tp3 = []
                    for dh in range(3):
                        p = pw.tile([64, 64], f32, name="pwt")
                        nc.tensor.transpose(p[:], wsrc[:, :, dh * 3 + dw], ident[0:64, 0:64])
                        tp3.append(p)
                    # pairA: K=(dh-1,dh0), M cols 0:64  (upper chunk rows t,t+1)
                    nc.vector.tensor_copy(pA[0:64, 0:64], tp3[0][:])
                    nc.vector.tensor_copy(pA[64:128, 0:64], tp3[1][:])
                    # pairB: K=(dh0,dh+1), M cols 64:128 (lower chunk rows t+2,t+3)
                    nc.vector.tensor_copy(pB[0:64, 64:128], tp3[1][:])
                    nc.vector.tensor_copy(pB[64:128, 64:128], tp3[2][:])
                    # cross: K=64, M = [W(+1) | W(-1)]
                    nc.vector.tensor_copy(cx[:, 0:64], tp3[2][:])
                    nc.vector.tensor_copy(cx[:, 64:128], tp3[0][:])
                    pairs.append((pA, pB)); sings.append(cx)
                lhsT_pair.append(pairs); lhsT_sing.append(sings)
            # conv0: lhsT0[dw] [9,64] = -w0[:,ci,dh,dw]^T/255 stacked over dh
            w0src = cp.tile([64, 3, 9], f32, name="w0src")
            nc.sync.dma_start(w0src[:], d["w0"][:].rearrange("o i a b -> o i (a b)"))
            lhsT0 = []
            for dw in range(3):
                l0 = cp.tile([18, 128], f32r, name=f"lhsT0{dw}")
                nc.vector.memset(l0[:].bitcast(f32), 0.0)
                for dh in range(3):
                    p = pw.tile([64, 64], f32, name="pwt")
                    nc.tensor.transpose(p[0:3, :], w0src[:, :, dh * 3 + dw],
                                        ident[0:64, 0:64])
                    tmp0 = wev.tile([3, 64], f32r, name="w0tmp")
                    nc.vector.tensor_scalar_mul(tmp0[:], p[0:3, :], -1.0 / 255.0)
                    nc.sync.dma_start(l0[dh * 3:dh * 3 + 3, 0:64], tmp0[:])
                    nc.sync.dma_start(l0[9 + dh * 3:9 + dh * 3 + 3, 64:128], tmp0[:])
                lhsT0.append(l0)
            # ac conv: lhsT_ac[dh*3+dw] [97,64]
            acsrc = cp.tile([64, 97, 9], f32, name="acsrc")
            nc.sync.dma_start(acsrc[:], d["acw"][:].rearrange("o i a b -> o i (a b)"))
            lhsT_ac = []
            ac_tp = []
            for tap in range(9):
                la = [cp.tile([97, 128], f32r, name=f"lhsTac{tap}{v}") for v in range(2)]
                for v in range(2):
                    nc.vector.memset(la[v][:].bitcast(f32), 0.0)
                p = pw.tile([97, 64], f32, name="pwa", bufs=4)
                nc.tensor.transpose(p[:], acsrc[:, :, tap], ident[0:64, 0:64])
                for v in range(2):
                    nc.vector.tensor_copy(la[v][:, v * 64:v * 64 + 64], p[:])
                lhsT_ac.append(la)
                ac_tp.append(p)
            lhsT_accx = []
            for dw in range(3):
                cxa = cp.tile([97, 128], f32r, name=f"lhsTaccx{dw}")
                nc.vector.tensor_copy(cxa[:, 0:64], ac_tp[6 + dw][:])
                nc.vector.tensor_copy(cxa[:, 64:128], ac_tp[0 + dw][:])
                lhsT_accx.append(cxa)
            # final 1x1: lhsT_fin [64,3]
            fwsrc = cp.tile([3, 64], f32, name="fwsrc")
            nc.sync.dma_start(fwsrc[:], d["fw"][:])
            lhsT_fin = cp.tile([128, 6], f32r, name="lhsT_fin")
            nc.vector.memset(lhsT_fin[:].bitcast(f32), 0.0)
            p = pw.tile([64, 64], f32, name="pwt")
            nc.tensor.transpose(p[:, 0:3], fwsrc[:], ident[0:3, 0:3])
            nc.vector.tensor_copy(lhsT_fin[0:64, 0:3], p[:, 0:3])
            nc.vector.tensor_copy(lhsT_fin[64:128, 3:6], p[:, 0:3])

            # ---------------- watermark value prep
            msgc = cp.tile([MSG, 9], f32, name="msgc")
            nc.vector.tensor_scalar(msgc[:], fk_t[:], msg_t[:, 0:1], None, op0=ALU.mult)
            pwm = pw.tile([9, 9], f32, name="pwm")
            nc.tensor.matmul(pwm[:], pk_t[:], msgc[:], start=True, stop=True)
            wmv = cp.tile([9, 9], f32, name="wmv")
            nc.vector.tensor_copy(wmv[:], pwm[:])
            wm_al = [cp.tile([128, 9], f32, name=f"wm_al{i}") for i in range(2)]
            onemk_al = [cp.tile([128, 9], f32, name=f"onemk_al{i}") for i in range(2)]
            for i in range(2):
                nc.vector.memset(wm_al[i][:], 0.0)
                nc.vector.memset(onemk_al[i][:], 1.0)
            nc.sync.dma_start(wm_al[0][124:128, :], wmv[0:4, :])
            nc.sync.dma_start(wm_al[1][0:5, :], wmv[4:9, :])
            nc.sync.dma_start(onemk_al[0][124:128, :], onemk_t[0:4, :])
            nc.sync.dma_start(onemk_al[1][0:5, :], onemk_t[4:9, :])

            self._consts = dict(DF=DF, ident=ident, maskt=maskt, ones30=ones30,
                                g_t=g_t, be_t=be_t, b128=b128, fb_t=fb_t,
                                lhsT_pair=lhsT_pair, lhsT_sing=lhsT_sing,
                                lhsT0=lhsT0, lhsT_ac=lhsT_ac, lhsT_accx=lhsT_accx, lhsT_fin=lhsT_fin,
                                wm_al=wm_al, onemk_al=onemk_al, msg_sc=msg_t,
                                zero128=zero128, eps64=eps64, cp=cp,
                                y=y, lpn_d=lpn_d, enc_d=enc_d,
                                cl_in=cl_in, cl_out=cl_out)
            pwX.close()
            self._build_body(X, tc)
        self.nc.compile()

    # ------------------------------------------------------------------
    def _stats_finalize(self, tc, pool, layer, ssum_cols, sqsum_cols, ncols):
        """Reduce per-chunk stat columns, AllReduce, return (scale, shift) [64,1]."""
        nc = self.nc
        C = self._consts
        red = pool.tile([128, 2], f32, name=f"red{layer}")
        nc.vector.tensor_reduce(red[:, 0:1], ssum_cols[:, 0:ncols], axis=mybir.AxisListType.X, op=ALU.add)
        nc.vector.tensor_reduce(red[:, 1:2], sqsum_cols[:, 0:ncols], axis=mybir.AxisListType.X, op=ALU.add)
        upper = pool.tile([64, 2], f32, name=f"upper{layer}")
        nc.sync.dma_start(upper[:], red[64:128, :])
        stats = pool.tile([64, 2], f32, name=f"stats{layer}")
        nc.vector.tensor_add(stats[:], red[0:64, :], upper[:])
        nc.sync.dma_start(C["cl_in"][layer][:], stats[:])
        ap = self.maybe_debug(f"st{layer}", (64, 2))
        if ap is not None:
            nc.sync.dma_start(ap[:], stats[:])
        nc.gpsimd.collective_compute(
            "AllReduce", ALU.add,
            replica_groups=[list(range(self.n_cores))],
            ins=[C["cl_in"][layer].opt()], outs=[C["cl_out"][layer].opt()])
        sr = pool.tile([64, 2], f32, name=f"sr{layer}")
        nc.sync.dma_start(sr[:], C["cl_out"][layer][:])
        ap = self.maybe_debug(f"sr{layer}", (64, 2))
        if ap is not None:
            nc.sync.dma_start(ap[:], sr[:])
        mean = pool.tile([64, 1], f32, name=f"mean{layer}")
        nc.vector.tensor_scalar_mul(mean[:], sr[:, 0:1], 1.0 / self.ntot)
        ms = pool.tile([64, 1], f32, name=f"ms{layer}")
        nc.vector.tensor_scalar_mul(ms[:], sr[:, 1:2], 1.0 / self.ntot)
        msq = pool.tile([64, 1], f32, name=f"msq{layer}")
        nc.vector.tensor_scalar(msq[:], mean[:], mean[:, 0:1], None, op0=ALU.mult)
        var = pool.tile([64, 1], f32, name=f"var{layer}")
        nc.vector.tensor_scalar(var[:], ms[:], msq[:, 0:1], None, op0=ALU.subtract)
        std = pool.tile([64, 1], f32, name=f"std{layer}")
        nc.scalar.activation(std[:], var[:], FT.Sqrt, bias=C["eps64"][:, 0:1], scale=1.0)
        istd = pool.tile([64, 1], f32, name=f"istd{layer}")
        nc.vector.reciprocal(istd[:], std[:])
        scale = pool.tile([64, 1], f32, name=f"scale{layer}")
        nc.vector.tensor_tensor(scale[:], C["g_t"][layer][:], istd[:], op=ALU.mult)
        prod = pool.tile([64, 1], f32, name=f"prod{layer}")
        nc.vector.tensor_tensor(prod[:], mean[:], scale[:], op=ALU.mult)
        shift = pool.tile([64, 1], f32, name=f"shift{layer}")
        nc.vector.scalar_tensor_tensor(shift[:], prod[:], -1.0, C["be_t"][layer][:],
                                       op0=ALU.mult, op1=ALU.add)
        ap = self.maybe_debug(f"sc{layer}", (64, 1))
        if ap is not None:
            nc.sync.dma_start(ap[:], scale[:])
        ap = self.maybe_debug(f"sh{layer}", (64, 1))
        if ap is not None:
            nc.sync.dma_start(ap[:], shift[:])
        return scale, shift

    # ------------------------------------------------------------------
    def _conv64(self, X, tc, layer, src, dst, scale, shift):
        """conv layers 1..3 (64->64) and layer 4 (ac, 97->64) share this via flags."""
        nc = self.nc
        C = self._consts
        is_ac = (layer == 4)
        KP = 97 if is_ac else 128
        with ExitStack() as S:
            bp = S.enter_context(tc.tile_pool(name=f"band{layer}", bufs=3))
            pp = S.enter_context(tc.tile_pool(name=f"psum{layer}", bufs=3, space="PSUM"))
            ep = S.enter_context(tc.tile_pool(name=f"evict{layer}", bufs=3))
            sp = S.enter_context(tc.tile_pool(name=f"stat{layer}", bufs=1))
            ssum_cols = sp.tile([128, 64], f32, name=f"ssc{layer}")
            sqsum_cols = sp.tile([128, 64], f32, name=f"sqc{layer}")
            cidx = 0
            for bi in range(NBAND):
                r0 = bi * R
                band = bp.tile([KP, (R + 2) * WP], f32r, name=f"bandt{layer}")
                b3 = band[:].rearrange("p (r c) -> p r c", c=WP)
                # rows of image present in band: band row i = image row r0-1+i
                i0 = 1 if bi == 0 else 0
                i1 = R + 1 if bi == NBAND - 1 else R + 2
                rl, rh = r0 - 1 + i0, r0 - 1 + i1
                if is_ac:
                    nc.vector.memset(b3[0:30, :, :].bitcast(f32), 0.0)
                    nc.vector.memset(b3[0:97, :, 0:1].bitcast(f32), 0.0)
                    nc.vector.memset(b3[0:97, :, 257:258].bitcast(f32), 0.0)
                    if bi == 0:
                        nc.vector.memset(b3[0:97, 0:1, :].bitcast(f32), 0.0)
                    if bi == NBAND - 1:
                        nc.vector.memset(b3[0:97, R + 1:R + 2, :].bitcast(f32), 0.0)
                    # msg channels: interior = message value
                    nc.scalar.activation(
                        b3[0:30, i0:i1, 1:257],
                        C["ones30"][:].broadcast_to([MSG, i1 - i0, 256]),
                        FT.Copy, bias=0.0, scale=C["msg_sc"][:, 0:1])
                    # enc channels from enc_d [h,c,w]
                    nc.sync.dma_start(
                        b3[30:94, i0:i1, 1:257],
                        C["enc_d"][rl:rh, :, :].transpose([1, 0, 2]).bitcast(f32r))
                    # image channels
                    nc.sync.dma_start(b3[94:97, i0:i1, 1:257],
                                      self.d["image"][:, rl:rh, :].bitcast(f32r))
                else:
                    nc.vector.memset(b3[0:64, :, 0:1].bitcast(f32), 0.0)
                    nc.vector.memset(b3[0:64, :, 257:258].bitcast(f32), 0.0)
                    if bi == 0:
                        nc.vector.memset(b3[0:64, 0:1, :].bitcast(f32), 0.0)
                    if bi == NBAND - 1:
                        nc.vector.memset(b3[0:64, R + 1:R + 2, :].bitcast(f32), 0.0)
                    nc.sync.dma_start(b3[0:64, i0:i1, 1:257],
                                      src[:, rl:rh, :].bitcast(f32r))
                    nc.scalar.activation(b3[0:64, i0:i1, 1:257],
                                         b3[0:64, i0:i1, 1:257].bitcast(f32),
                                         FT.Relu, bias=shift[:, 0:1], scale=scale[:, 0:1])
                    # dup: partitions 64:128 = band shifted one row down
                    nc.sync.dma_start(b3[64:128, 0:R + 1, :],
                                      b3[0:64, 1:R + 2, :])
                for t in range(0, R, 4):
                    pt = pp.tile([128, 512], f32, name=f"pchunk{layer}")
                    if is_ac:
                        nmm_tot = 15
                        mm = 0
                        for dw in range(3):
                            for dh in range(2):       # upper taps dh=-1,0
                                nc.tensor.matmul(
                                    pt[:], C["lhsT_ac"][dh * 3 + dw][0][:],
                                    b3[0:97, t + dh:t + dh + 2, dw:dw + 256],
                                    start=(mm == 0), stop=(mm == nmm_tot - 1))
                                mm += 1
                            for dh in range(1, 3):    # lower taps dh=0,+1
                                nc.tensor.matmul(
                                    pt[:], C["lhsT_ac"][dh * 3 + dw][1][:],
                                    b3[0:97, t + 2 + dh:t + 4 + dh, dw:dw + 256],
                                    start=(mm == 0), stop=(mm == nmm_tot - 1))
                                mm += 1
                            nc.tensor.matmul(         # cross: upper +1, lower -1
                                pt[:], C["lhsT_accx"][dw][:],
                                b3[0:97, t + 2:t + 4, dw:dw + 256],
                                start=(mm == 0), stop=(mm == nmm_tot - 1))
                            mm += 1
                    else:
                        nmm_tot = 9
                        mm = 0
                        for dw in range(3):
                            pA, pB = C["lhsT_pair"][layer - 1][dw]
                            nc.tensor.matmul(
                                pt[:], pA[:], b3[0:128, t:t + 2, dw:dw + 256],
                                start=(mm == 0), stop=(mm == nmm_tot - 1))
                            mm += 1
                            nc.tensor.matmul(
                                pt[:], pB[:], b3[0:128, t + 3:t + 5, dw:dw + 256],
                                start=(mm == 0), stop=(mm == nmm_tot - 1))
                            mm += 1
                            nc.tensor.matmul(
                                pt[:], C["lhsT_sing"][layer - 1][dw][:],
                                b3[0:64, t + 2:t + 4, dw:dw + 256],
                                start=(mm == 0), stop=(mm == nmm_tot - 1))
                            mm += 1
                    ysb = ep.tile([128, 512], f32, name=f"ysb{layer}")
                    nc.vector.tensor_scalar(ysb[:], pt[:], C["b128"][layer][:, 0:1],
                                            0.0, op0=ALU.add, op1=ALU.add,
                                            accum_out=ssum_cols[:, cidx:cidx + 1])
                    scr = ep.tile([128, 512], f32, name=f"sqscr{layer}")
                    nc.scalar.activation(scr[:], ysb[:], FT.Square,
                                         bias=C["zero128"][:, 0:1],
                                         accum_out=sqsum_cols[:, cidx:cidx + 1])
                    cidx += 1
                    q = r0 + t
                    nc.sync.dma_start(dst[:, q:q + 2, :],
                                      ysb[0:64, :].rearrange("p (r c) -> p r c", c=256))
                    nc.sync.dma_start(dst[:, q + 2:q + 4, :],
                                      ysb[64:128, :].rearrange("p (r c) -> p r c", c=256))
            sc, sh = self._stats_finalize(tc, sp, layer, ssum_cols, sqsum_cols, cidx)
            return sc, sh

    # ------------------------------------------------------------------
    def _fft_chain(self, tc, pools, *, src_loader, dt, suffix32,
                   wm=False, mask=False, evict_fn=None):
        """Transpose-free fused fft2 -> edit -> ifft2 for ONE channel.

        All passes are normal matmuls (HAM-visible). Odd passes use the data
        as the stationary operand, producing transposed output for free.
        Layouts (per channel):
          x   [h, w]                     (xt tiles, 2 h-blocks)
          Zt  [w, (Zre|Zim)]  = x^T @ [C|S]          (P1, data-stationary)
          f   [kw, (fre|fim)] = Fw^T @ Zt            (P2, const-stationary)
          Gt  [kh, (Gre|Gim)] = f^T @ [Ci|Sq]-combo  (P3, data-stationary)
          enc [h, w]          = Fi^T @ Gt            (P4, const-stationary)
        src_loader(xt, hbl) fills xt [128, 256]; evict_fn(ap, hbl) consumes
        the final real [128, 256] block (an SBUF tile).
        """
        nc = self.nc
        C = self._consts
        DF = C["DF"]
        xp, zp, pp, ptp = pools

        def LT(nm, chunk):
            return DF[nm + ("_hi" if chunk == 0 else "_lo") + suffix32]

        # ---- load x [h, w]
        xt = [xp.tile([128, 256], dt, name="fft_xt") for _ in range(2)]
        for hbl in range(2):
            src_loader(xt[hbl], hbl)
        # ---- P1: Zt[wbl] = x^T @ [C|S]   (psum [w128, 512])
        Zt = []
        for wbl in range(2):
            pZ = pp.tile([128, 512], f32, name="fft_ps", bufs=5)
            for ch in range(2):
                nc.tensor.matmul(pZ[:], xt[ch][:, wbl * 128:(wbl + 1) * 128],
                                 LT("cCS", ch)[:], start=(ch == 0), stop=(ch == 1))
            zt = zp.tile([128, 512], dt, name="fft_zt")
            nc.vector.tensor_copy(zt[:], pZ[:])
            Zt.append(zt)
        # ---- P2: f[kwbl] [kw, (fre|fim)]; cross-terms accumulate in PSUM
        fsb = []
        for kwbl in range(2):
            pf = pp.tile([128, 512], f32, name="fft_ps", bufs=5)
            sl = slice(kwbl * 128, (kwbl + 1) * 128)
            for ch in range(2):
                nc.tensor.matmul(pf[:], LT("cC", ch)[:, sl], Zt[ch][:],
                                 start=(ch == 0), stop=False)
            for ch in range(2):
                nc.tensor.matmul(pf[:, 0:256], LT("cNS", ch)[:, sl],
                                 Zt[ch][:, 256:512], start=False, stop=False)
            for ch in range(2):
                nc.tensor.matmul(pf[:, 256:512], LT("cS", ch)[:, sl],
                                 Zt[ch][:, 0:256], start=False, stop=(ch == 1))
            ft = zp.tile([128, 512], dt, name="fft_ft")
            if mask:
                nc.vector.tensor_tensor(ft[:, 0:256], pf[:, 0:256],
                                        C["maskt"][kwbl][:], op=ALU.mult)
                nc.vector.tensor_tensor(ft[:, 256:512], pf[:, 256:512],
                                        C["maskt"][kwbl][:], op=ALU.mult)
            else:
                nc.vector.tensor_copy(ft[:], pf[:])
            fsb.append(ft)
        # ---- P3: Gt[khbl] [kh, (Gre|Gim)], data-stationary, PSUM cross-accum
        Gt = []
        for khbl in range(2):
            pG = pp.tile([128, 512], f32, name="fft_ps", bufs=5)
            sl = slice(khbl * 128, (khbl + 1) * 128)
            for ch in range(2):
                nc.tensor.matmul(pG[:, 0:256], fsb[ch][:, sl],
                                 LT("cCi", ch)[:], start=(ch == 0), stop=False)
            for ch in range(2):
                nc.tensor.matmul(pG[:, 0:256], fsb[ch][:, 256 + khbl * 128:256 + (khbl + 1) * 128],
                                 LT("cSq", ch)[:], start=False, stop=False)
            for ch in range(2):
                nc.tensor.matmul(pG[:, 256:512], fsb[ch][:, 256 + khbl * 128:256 + (khbl + 1) * 128],
                                 LT("cCi", ch)[:], start=(ch == 0), stop=False)
            for ch in range(2):
                nc.tensor.matmul(pG[:, 256:512], fsb[ch][:, sl],
                                 LT("cNSq", ch)[:], start=False, stop=(ch == 1))
            gt = zp.tile([128, 512], dt, name="fft_gt")
            nc.vector.tensor_copy(gt[:], pG[:])
            Gt.append(gt)
        # ---- P4: enc[hbl] = Ci^T@Gre + Sq^T@Gim (real)
        for hbl in range(2):
            pE = pp.tile([128, 256], f32, name="fft_pe", bufs=2)
            sl = slice(hbl * 128, (hbl + 1) * 128)
            for ch in range(2):
                nc.tensor.matmul(pE[:], LT("cCi", ch)[:, sl], Gt[ch][:, 0:256],
                                 start=(ch == 0), stop=False)
            for ch in range(2):
                nc.tensor.matmul(pE[:], LT("cSq", ch)[:, sl], Gt[ch][:, 256:512],
                                 start=False, stop=(ch == 1))
            esb = zp.tile([128, 256], f32, name="fft_esb")
            nc.vector.tensor_copy(esb[:], pE[:])
            evict_fn(esb, hbl)

    # ------------------------------------------------------------------
    def _conv0(self, X, tc):
        """conv0: lpn(3ch, negated/255-scaled) -> y0. K=9 via 3 stacked row-shifts."""
        nc = self.nc
        C = self._consts
        with ExitStack() as S:
            bp = S.enter_context(tc.tile_pool(name="band0", bufs=3))
            pp = S.enter_context(tc.tile_pool(name="psum0", bufs=3, space="PSUM"))
            ep = S.enter_context(tc.tile_pool(name="evict0", bufs=3))
            sp = S.enter_context(tc.tile_pool(name="stat0", bufs=1))
            ssum_cols = sp.tile([128, 64], f32, name="ssc0")
            sqsum_cols = sp.tile([128, 64], f32, name="sqc0")
            cidx = 0
            for bi in range(NBAND):
                r0 = bi * R
                band = bp.tile([18, (R + 2) * WP], f32r, name="bandt0")
                b3 = band[:].rearrange("p (r c) -> p r c", c=WP)
                i0 = 1 if bi == 0 else 0
                i1 = R + 1 if bi == NBAND - 1 else R + 2
                rl, rh = r0 - 1 + i0, r0 - 1 + i1
                nc.vector.memset(b3[0:3, :, 0:1].bitcast(f32), 0.0)
                nc.vector.memset(b3[0:3, :, 257:258].bitcast(f32), 0.0)
                if bi == 0:
                    nc.vector.memset(b3[0:3, 0:1, :].bitcast(f32), 0.0)
                if bi == NBAND - 1:
                    nc.vector.memset(b3[0:3, R + 1:R + 2, :].bitcast(f32), 0.0)
                nc.sync.dma_start(b3[0:3, i0:i1, 1:257],
                                  C["lpn_d"][:, rl:rh, :].bitcast(f32r))
                nc.sync.dma_start(b3[3:6, 0:R + 1, :], b3[0:3, 1:R + 2, :])
                nc.sync.dma_start(b3[6:9, 0:R, :], b3[0:3, 2:R + 2, :])
                nc.sync.dma_start(b3[9:18, 0:R, :], b3[0:9, 2:R + 2, :])
                for t in range(0, R, 4):
                    pt = pp.tile([128, 512], f32, name="pchunk0")
                    for dw in range(3):
                        nc.tensor.matmul(
                            pt[:], C["lhsT0"][dw][:],
                            b3[0:18, t:t + 2, dw:dw + 256],
                            start=(dw == 0), stop=(dw == 2))
                    ysb = ep.tile([128, 512], f32, name="ysb0")
                    nc.vector.tensor_scalar(ysb[:], pt[:], C["b128"][0][:, 0:1],
                                            0.0, op0=ALU.add, op1=ALU.add,
                                            accum_out=ssum_cols[:, cidx:cidx + 1])
                    scr = ep.tile([128, 512], f32, name="sqscr0")
                    nc.scalar.activation(scr[:], ysb[:], FT.Square,
                                         bias=C["zero128"][:, 0:1],
                                         accum_out=sqsum_cols[:, cidx:cidx + 1])
                    cidx += 1
                    q = r0 + t
                    nc.sync.dma_start(C["y"][0][:, q:q + 2, :],
                                      ysb[0:64, :].rearrange("p (r c) -> p r c", c=256))
                    nc.sync.dma_start(C["y"][0][:, q + 2:q + 4, :],
                                      ysb[64:128, :].rearrange("p (r c) -> p r c", c=256))
            return self._stats_finalize(tc, sp, 0, ssum_cols, sqsum_cols, cidx)

    # ------------------------------------------------------------------
    def _build_body(self, X, tc):
        nc = self.nc
        C = self._consts
        d = self.d

        # ============ low-pass filter (fp32), 3 channels ============
        with ExitStack() as S:
            xp = S.enter_context(tc.tile_pool(name="lp_x", bufs=4))
            zp = S.enter_context(tc.tile_pool(name="lp_z", bufs=6))
            pp = S.enter_context(tc.tile_pool(name="lp_ps", bufs=1, space="PSUM"))
            ptp = S.enter_context(tc.tile_pool(name="lp_tmp", bufs=4))
            ep = S.enter_context(tc.tile_pool(name="lp_ev", bufs=3))
            for c in range(3):
                def loader(xt, hbl, c=c):
                    nc.sync.dma_start(
                        xt[:], d["image"][c, hbl * 128:(hbl + 1) * 128, :])
                def evict(esb, hbl, c=c):
                    t1 = ep.tile([128, 256], f32, name="lp_t1")
                    nc.vector.tensor_scalar(t1[:], esb[:], 0.0, 255.0,
                                            op0=ALU.max, op1=ALU.min)
                    xi = ep.tile([128, 256], mybir.dt.int32, name="lp_xi")
                    nc.vector.tensor_copy(xi[:], t1[:])
                    xr = ep.tile([128, 256], f32, name="lp_xr")
                    nc.vector.tensor_copy(xr[:], xi[:])
                    gt = ep.tile([128, 256], f32, name="lp_gt")
                    nc.vector.tensor_tensor(gt[:], xr[:], t1[:], op=ALU.is_gt)
                    t2 = ep.tile([128, 256], f32, name="lp_t2")
                    nc.vector.tensor_tensor(t2[:], gt[:], xr[:], op=ALU.subtract)
                    nc.sync.dma_start(C["lpn_d"][c, hbl * 128:(hbl + 1) * 128, :], t2[:])
                self._fft_chain(tc, (xp, zp, pp, ptp),
                                src_loader=loader, dt=f32, suffix32="32",
                                mask=True, evict_fn=evict)

        # ============ conv0 ============
        sc0, sh0 = self._conv0(X, tc)

        # ============ conv1..3 ============
        sc, sh = sc0, sh0
        for k in range(1, 4):
            sc, sh = self._conv64(X, tc, k, C["y"][k - 1], C["y"][k], sc, sh)

        # bn3 scale/shift broadcast to 128 partitions: [128, 64] via transpose+K=1 matmul
        with ExitStack() as S:
            pp = S.enter_context(tc.tile_pool(name="bc_ps", bufs=2, space="PSUM"))
            bcp = S.enter_context(tc.tile_pool(name="bc", bufs=1))
            ones1 = bcp.tile([1, 128], f32, name="ones1")
            nc.vector.memset(ones1[:], 1.0)
            scale_bc = C["cp"].tile([128, 64], f32, name="scale_bc")
            shift_bc = C["cp"].tile([128, 64], f32, name="shift_bc")
            for vec, dst in [(sc, scale_bc), (sh, shift_bc)]:
                ptr = pp.tile([1, 64], f32, name="bc_tr")
                nc.tensor.transpose(ptr[:], vec[:], C["ident"][0:64, 0:64])
                row = bcp.tile([1, 64], f32, name="bc_row")
                nc.vector.tensor_copy(row[:], ptr[:])
                pbc = pp.tile([128, 64], f32, name="bc_mm")
                nc.tensor.matmul(pbc[:], ones1[:], row[:], start=True, stop=True)
                nc.vector.tensor_copy(dst[:], pbc[:])
            C["scale_bc"] = scale_bc
            C["shift_bc"] = shift_bc

        # ============ main fft chain (f32r), 64 channels ============
        with ExitStack() as S:
            xp = S.enter_context(tc.tile_pool(name="m_x", bufs=4))
            zp = S.enter_context(tc.tile_pool(name="m_z", bufs=6))
            pp = S.enter_context(tc.tile_pool(name="m_ps", bufs=1, space="PSUM"))
            ptp = S.enter_context(tc.tile_pool(name="m_tmp", bufs=2))
            for c in range(64):
                def loader(xt, hbl, c=c):
                    nc.sync.dma_start(
                        xt[:],
                        C["y"][3][c, hbl * 128:(hbl + 1) * 128, :].bitcast(f32r))
                    nc.scalar.activation(
                        xt[:], xt[:].bitcast(f32),
                        FT.Relu, bias=C["shift_bc"][:, c:c + 1],
                        scale=C["scale_bc"][:, c:c + 1])
                def evict(esb, hbl, c=c):
                    nc.sync.dma_start(
                        C["enc_d"][hbl * 128:(hbl + 1) * 128, c, :], esb[:])
                self._fft_chain(tc, (xp, zp, pp, ptp),
                                src_loader=loader, dt=f32r, suffix32="",
                                wm=(c == 0), evict_fn=evict)

        # ============ ac conv (97 -> 64) ============
        sc4, sh4 = self._conv64(X, tc, 4, None, C["y"][4], None, None)

        # ============ final 1x1 conv ============
        # rows packed: partitions 0:64 = ch, rows q..; 64:128 = ch, rows q+128..
        with ExitStack() as S:
            bp = S.enter_context(tc.tile_pool(name="fin_b", bufs=3))
            pp = S.enter_context(tc.tile_pool(name="fin_ps", bufs=4, space="PSUM"))
            ep = S.enter_context(tc.tile_pool(name="fin_ev", bufs=3))
            fb6 = C["cp"].tile([6, 1], f32, name="fb6")
            nc.sync.dma_start(fb6[0:3, :], d["fb"][:])
            nc.sync.dma_start(fb6[3:6, :], d["fb"][:])
            sc128 = C["cp"].tile([128, 1], f32, name="fin_sc128")
            sh128 = C["cp"].tile([128, 1], f32, name="fin_sh128")
            for half in range(2):
                nc.sync.dma_start(sc128[64 * half:64 * half + 64, :], sc4[:])
                nc.sync.dma_start(sh128[64 * half:64 * half + 64, :], sh4[:])
            FR = 16   # rows per chunk (per half)
            for q in range(0, 128, FR):
                xf = bp.tile([128, FR, 256], f32r, name="fin_x")
                nc.sync.dma_start(xf[0:64, :, :], C["y"][4][:, q:q + FR, :].bitcast(f32r))
                nc.sync.dma_start(xf[64:128, :, :],
                                  C["y"][4][:, 128 + q:128 + q + FR, :].bitcast(f32r))
                nc.scalar.activation(xf[:].rearrange("p r c -> p (r c)"),
                                     xf[:].rearrange("p r c -> p (r c)").bitcast(f32),
                                     FT.Relu, bias=sh128[:, 0:1], scale=sc128[:, 0:1])
                for rr in range(0, FR, 2):
                    pt = pp.tile([6, 512], f32, name="fin_p")
                    nc.tensor.matmul(pt[:], C["lhsT_fin"][:],
                                     xf[:, rr:rr + 2, :].rearrange("p r c -> p (r c)"),
                                     start=True, stop=True)
                    osb = ep.tile([6, 512], f32, name="fin_o")
                    nc.vector.tensor_scalar(osb[:], pt[:], fb6[:, 0:1], None, op0=ALU.add)
                    nc.sync.dma_start(d["out"][:, q + rr:q + rr + 2, :],
                                      osb[0:3, :].rearrange("p (r c) -> p r c", c=256))
                    nc.sync.dma_start(d["out"][:, 128 + q + rr:128 + q + rr + 2, :],
                                      osb[3:6, :].rearrange("p (r c) -> p r c", c=256))

        # debug outputs
        for nm, src, shp in [("lpn", C["lpn_d"], (3, H, W)),
                             ("y0", C["y"][0], (64, H, W)),
                             ("y1", C["y"][1], (64, H, W)),
                             ("y2", C["y"][2], (64, H, W)),
                             ("y3", C["y"][3], (64, H, W)),
                             ("enc", C["enc_d"], (H, 64, W)),
                             ("y4", C["y"][4], (64, H, W))]:
            ap = self.maybe_debug(nm, shp)
            if ap is not None:
                nc.sync.dma_start(ap[:], src[:])



# ======================================================================
# harness entry point: full inputs in, full outputs out (8 cores SPMD)
# ======================================================================
from concourse.bass_utils import run_bass_kernel_spmd

_ENC = None

def _get_enc():
    global _ENC
    if _ENC is None:
        e = Enc(n_cores=8)
        e.build()
        _ENC = e
    return _ENC

def make_in_maps(inputs):
    consts = host_constants()
    g = lambda k: np.ascontiguousarray(np.asarray(inputs[k], dtype=np.float32))
    image, message = g("image"), g("message")
    shared = dict(
        w0=g("w0"), b0=g("b0").reshape(64, 1), g0=g("g0").reshape(64, 1),
        be0=g("be0").reshape(64, 1), ws=g("ws"), bs=g("bs").reshape(3, 64, 1),
        gs=g("gs").reshape(3, 64, 1), bes=g("bes").reshape(3, 64, 1),
        acw=g("acw"), acb=g("acb").reshape(64, 1), acg=g("acg").reshape(64, 1),
        acbe=g("acbe").reshape(64, 1), fw=np.ascontiguousarray(g("fw")[:, :, 0, 0]),
        fb=g("fb").reshape(3, 1), **consts)
    return [dict(image=np.ascontiguousarray(image[i]),
                 message=np.ascontiguousarray(message[i].reshape(MSG, 1)),
                 **shared) for i in range(8)]

def kernel(**inputs):
    e = _get_enc()
    in_maps = make_in_maps(inputs)
    res = run_bass_kernel_spmd(e.nc, in_maps, core_ids=list(range(8)))
    out = np.stack([res.results[i]["out"] for i in range(8)], axis=0)
    return np.ascontiguousarray(out.astype(np.float32))



# revision 18
# speedup vs baseline: 1.2996x; 1.2996x over previous
"""Encoder kernel: nn_Encoder (watermark encoder) on 8 TRN2 cores,
data-parallel over batch (1 image per core).

Key structure vs the straightforward version:
  - watermark: ifft2(fft2(x).at[pos].set(v)) == x for channels 1..63;
    channel 0 gets x0 + Re(idft(delta)) with delta nonzero on a 9x9
    frequency block -> tiny matmuls instead of a 64-channel FFT chain.
  - convs in bf16 (weights + moving operands) -> FWL weight loads.
  - ac conv: 30 constant message channels folded into the bias with
    edge-mask corrections; image channels packed conv0-style.
Layouts per core:
  y_k DRAM [64, 256, 256] bf16 (pre-BN, conv bias added)
  lpn DRAM [3, 256, 256] bf16  (holds -floor(clip(low,0,255)); conv0
    weights negated/255)
  imgb DRAM [3, 256, 256] bf16 (image cast)
  x0c DRAM [256, 256] bf16     (bn-relu'd x0 + watermark correction)
"""
import numpy as np
import concourse.bass as bass
import concourse.tile as tile
from concourse import bacc, mybir
from contextlib import ExitStack

f32 = mybir.dt.float32
f32r = mybir.dt.float32r
bf = mybir.dt.bfloat16
FT = mybir.ActivationFunctionType
ALU = mybir.AluOpType

H = W = 256
HW = H * W
CH = 64
MSG = 30
WP = W + 2          # padded row length 258
R = 32              # conv band rows
NBAND = H // R      # 8

# ---------------------------------------------------------------- host consts
def host_constants():
    j = np.arange(256)
    ang = 2.0 * np.pi * np.outer(j, j) / 256.0
    C = np.cos(ang).astype(np.float32)
    S = (-np.sin(ang)).astype(np.float32)      # F = C + iS
    Ci = (C / 256.0).astype(np.float32)
    Sq = (S / 256.0).astype(np.float32)
    consts = {
        "cC": C, "cS": S, "cNS": -S, "cCi": Ci, "cSq": Sq, "cNSq": -Sq,
        "cCS": np.hstack([C, S]).astype(np.float32),
        "cIdent": np.eye(128, dtype=np.float32),
    }
    yy = np.arange(H, dtype=np.float32) - H // 2
    xx = np.arange(W, dtype=np.float32) - W // 2
    m = ((yy[:, None] ** 2 + xx[None, :] ** 2) <= float(50 * 50)).astype(np.float32)
    consts["cMask"] = np.fft.ifftshift(m).astype(np.float32)
    # watermark positions (all in channel 0; block kh,kw in 124..132)
    cy = cx = 128
    rr, cc = [], []
    idx = 0
    for i in range(-4, 5):
        for j2 in range(-4, 5):
            if idx >= MSG:
                break
            if (i * i + j2 * j2) ** 0.5 <= 4:
                rr.append(cy + i); cc.append(cx + j2); idx += 1
    rr = np.array(rr); cc = np.array(cc)   # rr = kh (axis -2), cc = kw (axis -1)
    pk = np.zeros((MSG, 9), np.float32)    # kw indicator
    fk = np.zeros((MSG, 9), np.float32)    # kh indicator
    kapT = np.zeros((9, 9), np.float32)    # [kh, kw] position mask
    for k in range(MSG):
        pk[k, cc[k] - 124] = 1.0
        fk[k, rr[k] - 124] = 1.0
        kapT[rr[k] - 124, cc[k] - 124] = 1.0
    consts["cPk"] = pk
    consts["cFk"] = fk
    consts["cKapT"] = kapT
    # wm DFT block matrices (bf16 on host; kh,kw in 124..132)
    import ml_dtypes
    th = 2.0 * np.pi / 256.0
    K9 = np.arange(124, 133, dtype=np.float64)
    hh = np.arange(256, dtype=np.float64)
    A = th * np.outer(hh, K9)            # [256, 9] angle(h, k)
    cosA, sinA = np.cos(A), np.sin(A)
    bfc = lambda x: np.ascontiguousarray(x).astype(ml_dtypes.bfloat16)
    consts["cE1"] = bfc(np.hstack([cosA, -sinA]))            # [256,18] fwd rows
    consts["cE2"] = bfc(np.hstack([cosA, sinA]))             # [256,18] fwd cols
    consts["cE3"] = bfc(np.hstack([cosA, -sinA]).T)          # [18,256] inv M_re
    consts["cE4"] = bfc(np.hstack([sinA, cosA]).T)           # [18,256] inv M_im
    consts["cE5"] = bfc(np.hstack([cosA / 65536.0, -sinA / 65536.0]).T)
    return consts


# ---------------------------------------------------------------- builders
class Enc:
    def __init__(self, n_cores=8, debug_outs=()):
        self.n_cores = n_cores
        self.ntot = float(n_cores * HW)
        self.debug_outs = debug_outs
        nc = bacc.Bacc("TRN2", target_bir_lowering=False, debug=False,
                       num_devices=n_cores)
        self.nc = nc
        d = {}
        d["image"] = nc.dram_tensor("image", (3, H, W), f32, kind="ExternalInput").ap()
        d["message"] = nc.dram_tensor("message", (MSG, 1), f32, kind="ExternalInput").ap()
        d["w0"] = nc.dram_tensor("w0", (64, 3, 3, 3), f32, kind="ExternalInput").ap()
        d["b0"] = nc.dram_tensor("b0", (64, 1), f32, kind="ExternalInput").ap()
        d["g0"] = nc.dram_tensor("g0", (64, 1), f32, kind="ExternalInput").ap()
        d["be0"] = nc.dram_tensor("be0", (64, 1), f32, kind="ExternalInput").ap()
        d["ws"] = nc.dram_tensor("ws", (3, 64, 64, 3, 3), f32, kind="ExternalInput").ap()
        d["bs"] = nc.dram_tensor("bs", (3, 64, 1), f32, kind="ExternalInput").ap()
        d["gs"] = nc.dram_tensor("gs", (3, 64, 1), f32, kind="ExternalInput").ap()
        d["bes"] = nc.dram_tensor("bes", (3, 64, 1), f32, kind="ExternalInput").ap()
        d["acw"] = nc.dram_tensor("acw", (64, 97, 3, 3), f32, kind="ExternalInput").ap()
        d["acb"] = nc.dram_tensor("acb", (64, 1), f32, kind="ExternalInput").ap()
        d["acg"] = nc.dram_tensor("acg", (64, 1), f32, kind="ExternalInput").ap()
        d["acbe"] = nc.dram_tensor("acbe", (64, 1), f32, kind="ExternalInput").ap()
        d["fw"] = nc.dram_tensor("fw", (3, 64), f32, kind="ExternalInput").ap()
        d["fb"] = nc.dram_tensor("fb", (3, 1), f32, kind="ExternalInput").ap()
        for k, shp, dt in [("cC", (256, 256), f32), ("cS", (256, 256), f32),
                           ("cNS", (256, 256), f32), ("cCi", (256, 256), f32),
                           ("cSq", (256, 256), f32), ("cNSq", (256, 256), f32),
                           ("cCS", (256, 512), f32), ("cIdent", (128, 128), f32),
                           ("cMask", (256, 256), f32),
                           ("cPk", (MSG, 9), f32), ("cFk", (MSG, 9), f32),
                           ("cKapT", (9, 9), f32),
                           ("cE1", (256, 18), bf), ("cE2", (256, 18), bf),
                           ("cE3", (18, 256), bf), ("cE4", (18, 256), bf),
                           ("cE5", (18, 256), bf)]:
            d[k] = nc.dram_tensor(k, shp, dt, kind="ExternalInput").ap()
        d["out"] = nc.dram_tensor("out", (3, H, W), f32, kind="ExternalOutput").ap()
        self.d = d
        self.dbg = {}

    def maybe_debug(self, name, shape, dt=f32):
        if name in self.debug_outs:
            self.dbg[name] = self.nc.dram_tensor(
                "dbg_" + name, shape, dt, kind="ExternalOutput").ap()
            return self.dbg[name]
        return None

    # ------------------------------------------------------------------
    def build(self):
        nc, d = self.nc, self.d
        with tile.TileContext(nc) as tc, ExitStack() as X:
            cp = X.enter_context(tc.tile_pool(name="consts", bufs=1))
            dp = X.enter_context(tc.tile_pool(name="dram", bufs=1, space="DRAM"))
            pwX = ExitStack()
            pw = pwX.enter_context(tc.tile_pool(name="pw", bufs=2, space="PSUM"))
            wev = pwX.enter_context(tc.tile_pool(name="wev", bufs=2))

            # ---------------- DRAM scratch
            y = [dp.tile([64, H, W], bf, name=f"yact{k}") for k in range(5)]
            lpn_d = dp.tile([3, H, W], bf, name="lpn_d")
            imgb_d = dp.tile([3, H, W], bf, name="imgb_d")
            x0c_d = dp.tile([H, W], bf, name="x0c_d")
            cl_in = [dp.tile([64, 2], f32, name=f"clin{k}") for k in range(5)]
            cl_out = [dp.tile([64, 2], f32, name=f"clout{k}", addr_space="Shared")
                      for k in range(5)]

            # ---------------- constants into SBUF
            def cload(name, src, shape, dtype):
                t = cp.tile(shape, dtype, name=name)
                nc.sync.dma_start(t[:], src)
                return t
            DF = {}
            for nm, wdt in [("cC", 256), ("cS", 256), ("cNS", 256), ("cCi", 256),
                            ("cSq", 256), ("cNSq", 256), ("cCS", 512)]:
                DF[nm + "_hi"] = cload(nm + "_hi", d[nm][0:128, :].bitcast(f32r),
                                       [128, wdt], f32r)
                DF[nm + "_lo"] = cload(nm + "_lo", d[nm][128:256, :].bitcast(f32r),
                                       [128, wdt], f32r)
            ident = cload("ident", d["cIdent"][:], [128, 128], f32)
            maskt = [cload(f"maskt{i}", d["cMask"][i * 128:(i + 1) * 128, :],
                           [128, 256], f32) for i in range(2)]
            pk_t = cload("pk_t", d["cPk"][:], [MSG, 9], f32)
            fk_t = cload("fk_t", d["cFk"][:], [MSG, 9], f32)
            kapT_t = cload("kapT_t", d["cKapT"][:], [9, 9], f32)
            msg_t = cload("msg_t", d["message"][:], [MSG, 1], f32)
            cE1 = [cload(f"cE1_{i}", d["cE1"][i * 128:(i + 1) * 128, :], [128, 18], bf)
                   for i in range(2)]
            cE2 = [cload(f"cE2_{i}", d["cE2"][i * 128:(i + 1) * 128, :], [128, 18], bf)
                   for i in range(2)]
            cE3 = cload("cE3", d["cE3"][:], [18, 256], bf)
            cE4 = cload("cE4", d["cE4"][:], [18, 256], bf)
            cE5 = cload("cE5", d["cE5"][:], [18, 256], bf)
            eps64 = cp.tile([64, 1], f32, name="eps64")
            nc.vector.memset(eps64[:], 1e-5)
            zero128 = cp.tile([128, 1], f32, name="zero128")
            nc.vector.memset(zero128[:], 0.0)
            ones1 = cp.tile([1, 128], f32, name="ones1")
            nc.vector.memset(ones1[:], 1.0)
            msg_b = cp.tile([MSG, 1], bf, name="msg_b")
            nc.vector.tensor_copy(msg_b[:], msg_t[:])

            # per-layer bn param tiles
            def vload(name, src):
                t = cp.tile([64, 1], f32, name=name)
                nc.sync.dma_start(t[:], src)
                return t
            g_t = [vload("g_t0", d["g0"][:])] + \
                  [vload(f"g_t{k+1}", d["gs"][k]) for k in range(3)] + \
                  [vload("g_t4", d["acg"][:])]
            be_t = [vload("be_t0", d["be0"][:])] + \
                   [vload(f"be_t{k+1}", d["bes"][k]) for k in range(3)] + \
                   [vload("be_t4", d["acbe"][:])]
            acb_t = vload("acb_t", d["acb"][:])
            # conv bias replicated to 128 partitions; b128[4] (ac) filled later
            b128 = []
            for k, src in enumerate([d["b0"], d["bs"][0], d["bs"][1], d["bs"][2]]):
                t = cp.tile([128, 1], f32, name=f"b128_{k}")
                nc.sync.dma_start(t[0:64, :], src)
                nc.sync.dma_start(t[64:128, :], src)
                b128.append(t)
            b128.append(cp.tile([128, 1], f32, name="b128_4"))
            fb_t = cp.tile([3, 1], f32, name="fb_t")
            nc.sync.dma_start(fb_t[:], d["fb"][:])

            # ---------------- weight transposes (lhsT prep), all bf16
            # conv1..3: pair lhsT [128,128] x3(dw), cross lhsT [64,128] x3
            lhsT_pair, lhsT_sing = [], []
            for k in range(3):
                wsrc = cp.tile([64, 64, 9], f32, name=f"wsrc{k}")
                nc.sync.dma_start(wsrc[:], d["ws"][k].rearrange("o i a b -> o i (a b)"))
                pairs, sings = self._make_pairs64(pw, cp, wsrc, ident, f"c{k}")
                lhsT_pair.append(pairs); lhsT_sing.append(sings)
            # ac conv enc part (input ch 30:94) -> same pair structure
            acsrc = cp.tile([64, 97, 9], f32, name="acsrc")
            nc.sync.dma_start(acsrc[:], d["acw"][:].rearrange("o i a b -> o i (a b)"))
            ac_pairs, ac_sings = self._make_pairs64(
                pw, cp, acsrc, ident, "ac", in_off=30)
            # full acw transposes for aux/msg handling: acT_sb[tap] [97,64] bf16
            acT_sb = []
            for tap in range(9):
                p = pw.tile([97, 64], f32, name="pwa", bufs=2)
                nc.tensor.transpose(p[:], acsrc[:, :, tap], ident[0:64, 0:64])
                t = cp.tile([97, 64], bf, name=f"acT{tap}")
                nc.vector.tensor_copy(t[:], p[:])
                acT_sb.append(t)
            # msg-fold: M9[o, tap] = sum_c acw[o, c<30, tap] * msg[c]
            pM9 = pw.tile([64, 9], f32, name="pM9", bufs=1)
            for tap in range(9):
                nc.tensor.matmul(pM9[:, tap:tap + 1], acT_sb[tap][0:30, :],
                                 msg_b[:], start=True, stop=True)
            M9sb = cp.tile([64, 9], f32, name="M9sb")
            nc.vector.tensor_copy(M9sb[:], pM9[:])
            # bias_eff = acb + sum_j M9[:, j]  -> b128[4]
            bsum = wev.tile([64, 1], f32, name="bsum")
            nc.vector.tensor_reduce(bsum[:], M9sb[:], axis=mybir.AxisListType.X,
                                    op=ALU.add)
            beff = cp.tile([64, 1], f32, name="beff")
            nc.vector.tensor_tensor(beff[:], acb_t[:], bsum[:], op=ALU.add)
            nc.sync.dma_start(b128[4][0:64, :], beff[:])
            nc.sync.dma_start(b128[4][64:128, :], beff[:])
            # edge sums: S_top=j0:3, S_bot=j6:9, S_left=j{0,3,6}, S_right=j{2,5,8}
            sb_top = cp.tile([64, 1], f32, name="sb_top")
            nc.vector.tensor_reduce(sb_top[:], M9sb[:, 0:3], axis=mybir.AxisListType.X,
                                    op=ALU.add)
            sb_bot = cp.tile([64, 1], f32, name="sb_bot")
            nc.vector.tensor_reduce(sb_bot[:], M9sb[:, 6:9], axis=mybir.AxisListType.X,
                                    op=ALU.add)
            s_lr = []
            for nm, js in [("sl", (0, 3, 6)), ("sr", (2, 5, 8))]:
                t0 = wev.tile([64, 1], f32, name=nm + "a")
                nc.vector.tensor_tensor(t0[:], M9sb[:, js[0]:js[0] + 1],
                                        M9sb[:, js[1]:js[1] + 1], op=ALU.add)
                t1 = cp.tile([64, 1], f32, name=nm)
                nc.vector.tensor_tensor(t1[:], t0[:], M9sb[:, js[2]:js[2] + 1],
                                        op=ALU.add)
                s_lr.append(t1)
            # bottom fixups need partition base 64: fixB [128,3]
            fixB = cp.tile([128, 3], f32, name="fixB")
            nc.sync.dma_start(fixB[64:128, 0:1], sb_bot[:])
            nc.sync.dma_start(fixB[64:128, 1:2], M9sb[:, 6:7])
            nc.sync.dma_start(fixB[64:128, 2:3], M9sb[:, 8:9])
            # aux lhsT [17,128] x3(dw): rows 0:15 img (sigma,ch), 15:17 -S_l/-S_r
            lhsT_aux = []
            for dw in range(3):
                la = cp.tile([17, 128], bf, name=f"lhsTaux{dw}")
                nc.vector.memset(la[:], 0.0)
                lhsT_aux.append(la)
            for dw in range(3):
                for sg in range(3):       # g0: tap dh=sg
                    nc.sync.dma_start(lhsT_aux[dw][sg * 3:sg * 3 + 3, 0:64],
                                      acT_sb[sg * 3 + dw][94:97, :])
                for sg in range(2, 5):    # g1: tap dh=sg-2
                    nc.sync.dma_start(lhsT_aux[dw][sg * 3:sg * 3 + 3, 64:128],
                                      acT_sb[(sg - 2) * 3 + dw][94:97, :])
            # mask rows: transpose S vectors to rows, negate, bf16
            for i, sv in enumerate(s_lr):
                prow = pw.tile([1, 64], f32, name="prow", bufs=1)
                nc.tensor.transpose(prow[:], sv[:], ident[0:64, 0:64])
                rowneg = wev.tile([1, 64], bf, name="rowneg")
                nc.vector.tensor_scalar_mul(rowneg[:], prow[:], -1.0)
                nc.sync.dma_start(lhsT_aux[1][15 + i:16 + i, 0:64], rowneg[:])
                nc.sync.dma_start(lhsT_aux[1][15 + i:16 + i, 64:128], rowneg[:])
            # conv0: lhsT0x[dw] [15,128] = -w0^T/255, (sigma,ch) rows
            w0src = cp.tile([64, 3, 9], f32, name="w0src")
            nc.sync.dma_start(w0src[:], d["w0"][:].rearrange("o i a b -> o i (a b)"))
            lhsT0 = []
            for dw in range(3):
                l0 = cp.tile([15, 128], bf, name=f"lhsT0{dw}")
                nc.vector.memset(l0[:], 0.0)
                lhsT0.append(l0)
            for dw in range(3):
                for dh in range(3):
                    p = pw.tile([64, 64], f32, name="pwt")
                    nc.tensor.transpose(p[0:3, :], w0src[:, :, dh * 3 + dw],
                                        ident[0:64, 0:64])
                    tmp0 = wev.tile([3, 64], bf, name="w0tmp")
                    nc.vector.tensor_scalar_mul(tmp0[:], p[0:3, :], -1.0 / 255.0)
                    nc.sync.dma_start(lhsT0[dw][dh * 3:dh * 3 + 3, 0:64], tmp0[:])
                    nc.sync.dma_start(lhsT0[dw][(dh + 2) * 3:(dh + 2) * 3 + 3, 64:128],
                                      tmp0[:])
            # final 1x1: lhsT_fin [128,6]
            fwsrc = cp.tile([3, 64], f32, name="fwsrc")
            nc.sync.dma_start(fwsrc[:], d["fw"][:])
            lhsT_fin = cp.tile([128, 6], bf, name="lhsT_fin")
            nc.vector.memset(lhsT_fin[:], 0.0)
            p = pw.tile([64, 64], f32, name="pwt")
            nc.tensor.transpose(p[:, 0:3], fwsrc[:], ident[0:3, 0:3])
            pbf = wev.tile([64, 3], bf, name="pbf")
            nc.vector.tensor_copy(pbf[:], p[:, 0:3])
            nc.sync.dma_start(lhsT_fin[0:64, 0:3], pbf[:])
            nc.sync.dma_start(lhsT_fin[64:128, 3:6], pbf[:])

            # watermark grid wmvT [kh, kw]: fk^T @ (pk * msg)
            msgc2 = cp.tile([MSG, 9], f32, name="msgc2")
            nc.vector.tensor_scalar(msgc2[:], pk_t[:], msg_t[:, 0:1], None,
                                    op0=ALU.mult)
            pwm = pw.tile([9, 9], f32, name="pwm", bufs=1)
            nc.tensor.matmul(pwm[:], fk_t[:], msgc2[:], start=True, stop=True)
            wmvT = cp.tile([9, 9], f32, name="wmvT")
            nc.vector.tensor_copy(wmvT[:], pwm[:])

            # ac aux band tiles (manual double-buffer) + mask columns.
            # Engine ops need 32-aligned partition bases; masks and odd-offset
            # zero fills go through base-0 staging tiles + DMA.
            ml_t = cp.tile([1, R + 2, WP], bf, name="ml_t")
            nc.vector.memset(ml_t[:].rearrange("p r c -> p (r c)"), 0.0)
            nc.vector.memset(ml_t[0:1, :, 1:2], 1.0)
            mr_t = cp.tile([1, R + 2, WP], bf, name="mr_t")
            nc.vector.memset(mr_t[:].rearrange("p r c -> p (r c)"), 0.0)
            nc.vector.memset(mr_t[0:1, :, 256:257], 1.0)
            zrow = cp.tile([3, 1, WP], bf, name="zrow")
            nc.vector.memset(zrow[:].rearrange("p r c -> p (r c)"), 0.0)
            aux_bufs = [cp.tile([17, R + 2, WP], bf, name=f"auxb{i}")
                        for i in range(2)]
            for t in aux_bufs:
                nc.vector.memset(t[:].rearrange("p r c -> p (r c)"), 0.0)
                nc.sync.dma_start(t[15:16, :, :], ml_t[:])
                nc.sync.dma_start(t[16:17, :, :], mr_t[:])

            self._consts = dict(DF=DF, ident=ident, maskt=maskt,
                                g_t=g_t, be_t=be_t, b128=b128, fb_t=fb_t,
                                lhsT_pair=lhsT_pair, lhsT_sing=lhsT_sing,
                                lhsT0=lhsT0, ac_pairs=ac_pairs, ac_sings=ac_sings,
                                lhsT_aux=lhsT_aux, lhsT_fin=lhsT_fin,
                                cE1=cE1, cE2=cE2, cE3=cE3, cE4=cE4, cE5=cE5,
                                kapT=kapT_t, wmvT=wmvT, ones1=ones1,
                                sb_top=sb_top, fixB=fixB, M9sb=M9sb,
                                eps64=eps64, zero128=zero128, cp=cp, zrow=zrow,
                                y=y, lpn_d=lpn_d, imgb_d=imgb_d, x0c_d=x0c_d,
                                aux_bufs=aux_bufs,
                                cl_in=cl_in, cl_out=cl_out)
            pwX.close()
            self._build_body(X, tc)
        self.nc.compile()

    # ------------------------------------------------------------------
    def _make_pairs64(self, pw, cp, wsrc, ident, tag, in_off=0):
        """Pair/cross lhsT tiles (bf16) for a 64->64 3x3 conv.
        wsrc [64, >=in_off+64, 9] f32 (o, i, tap)."""
        nc = self.nc
        pairs, sings = [], []
        for dw in range(3):
            pA = cp.tile([128, 128], bf, name=f"lTpA{tag}{dw}")
            pB = cp.tile([128, 128], bf, name=f"lTpB{tag}{dw}")
            cx = cp.tile([64, 128], bf, name=f"lTcx{tag}{dw}")
            nc.vector.memset(pA[:], 0.0)
            nc.vector.memset(pB[:], 0.0)
            tp3 = []
            for dh in range(3):
                p = pw.tile([64, 64], f32, name="pwt")
                nc.tensor.transpose(p[:], wsrc[:, in_off:in_off + 64, dh * 3 + dw],
                                    ident[0:64, 0:64])
                tp3.append(p)
            nc.vector.tensor_copy(pA[0:64, 0:64], tp3[0][:])
            nc.vector.tensor_copy(pA[64:128, 0:64], tp3[1][:])
            nc.vector.tensor_copy(pB[0:64, 64:128], tp3[1][:])
            nc.vector.tensor_copy(pB[64:128, 64:128], tp3[2][:])
            nc.vector.tensor_copy(cx[:, 0:64], tp3[2][:])
            nc.vector.tensor_copy(cx[:, 64:128], tp3[0][:])
            pairs.append((pA, pB)); sings.append(cx)
        return pairs, sings

    # ------------------------------------------------------------------
    def _stats_finalize(self, tc, pool, layer, ssum_cols, sqsum_cols, ncols):
        nc = self.nc
        C = self._consts
        red = pool.tile([128, 2], f32, name=f"red{layer}")
        nc.vector.tensor_reduce(red[:, 0:1], ssum_cols[:, 0:ncols],
                                axis=mybir.AxisListType.X, op=ALU.add)
        nc.vector.tensor_reduce(red[:, 1:2], sqsum_cols[:, 0:ncols],
                                axis=mybir.AxisListType.X, op=ALU.add)
        upper = pool.tile([64, 2], f32, name=f"upper{layer}")
        nc.sync.dma_start(upper[:], red[64:128, :])
        stats = pool.tile([64, 2], f32, name=f"stats{layer}")
        nc.vector.tensor_add(stats[:], red[0:64, :], upper[:])
        nc.sync.dma_start(C["cl_in"][layer][:], stats[:])
        nc.gpsimd.collective_compute(
            "AllReduce", ALU.add,
            replica_groups=[list(range(self.n_cores))],
            ins=[C["cl_in"][layer].opt()], outs=[C["cl_out"][layer].opt()])
        sr = pool.tile([64, 2], f32, name=f"sr{layer}")
        nc.sync.dma_start(sr[:], C["cl_out"][layer][:])
        mean = pool.tile([64, 1], f32, name=f"mean{layer}")
        nc.vector.tensor_scalar_mul(mean[:], sr[:, 0:1], 1.0 / self.ntot)
        ms = pool.tile([64, 1], f32, name=f"ms{layer}")
        nc.vector.tensor_scalar_mul(ms[:], sr[:, 1:2], 1.0 / self.ntot)
        msq = pool.tile([64, 1], f32, name=f"msq{layer}")
        nc.vector.tensor_scalar(msq[:], mean[:], mean[:, 0:1], None, op0=ALU.mult)
        var = pool.tile([64, 1], f32, name=f"var{layer}")
        nc.vector.tensor_scalar(var[:], ms[:], msq[:, 0:1], None, op0=ALU.subtract)
        std = pool.tile([64, 1], f32, name=f"std{layer}")
        nc.scalar.activation(std[:], var[:], FT.Sqrt, bias=C["eps64"][:, 0:1],
                             scale=1.0)
        istd = pool.tile([64, 1], f32, name=f"istd{layer}")
        nc.vector.reciprocal(istd[:], std[:])
        scale = pool.tile([64, 1], f32, name=f"scale{layer}")
        nc.vector.tensor_tensor(scale[:], C["g_t"][layer][:], istd[:], op=ALU.mult)
        prod = pool.tile([64, 1], f32, name=f"prod{layer}")
        nc.vector.tensor_tensor(prod[:], mean[:], scale[:], op=ALU.mult)
        shift = pool.tile([64, 1], f32, name=f"shift{layer}")
        nc.vector.scalar_tensor_tensor(shift[:], prod[:], -1.0, C["be_t"][layer][:],
                                       op0=ALU.mult, op1=ALU.add)
        ap = self.maybe_debug(f"sc{layer}", (64, 1))
        if ap is not None:
            nc.sync.dma_start(ap[:], scale[:])
        ap = self.maybe_debug(f"sh{layer}", (64, 1))
        if ap is not None:
            nc.sync.dma_start(ap[:], shift[:])
        return scale, shift

    # ------------------------------------------------------------------
    def _evict(self, pp_t, ep, sp_cols, layer, cidx, dst, q):
        """PSUM chunk -> +bias (scalar, ssum accum) -> bf16 -> DRAM;
        sqsum via vector tensor_tensor_reduce."""
        nc = self.nc
        C = self._consts
        ysb = ep.tile([128, 512], bf, name=f"ysb{layer}")
        nc.vector.tensor_scalar(ysb[:], pp_t[:], C["b128"][layer][:, 0:1],
                                0.0, op0=ALU.add, op1=ALU.add,
                                accum_out=sp_cols[0][:, cidx:cidx + 1])
        scr = ep.tile([128, 512], bf, name=f"sqscr{layer}")
        nc.scalar.activation(scr[:], ysb[:], FT.Square,
                             bias=C["zero128"][:, 0:1],
                             accum_out=sp_cols[1][:, cidx:cidx + 1])
        nc.sync.dma_start(dst[:, q:q + 2, :],
                          ysb[0:64, :].rearrange("p (r c) -> p r c", c=256))
        nc.sync.dma_start(dst[:, q + 2:q + 4, :],
                          ysb[64:128, :].rearrange("p (r c) -> p r c", c=256))

    # ------------------------------------------------------------------
    def _conv64(self, X, tc, layer, src, dst, scale, shift):
        """conv layers 1..3 (64->64) and 4 (ac: 64 enc + aux)."""
        nc = self.nc
        C = self._consts
        is_ac = (layer == 4)
        pairs = C["ac_pairs"] if is_ac else C["lhsT_pair"][layer - 1]
        sings = C["ac_sings"] if is_ac else C["lhsT_sing"][layer - 1]
        with ExitStack() as S:
            bp = S.enter_context(tc.tile_pool(name=f"band{layer}", bufs=3))
            pp = S.enter_context(tc.tile_pool(name=f"psum{layer}", bufs=4,
                                              space="PSUM"))
            ep = S.enter_context(tc.tile_pool(name=f"evict{layer}", bufs=3))
            sp = S.enter_context(tc.tile_pool(name=f"stat{layer}", bufs=1))
            ssum_cols = sp.tile([128, 64], f32, name=f"ssc{layer}")
            sqsum_cols = sp.tile([128, 64], f32, name=f"sqc{layer}")
            cidx = 0
            for bi in range(NBAND):
                r0 = bi * R
                band = bp.tile([128, (R + 2) * WP], bf, name=f"bandt{layer}")
                b3 = band[:].rearrange("p (r c) -> p r c", c=WP)
                i0 = 1 if bi == 0 else 0
                i1 = R + 1 if bi == NBAND - 1 else R + 2
                rl, rh = r0 - 1 + i0, r0 - 1 + i1
                nc.vector.memset(b3[0:64, :, 0:1], 0.0)
                nc.vector.memset(b3[0:64, :, 257:258], 0.0)
                if bi == 0:
                    nc.vector.memset(b3[0:64, 0:1, :], 0.0)
                if bi == NBAND - 1:
                    nc.vector.memset(b3[0:64, R + 1:R + 2, :], 0.0)
                nc.sync.dma_start(b3[0:64, i0:i1, 1:257], src[:, rl:rh, :])
                nc.scalar.activation(b3[0:64, i0:i1, 1:257],
                                     b3[0:64, i0:i1, 1:257],
                                     FT.Relu, bias=shift[:, 0:1],
                                     scale=scale[:, 0:1])
                if is_ac:
                    # ch0 = x0corr (already bn-relu'd + wm corr): overwrite
                    nc.sync.dma_start(b3[0:1, i0:i1, 1:257], C["x0c_d"][rl:rh, :])
                nc.sync.dma_start(b3[64:128, 0:R + 1, :], b3[0:64, 1:R + 2, :])
                if is_ac:
                    aux = C["aux_bufs"][bi % 2]
                    # img partitions (sigma,ch): band row i <- img row r0-1+i+sg
                    for sg in range(5):
                        a0 = 1 if (bi == 0 and sg == 0) else 0
                        a1 = min(30, 257 - r0 - sg)
                        if a1 <= a0:
                            continue
                        nc.sync.dma_start(
                            aux[sg * 3:sg * 3 + 3, a0:a1, 1:257],
                            C["imgb_d"][:, r0 - 1 + a0 + sg:r0 - 1 + a1 + sg, :])
                    if bi == NBAND - 1:
                        nc.sync.dma_start(aux[12:15, 29:30, :], C["zrow"][:])
                for t in range(0, R, 4):
                    pt = pp.tile([128, 512], f32, name=f"pchunk{layer}")
                    nmm = 12 if is_ac else 9
                    mm = 0
                    for dw in range(3):
                        pA, pB = pairs[dw]
                        nc.tensor.matmul(
                            pt[:], pA[:], b3[0:128, t:t + 2, dw:dw + 256],
                            start=(mm == 0), stop=(mm == nmm - 1)); mm += 1
                        nc.tensor.matmul(
                            pt[:], pB[:], b3[0:128, t + 3:t + 5, dw:dw + 256],
                            start=(mm == 0), stop=(mm == nmm - 1)); mm += 1
                        nc.tensor.matmul(
                            pt[:], sings[dw][:], b3[0:64, t + 2:t + 4, dw:dw + 256],
                            start=(mm == 0), stop=(mm == nmm - 1)); mm += 1
                        if is_ac:
                            np_aux = 17 if dw == 1 else 15
                            nc.tensor.matmul(
                                pt[:], C["lhsT_aux"][dw][0:np_aux, :],
                                aux[0:np_aux, t:t + 2, dw:dw + 256],
                                start=(mm == 0), stop=(mm == nmm - 1)); mm += 1
                    if is_ac and bi == 0 and t == 0:
                        # row 0 fixups: -S_top everywhere, +A_tl/+A_tr corners
                        nc.vector.tensor_scalar(pt[0:64, 0:256], pt[0:64, 0:256],
                                                C["sb_top"][:, 0:1], None,
                                                op0=ALU.subtract)
                        nc.vector.tensor_tensor(pt[0:64, 0:1], pt[0:64, 0:1],
                                                C["M9sb"][:, 0:1], op=ALU.add)
                        nc.vector.tensor_tensor(pt[0:64, 255:256], pt[0:64, 255:256],
                                                C["M9sb"][:, 2:3], op=ALU.add)
                    if is_ac and bi == NBAND - 1 and t == R - 4:
                        nc.vector.tensor_scalar(pt[64:128, 256:512],
                                                pt[64:128, 256:512],
                                                C["fixB"][64:128, 0:1], None,
                                                op0=ALU.subtract)
                        nc.vector.tensor_tensor(pt[64:128, 256:257],
                                                pt[64:128, 256:257],
                                                C["fixB"][64:128, 1:2], op=ALU.add)
                        nc.vector.tensor_tensor(pt[64:128, 511:512],
                                                pt[64:128, 511:512],
                                                C["fixB"][64:128, 2:3], op=ALU.add)
                    self._evict(pt, ep, (ssum_cols, sqsum_cols), layer, cidx,
                                dst, r0 + t)
                    cidx += 1
            return self._stats_finalize(tc, sp, layer, ssum_cols, sqsum_cols, cidx)

    # ------------------------------------------------------------------
    def _conv0(self, X, tc):
        """conv0: lpn(3ch, negated/255) -> y0. (sigma,ch) 15-partition bands."""
        nc = self.nc
        C = self._consts
        with ExitStack() as S:
            bp = S.enter_context(tc.tile_pool(name="band0", bufs=3))
            pp = S.enter_context(tc.tile_pool(name="psum0", bufs=4, space="PSUM"))
            ep = S.enter_context(tc.tile_pool(name="evict0", bufs=3))
            sp = S.enter_context(tc.tile_pool(name="stat0", bufs=1))
            ssum_cols = sp.tile([128, 64], f32, name="ssc0")
            sqsum_cols = sp.tile([128, 64], f32, name="sqc0")
            cidx = 0
            for bi in range(NBAND):
                r0 = bi * R
                band = bp.tile([15, (R + 2) * WP], bf, name="bandt0")
                b3 = band[:].rearrange("p (r c) -> p r c", c=WP)
                nc.vector.memset(b3[:, :, 0:1], 0.0)
                nc.vector.memset(b3[:, :, 257:258], 0.0)
                if bi == 0:
                    nc.vector.memset(b3[0:3, 0:1, :], 0.0)
                if bi == NBAND - 1:
                    nc.sync.dma_start(b3[12:15, 29:30, :], C["zrow"][:])
                for sg in range(5):
                    a0 = 1 if (bi == 0 and sg == 0) else 0
                    a1 = min(30, 256 - r0 - sg + 1)
                    if a1 <= a0:
                        continue
                    nc.sync.dma_start(
                        b3[sg * 3:sg * 3 + 3, a0:a1, 1:257],
                        C["lpn_d"][:, r0 - 1 + a0 + sg:r0 - 1 + a1 + sg, :])
                for t in range(0, R, 4):
                    pt = pp.tile([128, 512], f32, name="pchunk0")
                    for dw in range(3):
                        nc.tensor.matmul(
                            pt[:], C["lhsT0"][dw][:],
                            b3[0:15, t:t + 2, dw:dw + 256],
                            start=(dw == 0), stop=(dw == 2))
                    self._evict(pt, ep, (ssum_cols, sqsum_cols), 0, cidx,
                                C["y"][0], r0 + t)
                    cidx += 1
            return self._stats_finalize(tc, sp, 0, ssum_cols, sqsum_cols, cidx)

    # ------------------------------------------------------------------
    def _fft_chain(self, tc, pools, *, src_loader, mask=False, evict_fn=None):
        """fft2 -> mask -> ifft2 for ONE channel (lowpass), f32r tiles."""
        nc = self.nc
        C = self._consts
        DF = C["DF"]
        xp, zp, pp = pools

        def LT(nm, chunk):
            return DF[nm + ("_hi" if chunk == 0 else "_lo")]

        xt = [xp.tile([128, 256], f32r, name="fft_xt") for _ in range(2)]
        for hbl in range(2):
            src_loader(xt[hbl], hbl)
        Zt = []
        for wbl in range(2):
            pZ = pp.tile([128, 512], f32, name="fft_ps", bufs=5)
            for ch in range(2):
                nc.tensor.matmul(pZ[:], xt[ch][:, wbl * 128:(wbl + 1) * 128],
                                 LT("cCS", ch)[:], start=(ch == 0), stop=(ch == 1))
            zt = zp.tile([128, 512], f32r, name="fft_zt")
            nc.vector.tensor_copy(zt[:], pZ[:])
            Zt.append(zt)
        fsb = []
        for kwbl in range(2):
            pf = pp.tile([128, 512], f32, name="fft_ps", bufs=5)
            sl = slice(kwbl * 128, (kwbl + 1) * 128)
            for ch in range(2):
                nc.tensor.matmul(pf[:], LT("cC", ch)[:, sl], Zt[ch][:],
                                 start=(ch == 0), stop=False)
            for ch in range(2):
                nc.tensor.matmul(pf[:, 0:256], LT("cNS", ch)[:, sl],
                                 Zt[ch][:, 256:512], start=False, stop=False)
            for ch in range(2):
                nc.tensor.matmul(pf[:, 256:512], LT("cS", ch)[:, sl],
                                 Zt[ch][:, 0:256], start=False, stop=(ch == 1))
            ft = zp.tile([128, 512], f32r, name="fft_ft")
            if mask:
                nc.vector.tensor_tensor(ft[:, 0:256], pf[:, 0:256],
                                        C["maskt"][kwbl][:], op=ALU.mult)
                nc.vector.tensor_tensor(ft[:, 256:512], pf[:, 256:512],
                                        C["maskt"][kwbl][:], op=ALU.mult)
            else:
                nc.vector.tensor_copy(ft[:], pf[:])
            fsb.append(ft)
        Gt = []
        for khbl in range(2):
            pG = pp.tile([128, 512], f32, name="fft_ps", bufs=5)
            sl = slice(khbl * 128, (khbl + 1) * 128)
            for ch in range(2):
                nc.tensor.matmul(pG[:, 0:256], fsb[ch][:, sl],
                                 LT("cCi", ch)[:], start=(ch == 0), stop=False)
            for ch in range(2):
                nc.tensor.matmul(pG[:, 0:256],
                                 fsb[ch][:, 256 + khbl * 128:256 + (khbl + 1) * 128],
                                 LT("cSq", ch)[:], start=False, stop=False)
            for ch in range(2):
                nc.tensor.matmul(pG[:, 256:512],
                                 fsb[ch][:, 256 + khbl * 128:256 + (khbl + 1) * 128],
                                 LT("cCi", ch)[:], start=(ch == 0), stop=False)
            for ch in range(2):
                nc.tensor.matmul(pG[:, 256:512], fsb[ch][:, sl],
                                 LT("cNSq", ch)[:], start=False, stop=(ch == 1))
            gt = zp.tile([128, 512], f32r, name="fft_gt")
            nc.vector.tensor_copy(gt[:], pG[:])
            Gt.append(gt)
        for hbl in range(2):
            pE = pp.tile([128, 256], f32, name="fft_pe", bufs=2)
            sl = slice(hbl * 128, (hbl + 1) * 128)
            for ch in range(2):
                nc.tensor.matmul(pE[:], LT("cCi", ch)[:, sl], Gt[ch][:, 0:256],
                                 start=(ch == 0), stop=False)
            for ch in range(2):
                nc.tensor.matmul(pE[:], LT("cSq", ch)[:, sl], Gt[ch][:, 256:512],
                                 start=False, stop=(ch == 1))
            esb = zp.tile([128, 256], f32, name="fft_esb")
            nc.vector.tensor_copy(esb[:], pE[:])
            evict_fn(esb, hbl)

    # ------------------------------------------------------------------
    def _wm_correction(self, tc, sc3, sh3):
        """x0 = relu(bn3(y3[0])); F0 = 9x9 DFT block of x0;
        delta = (wmvT - F0)*kapT; corr = Re(idft(delta))/N^2;
        x0c_d = x0 + corr."""
        nc = self.nc
        C = self._consts
        with ExitStack() as S:
            wp = S.enter_context(tc.tile_pool(name="wm", bufs=1))
            wpp = S.enter_context(tc.tile_pool(name="wm_ps", bufs=1, space="PSUM"))
            # broadcast sc3[0]/sh3[0] to 128 partitions via ones-matmul
            bc = []
            for vi, vec in enumerate((sc3, sh3)):
                prow = wpp.tile([1, 64], f32, name="wm_tr")
                nc.tensor.transpose(prow[:], vec[:], C["ident"][0:64, 0:64])
                row = wp.tile([1, 64], f32, name=f"wm_row{vi}")
                nc.vector.tensor_copy(row[:], prow[:])
                pbc = wpp.tile([128, 1], f32, name="wm_bc")
                nc.tensor.matmul(pbc[:], C["ones1"][:], row[:, 0:1],
                                 start=True, stop=True)
                sb = wp.tile([128, 1], f32, name=f"wm_bcs{vi}")
                nc.vector.tensor_copy(sb[:], pbc[:])
                bc.append(sb)
            sc_bc, sh_bc = bc
            # x0 tiles
            xt0 = []
            for hb in range(2):
                t = wp.tile([128, 256], bf, name=f"wm_x{hb}")
                nc.sync.dma_start(t[:], C["y"][3][0, hb * 128:(hb + 1) * 128, :])
                nc.scalar.activation(t[:], t[:], FT.Relu, bias=sh_bc[:, 0:1],
                                     scale=sc_bc[:, 0:1])
                xt0.append(t)
            # T[18, 256] = E1^T @ x0
            pT = wpp.tile([18, 256], f32, name="wm_pT")
            for hb in range(2):
                nc.tensor.matmul(pT[:], C["cE1"][hb][:], xt0[hb][:],
                                 start=(hb == 0), stop=(hb == 1))
            Tsb = wp.tile([18, 256], f32, name="wm_T")
            nc.vector.tensor_copy(Tsb[:], pT[:])
            # transpose T -> Tt [128,18] x2 (bf16)
            Ttsb = []
            for hc in range(2):
                pTt = wpp.tile([128, 18], f32, name="wm_pTt")
                nc.tensor.transpose(pTt[:], Tsb[:, hc * 128:(hc + 1) * 128],
                                    C["ident"][0:18, 0:18])
                t = wp.tile([128, 18], bf, name=f"wm_Tt{hc}")
                nc.vector.tensor_copy(t[:], pTt[:])
                Ttsb.append(t)
            # F0 combos [18, 18] = Tt^T @ E2
            pF = wpp.tile([18, 18], f32, name="wm_pF")
            for hc in range(2):
                nc.tensor.matmul(pF[:], Ttsb[hc][:], C["cE2"][hc][:],
                                 start=(hc == 0), stop=(hc == 1))
            Fsb = wp.tile([18, 18], f32, name="wm_F")
            nc.vector.tensor_copy(Fsb[:], pF[:])
            Fsh = wp.tile([9, 18], f32, name="wm_Fsh")
            nc.sync.dma_start(Fsh[:], Fsb[9:18, :])
            # F0_re = Fsb[0:9,0:9] + Fsh[:,9:18]; F0_im = Fsh[:,0:9] - Fsb[0:9,9:18]
            F0re = wp.tile([9, 9], f32, name="wm_F0re")
            nc.vector.tensor_tensor(F0re[:], Fsb[0:9, 0:9], Fsh[:, 9:18], op=ALU.add)
            F0im = wp.tile([9, 9], f32, name="wm_F0im")
            nc.vector.tensor_tensor(F0im[:], Fsh[:, 0:9], Fsb[0:9, 9:18],
                                    op=ALU.subtract)
            # delta = (wmvT - F0) * kapT  -> Dstack [18, 9] bf16
            Dstack = wp.tile([18, 9], bf, name="wm_D")
            dre = wp.tile([9, 9], f32, name="wm_dre")
            nc.vector.tensor_tensor(dre[:], C["wmvT"][:], F0re[:], op=ALU.subtract)
            nc.vector.tensor_tensor(Dstack[0:9, :], dre[:], C["kapT"][:], op=ALU.mult)
            dim = wp.tile([9, 9], f32, name="wm_dim")
            nc.vector.tensor_tensor(dim[:], C["wmvT"][:], F0im[:], op=ALU.subtract)
            dimk = wp.tile([9, 9], bf, name="wm_dimk")
            nc.vector.tensor_tensor(dimk[:], dim[:], C["kapT"][:], op=ALU.mult)
            nc.sync.dma_start(Dstack[9:18, :], dimk[:])
            # M rows: Mst [18, 256] bf16 = [M_re; M_im]
            Mst = wp.tile([18, 256], bf, name="wm_Mst")
            pMre = wpp.tile([9, 256], f32, name="wm_pM")
            nc.tensor.matmul(pMre[:], Dstack[:], C["cE3"][:], start=True, stop=True)
            nc.vector.tensor_copy(Mst[0:9, :], pMre[:])
            pMim = wpp.tile([9, 256], f32, name="wm_pM")
            nc.tensor.matmul(pMim[:], Dstack[:], C["cE4"][:], start=True, stop=True)
            mtmp = wp.tile([9, 256], bf, name="wm_mtmp")
            nc.vector.tensor_copy(mtmp[:], pMim[:])
            nc.sync.dma_start(Mst[9:18, :], mtmp[:])
            # corr chunks + add x0 -> x0c_d
            for hc in range(2):
                pC = wpp.tile([128, 256], f32, name="wm_pC")
                nc.tensor.matmul(pC[:], Mst[:, hc * 128:(hc + 1) * 128],
                                 C["cE5"][:], start=True, stop=True)
                cb = wp.tile([128, 256], bf, name="wm_cb")
                nc.vector.tensor_copy(cb[:], pC[:])
                xo = wp.tile([128, 256], bf, name="wm_xo")
                nc.vector.tensor_tensor(xo[:], xt0[hc][:], cb[:], op=ALU.add)
                nc.sync.dma_start(C["x0c_d"][hc * 128:(hc + 1) * 128, :], xo[:])
            ap = self.maybe_debug("x0c", (H, W))
            if ap is not None:
                xf = wp.tile([128, 256], f32, name="wm_xf")
                for hc in range(2):
                    t = wp.tile([128, 256], bf, name="wm_rb")
                    nc.sync.dma_start(t[:], C["x0c_d"][hc * 128:(hc + 1) * 128, :])
                    nc.vector.tensor_copy(xf[:], t[:])
                    nc.sync.dma_start(ap[hc * 128:(hc + 1) * 128, :], xf[:])

    # ------------------------------------------------------------------
    def _build_body(self, X, tc):
        nc = self.nc
        C = self._consts
        d = self.d

        # ============ low-pass filter + image bf16 cast ============
        with ExitStack() as S:
            xp = S.enter_context(tc.tile_pool(name="lp_x", bufs=4))
            zp = S.enter_context(tc.tile_pool(name="lp_z", bufs=6))
            pp = S.enter_context(tc.tile_pool(name="lp_ps", bufs=1, space="PSUM"))
            ep = S.enter_context(tc.tile_pool(name="lp_ev", bufs=3))
            for c in range(3):
                def loader(xt, hbl, c=c):
                    nc.sync.dma_start(
                        xt[:],
                        d["image"][c, hbl * 128:(hbl + 1) * 128, :].bitcast(f32r))
                    xb = ep.tile([128, 256], bf, name="lp_xb")
                    nc.vector.tensor_copy(xb[:], xt[:].bitcast(f32))
                    nc.sync.dma_start(
                        C["imgb_d"][c, hbl * 128:(hbl + 1) * 128, :], xb[:])
                def evict(esb, hbl, c=c):
                    t1 = ep.tile([128, 256], f32, name="lp_t1")
                    nc.vector.tensor_scalar(t1[:], esb[:], 0.0, 255.0,
                                            op0=ALU.max, op1=ALU.min)
                    xi = ep.tile([128, 256], mybir.dt.int32, name="lp_xi")
                    nc.vector.tensor_copy(xi[:], t1[:])
                    xr = ep.tile([128, 256], f32, name="lp_xr")
                    nc.vector.tensor_copy(xr[:], xi[:])
                    gt = ep.tile([128, 256], f32, name="lp_gt")
                    nc.vector.tensor_tensor(gt[:], xr[:], t1[:], op=ALU.is_gt)
                    t2 = ep.tile([128, 256], bf, name="lp_t2")
                    nc.vector.tensor_tensor(t2[:], gt[:], xr[:], op=ALU.subtract)
                    nc.sync.dma_start(C["lpn_d"][c, hbl * 128:(hbl + 1) * 128, :],
                                      t2[:])
                self._fft_chain(tc, (xp, zp, pp),
                                src_loader=loader, mask=True, evict_fn=evict)

        # ============ conv0..3 ============
        sc, sh = self._conv0(X, tc)
        for k in range(1, 4):
            sc, sh = self._conv64(X, tc, k, C["y"][k - 1], C["y"][k], sc, sh)

        # ============ watermark correction (channel 0) ============
        self._wm_correction(tc, sc, sh)

        # ============ ac conv ============
        sc4, sh4 = self._conv64(X, tc, 4, C["y"][3], C["y"][4], sc, sh)

        # ============ final 1x1 conv ============
        with ExitStack() as S:
            bp = S.enter_context(tc.tile_pool(name="fin_b", bufs=3))
            pp = S.enter_context(tc.tile_pool(name="fin_ps", bufs=4, space="PSUM"))
            ep = S.enter_context(tc.tile_pool(name="fin_ev", bufs=3))
            fb6 = C["cp"].tile([6, 1], f32, name="fb6")
            nc.sync.dma_start(fb6[0:3, :], d["fb"][:])
            nc.sync.dma_start(fb6[3:6, :], d["fb"][:])
            sc128 = C["cp"].tile([128, 1], f32, name="fin_sc128")
            sh128 = C["cp"].tile([128, 1], f32, name="fin_sh128")
            for half in range(2):
                nc.sync.dma_start(sc128[64 * half:64 * half + 64, :], sc4[:])
                nc.sync.dma_start(sh128[64 * half:64 * half + 64, :], sh4[:])
            FR = 16
            for q in range(0, 128, FR):
                xf = bp.tile([128, FR, 256], bf, name="fin_x")
                nc.sync.dma_start(xf[0:64, :, :], C["y"][4][:, q:q + FR, :])
                nc.sync.dma_start(xf[64:128, :, :],
                                  C["y"][4][:, 128 + q:128 + q + FR, :])
                nc.scalar.activation(xf[:].rearrange("p r c -> p (r c)"),
                                     xf[:].rearrange("p r c -> p (r c)"),
                                     FT.Relu, bias=sh128[:, 0:1], scale=sc128[:, 0:1])
                for rr in range(0, FR, 2):
                    pt = pp.tile([6, 512], f32, name="fin_p")
                    nc.tensor.matmul(pt[:], C["lhsT_fin"][:],
                                     xf[:, rr:rr + 2, :].rearrange("p r c -> p (r c)"),
                                     start=True, stop=True)
                    osb = ep.tile([6, 512], f32, name="fin_o")
                    nc.vector.tensor_scalar(osb[:], pt[:], fb6[:, 0:1], None,
                                            op0=ALU.add)
                    nc.sync.dma_start(d["out"][:, q + rr:q + rr + 2, :],
                                      osb[0:3, :].rearrange("p (r c) -> p r c", c=256))
                    nc.sync.dma_start(d["out"][:, 128 + q + rr:128 + q + rr + 2, :],
                                      osb[3:6, :].rearrange("p (r c) -> p r c", c=256))

        # debug outputs
        for nm, src, shp in [("lpn", C["lpn_d"], (3, H, W)),
                             ("y0", C["y"][0], (64, H, W)),
                             ("y1", C["y"][1], (64, H, W)),
                             ("y2", C["y"][2], (64, H, W)),
                             ("y3", C["y"][3], (64, H, W)),
                             ("y4", C["y"][4], (64, H, W))]:
            ap = self.maybe_debug(nm, shp, dt=bf)
            if ap is not None:
                nc.sync.dma_start(ap[:], src[:])


# ======================================================================
# harness entry point: full inputs in, full outputs out (8 cores SPMD)
# ======================================================================
from concourse.bass_utils import run_bass_kernel_spmd

_ENC = None

def _get_enc():
    global _ENC
    if _ENC is None:
        e = Enc(n_cores=8)
        e.build()
        _ENC = e
    return _ENC

def make_in_maps(inputs):
    consts = host_constants()
    g = lambda k: np.ascontiguousarray(np.asarray(inputs[k], dtype=np.float32))
    image, message = g("image"), g("message")
    shared = dict(
        w0=g("w0"), b0=g("b0").reshape(64, 1), g0=g("g0").reshape(64, 1),
        be0=g("be0").reshape(64, 1), ws=g("ws"), bs=g("bs").reshape(3, 64, 1),
        gs=g("gs").reshape(3, 64, 1), bes=g("bes").reshape(3, 64, 1),
        acw=g("acw"), acb=g("acb").reshape(64, 1), acg=g("acg").reshape(64, 1),
        acbe=g("acbe").reshape(64, 1), fw=np.ascontiguousarray(g("fw")[:, :, 0, 0]),
        fb=g("fb").reshape(3, 1), **consts)
    return [dict(image=np.ascontiguousarray(image[i]),
                 message=np.ascontiguousarray(message[i].reshape(MSG, 1)),
                 **shared) for i in range(8)]

def kernel(**inputs):
    e = _get_enc()
    in_maps = make_in_maps(inputs)
    res = run_bass_kernel_spmd(e.nc, in_maps, core_ids=list(range(8)))
    out = np.stack([res.results[i]["out"] for i in range(8)], axis=0)
    return np.ascontiguousarray(out.astype(np.float32))


# revision 23
# speedup vs baseline: 1.3216x; 1.0169x over previous
"""Encoder kernel: nn_Encoder (watermark encoder) on 8 TRN2 cores,
data-parallel over batch (1 image per core).

Key structure vs the straightforward version:
  - watermark: ifft2(fft2(x).at[pos].set(v)) == x for channels 1..63;
    channel 0 gets x0 + Re(idft(delta)) with delta nonzero on a 9x9
    frequency block -> tiny matmuls instead of a 64-channel FFT chain.
  - convs in bf16 (weights + moving operands) -> FWL weight loads.
  - ac conv: 30 constant message channels folded into the bias with
    edge-mask corrections; image channels packed conv0-style.
Layouts per core:
  y_k DRAM [64, 256, 256] bf16 (pre-BN, conv bias added)
  lpn DRAM [3, 256, 256] bf16  (holds -floor(clip(low,0,255)); conv0
    weights negated/255)
  imgb DRAM [3, 256, 256] bf16 (image cast)
  x0c DRAM [256, 256] bf16     (bn-relu'd x0 + watermark correction)
"""
import numpy as np
import concourse.bass as bass
import concourse.tile as tile
from concourse import bacc, mybir
from contextlib import ExitStack

f32 = mybir.dt.float32
f32r = mybir.dt.float32r
bf = mybir.dt.bfloat16
FT = mybir.ActivationFunctionType
ALU = mybir.AluOpType

H = W = 256
HW = H * W
CH = 64
MSG = 30
WP = W + 2          # padded row length 258
R = 32              # conv band rows
NBAND = H // R      # 8

# ---------------------------------------------------------------- host consts
def host_constants():
    j = np.arange(256)
    ang = 2.0 * np.pi * np.outer(j, j) / 256.0
    C = np.cos(ang).astype(np.float32)
    S = (-np.sin(ang)).astype(np.float32)      # F = C + iS
    Ci = (C / 256.0).astype(np.float32)
    Sq = (S / 256.0).astype(np.float32)
    consts = {
        "cC": C, "cS": S, "cNS": -S, "cCi": Ci, "cSq": Sq, "cNSq": -Sq,
        "cCS": np.hstack([C, S]).astype(np.float32),
        "cIdent": np.eye(128, dtype=np.float32),
    }
    yy = np.arange(H, dtype=np.float32) - H // 2
    xx = np.arange(W, dtype=np.float32) - W // 2
    m = ((yy[:, None] ** 2 + xx[None, :] ** 2) <= float(50 * 50)).astype(np.float32)
    consts["cMask"] = np.fft.ifftshift(m).astype(np.float32)
    # watermark positions (all in channel 0; block kh,kw in 124..132)
    cy = cx = 128
    rr, cc = [], []
    idx = 0
    for i in range(-4, 5):
        for j2 in range(-4, 5):
            if idx >= MSG:
                break
            if (i * i + j2 * j2) ** 0.5 <= 4:
                rr.append(cy + i); cc.append(cx + j2); idx += 1
    rr = np.array(rr); cc = np.array(cc)   # rr = kh (axis -2), cc = kw (axis -1)
    pk = np.zeros((MSG, 9), np.float32)    # kw indicator
    fk = np.zeros((MSG, 9), np.float32)    # kh indicator
    kapT = np.zeros((9, 9), np.float32)    # [kh, kw] position mask
    for k in range(MSG):
        pk[k, cc[k] - 124] = 1.0
        fk[k, rr[k] - 124] = 1.0
        kapT[rr[k] - 124, cc[k] - 124] = 1.0
    consts["cPk"] = pk
    consts["cFk"] = fk
    consts["cKapT"] = kapT
    # wm DFT block matrices (bf16 on host; kh,kw in 124..132)
    import ml_dtypes
    th = 2.0 * np.pi / 256.0
    K9 = np.arange(124, 133, dtype=np.float64)
    hh = np.arange(256, dtype=np.float64)
    A = th * np.outer(hh, K9)            # [256, 9] angle(h, k)
    cosA, sinA = np.cos(A), np.sin(A)
    bfc = lambda x: np.ascontiguousarray(x).astype(ml_dtypes.bfloat16)
    consts["cE1"] = bfc(np.hstack([cosA, -sinA]))            # [256,18] fwd rows
    consts["cE2"] = bfc(np.hstack([cosA, sinA]))             # [256,18] fwd cols
    consts["cE3"] = bfc(np.hstack([cosA, -sinA]).T)          # [18,256] inv M_re
    consts["cE4"] = bfc(np.hstack([sinA, cosA]).T)           # [18,256] inv M_im
    consts["cE5"] = bfc(np.hstack([cosA / 65536.0, -sinA / 65536.0]).T)
    return consts


# ---------------------------------------------------------------- builders
class Enc:
    def __init__(self, n_cores=8, debug_outs=()):
        self.n_cores = n_cores
        self.ntot = float(n_cores * HW)
        self.debug_outs = debug_outs
        nc = bacc.Bacc("TRN2", target_bir_lowering=False, debug=False,
                       num_devices=n_cores)
        self.nc = nc
        d = {}
        d["image"] = nc.dram_tensor("image", (3, H, W), f32, kind="ExternalInput").ap()
        d["message"] = nc.dram_tensor("message", (MSG, 1), f32, kind="ExternalInput").ap()
        d["w0"] = nc.dram_tensor("w0", (64, 3, 3, 3), f32, kind="ExternalInput").ap()
        d["b0"] = nc.dram_tensor("b0", (64, 1), f32, kind="ExternalInput").ap()
        d["g0"] = nc.dram_tensor("g0", (64, 1), f32, kind="ExternalInput").ap()
        d["be0"] = nc.dram_tensor("be0", (64, 1), f32, kind="ExternalInput").ap()
        d["ws"] = nc.dram_tensor("ws", (3, 64, 64, 3, 3), f32, kind="ExternalInput").ap()
        d["bs"] = nc.dram_tensor("bs", (3, 64, 1), f32, kind="ExternalInput").ap()
        d["gs"] = nc.dram_tensor("gs", (3, 64, 1), f32, kind="ExternalInput").ap()
        d["bes"] = nc.dram_tensor("bes", (3, 64, 1), f32, kind="ExternalInput").ap()
        d["acw"] = nc.dram_tensor("acw", (64, 97, 3, 3), f32, kind="ExternalInput").ap()
        d["acb"] = nc.dram_tensor("acb", (64, 1), f32, kind="ExternalInput").ap()
        d["acg"] = nc.dram_tensor("acg", (64, 1), f32, kind="ExternalInput").ap()
        d["acbe"] = nc.dram_tensor("acbe", (64, 1), f32, kind="ExternalInput").ap()
        d["fw"] = nc.dram_tensor("fw", (3, 64), f32, kind="ExternalInput").ap()
        d["fb"] = nc.dram_tensor("fb", (3, 1), f32, kind="ExternalInput").ap()
        for k, shp, dt in [("cC", (256, 256), f32), ("cS", (256, 256), f32),
                           ("cNS", (256, 256), f32), ("cCi", (256, 256), f32),
                           ("cSq", (256, 256), f32), ("cNSq", (256, 256), f32),
                           ("cCS", (256, 512), f32), ("cIdent", (128, 128), f32),
                           ("cMask", (256, 256), f32),
                           ("cPk", (MSG, 9), f32), ("cFk", (MSG, 9), f32),
                           ("cKapT", (9, 9), f32),
                           ("cE1", (256, 18), bf), ("cE2", (256, 18), bf),
                           ("cE3", (18, 256), bf), ("cE4", (18, 256), bf),
                           ("cE5", (18, 256), bf)]:
            d[k] = nc.dram_tensor(k, shp, dt, kind="ExternalInput").ap()
        d["out"] = nc.dram_tensor("out", (3, H, W), f32, kind="ExternalOutput").ap()
        self.d = d
        self.dbg = {}

    def maybe_debug(self, name, shape, dt=f32):
        if name in self.debug_outs:
            self.dbg[name] = self.nc.dram_tensor(
                "dbg_" + name, shape, dt, kind="ExternalOutput").ap()
            return self.dbg[name]
        return None

    # ------------------------------------------------------------------
    def build(self):
        nc, d = self.nc, self.d
        with tile.TileContext(nc) as tc, ExitStack() as X:
            cp = X.enter_context(tc.tile_pool(name="consts", bufs=1))
            dp = X.enter_context(tc.tile_pool(name="dram", bufs=1, space="DRAM"))
            pwX = ExitStack()
            pw = pwX.enter_context(tc.tile_pool(name="pw", bufs=2, space="PSUM"))
            wev = pwX.enter_context(tc.tile_pool(name="wev", bufs=2))

            # ---------------- DRAM scratch
            y = [dp.tile([64, H, W], bf, name=f"yact{k}") for k in range(5)]
            lpn_d = dp.tile([3, H, W], bf, name="lpn_d")
            imgb_d = dp.tile([3, H, W], bf, name="imgb_d")
            x0c_d = dp.tile([H, W], bf, name="x0c_d")
            cl_in = [[dp.tile([64, 2], f32, name=f"clin{k}_{p}") for p in range(2)]
                     for k in range(5)]
            cl_out = [[dp.tile([64, 2], f32, name=f"clout{k}_{p}",
                               addr_space="Shared") for p in range(2)]
                      for k in range(5)]

            # ---------------- constants into SBUF
            def cload(name, src, shape, dtype):
                t = cp.tile(shape, dtype, name=name)
                nc.sync.dma_start(t[:], src)
                return t
            DF = {}
            for nm, wdt in [("cC", 256), ("cS", 256), ("cNS", 256), ("cCi", 256),
                            ("cSq", 256), ("cNSq", 256), ("cCS", 512)]:
                DF[nm + "_hi"] = cload(nm + "_hi", d[nm][0:128, :].bitcast(f32r),
                                       [128, wdt], f32r)
                DF[nm + "_lo"] = cload(nm + "_lo", d[nm][128:256, :].bitcast(f32r),
                                       [128, wdt], f32r)
            ident = cload("ident", d["cIdent"][:], [128, 128], f32)
            maskt = [cload(f"maskt{i}", d["cMask"][i * 128:(i + 1) * 128, :],
                           [128, 256], f32) for i in range(2)]
            pk_t = cload("pk_t", d["cPk"][:], [MSG, 9], f32)
            fk_t = cload("fk_t", d["cFk"][:], [MSG, 9], f32)
            kapT_t = cload("kapT_t", d["cKapT"][:], [9, 9], f32)
            msg_t = cload("msg_t", d["message"][:], [MSG, 1], f32)
            cE1 = [cload(f"cE1_{i}", d["cE1"][i * 128:(i + 1) * 128, :], [128, 18], bf)
                   for i in range(2)]
            cE2 = [cload(f"cE2_{i}", d["cE2"][i * 128:(i + 1) * 128, :], [128, 18], bf)
                   for i in range(2)]
            cE3 = cload("cE3", d["cE3"][:], [18, 256], bf)
            cE4 = cload("cE4", d["cE4"][:], [18, 256], bf)
            cE5 = cload("cE5", d["cE5"][:], [18, 256], bf)
            eps64 = cp.tile([64, 1], f32, name="eps64")
            nc.vector.memset(eps64[:], 1e-5)
            zero128 = cp.tile([128, 1], f32, name="zero128")
            nc.vector.memset(zero128[:], 0.0)
            ones1 = cp.tile([1, 128], f32, name="ones1")
            nc.vector.memset(ones1[:], 1.0)
            msg_b = cp.tile([MSG, 1], bf, name="msg_b")
            nc.vector.tensor_copy(msg_b[:], msg_t[:])

            # per-layer bn param tiles
            def vload(name, src):
                t = cp.tile([64, 1], f32, name=name)
                nc.sync.dma_start(t[:], src)
                return t
            g_t = [vload("g_t0", d["g0"][:])] + \
                  [vload(f"g_t{k+1}", d["gs"][k]) for k in range(3)] + \
                  [vload("g_t4", d["acg"][:])]
            be_t = [vload("be_t0", d["be0"][:])] + \
                   [vload(f"be_t{k+1}", d["bes"][k]) for k in range(3)] + \
                   [vload("be_t4", d["acbe"][:])]
            acb_t = vload("acb_t", d["acb"][:])
            # conv bias replicated to 128 partitions; b128[4] (ac) filled later
            b128 = []
            for k, src in enumerate([d["b0"], d["bs"][0], d["bs"][1], d["bs"][2]]):
                t = cp.tile([128, 1], f32, name=f"b128_{k}")
                nc.sync.dma_start(t[0:64, :], src)
                nc.sync.dma_start(t[64:128, :], src)
                b128.append(t)
            b128.append(cp.tile([128, 1], f32, name="b128_4"))
            fb_t = cp.tile([3, 1], f32, name="fb_t")
            nc.sync.dma_start(fb_t[:], d["fb"][:])

            # ---------------- weight transposes (lhsT prep), all bf16
            # conv1..3: pair lhsT [128,128] x3(dw), cross lhsT [64,128] x3
            lhsT_pair, lhsT_sing = [], []
            for k in range(3):
                wsrc = cp.tile([64, 64, 9], f32, name=f"wsrc{k}")
                nc.sync.dma_start(wsrc[:], d["ws"][k].rearrange("o i a b -> o i (a b)"))
                pairs, sings = self._make_pairs64(pw, cp, wsrc, ident, f"c{k}")
                lhsT_pair.append(pairs); lhsT_sing.append(sings)
            # ac conv enc part (input ch 30:94) -> same pair structure
            acsrc = cp.tile([64, 97, 9], f32, name="acsrc")
            nc.sync.dma_start(acsrc[:], d["acw"][:].rearrange("o i a b -> o i (a b)"))
            ac_pairs, ac_sings = self._make_pairs64(
                pw, cp, acsrc, ident, "ac", in_off=30)
            # full acw transposes for aux/msg handling: acT_sb[tap] [97,64] bf16
            acT_sb = []
            for tap in range(9):
                p = pw.tile([97, 64], f32, name="pwa", bufs=2)
                nc.tensor.transpose(p[:], acsrc[:, :, tap], ident[0:64, 0:64])
                t = cp.tile([97, 64], bf, name=f"acT{tap}")
                nc.vector.tensor_copy(t[:], p[:])
                acT_sb.append(t)
            # msg-fold: M9[o, tap] = sum_c acw[o, c<30, tap] * msg[c]
            pM9 = pw.tile([64, 9], f32, name="pM9", bufs=1)
            for tap in range(9):
                nc.tensor.matmul(pM9[:, tap:tap + 1], acT_sb[tap][0:30, :],
                                 msg_b[:], start=True, stop=True)
            M9sb = cp.tile([64, 9], f32, name="M9sb")
            nc.vector.tensor_copy(M9sb[:], pM9[:])
            # bias_eff = acb + sum_j M9[:, j]  -> b128[4]
            bsum = wev.tile([64, 1], f32, name="bsum")
            nc.vector.tensor_reduce(bsum[:], M9sb[:], axis=mybir.AxisListType.X,
                                    op=ALU.add)
            beff = cp.tile([64, 1], f32, name="beff")
            nc.vector.tensor_tensor(beff[:], acb_t[:], bsum[:], op=ALU.add)
            nc.sync.dma_start(b128[4][0:64, :], beff[:])
            nc.sync.dma_start(b128[4][64:128, :], beff[:])
            # edge sums: S_top=j0:3, S_bot=j6:9, S_left=j{0,3,6}, S_right=j{2,5,8}
            sb_top = cp.tile([64, 1], f32, name="sb_top")
            nc.vector.tensor_reduce(sb_top[:], M9sb[:, 0:3], axis=mybir.AxisListType.X,
                                    op=ALU.add)
            sb_bot = cp.tile([64, 1], f32, name="sb_bot")
            nc.vector.tensor_reduce(sb_bot[:], M9sb[:, 6:9], axis=mybir.AxisListType.X,
                                    op=ALU.add)
            s_lr = []
            for nm, js in [("sl", (0, 3, 6)), ("sr", (2, 5, 8))]:
                t0 = wev.tile([64, 1], f32, name=nm + "a")
                nc.vector.tensor_tensor(t0[:], M9sb[:, js[0]:js[0] + 1],
                                        M9sb[:, js[1]:js[1] + 1], op=ALU.add)
                t1 = cp.tile([64, 1], f32, name=nm)
                nc.vector.tensor_tensor(t1[:], t0[:], M9sb[:, js[2]:js[2] + 1],
                                        op=ALU.add)
                s_lr.append(t1)
            # bottom fixups need partition base 64: fixB [128,3]
            fixB = cp.tile([128, 3], f32, name="fixB")
            nc.sync.dma_start(fixB[64:128, 0:1], sb_bot[:])
            nc.sync.dma_start(fixB[64:128, 1:2], M9sb[:, 6:7])
            nc.sync.dma_start(fixB[64:128, 2:3], M9sb[:, 8:9])
            # aux lhsT [17,128] x3(dw): rows 0:15 img (sigma,ch), 15:17 -S_l/-S_r
            lhsT_aux = []
            for dw in range(3):
                la = cp.tile([17, 128], bf, name=f"lhsTaux{dw}")
                nc.vector.memset(la[:], 0.0)
                lhsT_aux.append(la)
            for dw in range(3):
                for sg in range(3):       # g0: tap dh=sg
                    nc.sync.dma_start(lhsT_aux[dw][sg * 3:sg * 3 + 3, 0:64],
                                      acT_sb[sg * 3 + dw][94:97, :])
                for sg in range(2, 5):    # g1: tap dh=sg-2
                    nc.sync.dma_start(lhsT_aux[dw][sg * 3:sg * 3 + 3, 64:128],
                                      acT_sb[(sg - 2) * 3 + dw][94:97, :])
            # mask rows: transpose S vectors to rows, negate, bf16
            for i, sv in enumerate(s_lr):
                prow = pw.tile([1, 64], f32, name="prow", bufs=1)
                nc.tensor.transpose(prow[:], sv[:], ident[0:64, 0:64])
                rowneg = wev.tile([1, 64], bf, name="rowneg")
                nc.vector.tensor_scalar_mul(rowneg[:], prow[:], -1.0)
                nc.sync.dma_start(lhsT_aux[1][15 + i:16 + i, 0:64], rowneg[:])
                nc.sync.dma_start(lhsT_aux[1][15 + i:16 + i, 64:128], rowneg[:])
            # conv0: lhsT0x[dw] [15,128] = -w0^T/255, (sigma,ch) rows
            w0src = cp.tile([64, 3, 9], f32, name="w0src")
            nc.sync.dma_start(w0src[:], d["w0"][:].rearrange("o i a b -> o i (a b)"))
            lhsT0 = []
            for dw in range(3):
                l0 = cp.tile([15, 128], bf, name=f"lhsT0{dw}")
                nc.vector.memset(l0[:], 0.0)
                lhsT0.append(l0)
            for dw in range(3):
                for dh in range(3):
                    p = pw.tile([64, 64], f32, name="pwt")
                    nc.tensor.transpose(p[0:3, :], w0src[:, :, dh * 3 + dw],
                                        ident[0:64, 0:64])
                    tmp0 = wev.tile([3, 64], bf, name="w0tmp")
                    nc.vector.tensor_scalar_mul(tmp0[:], p[0:3, :], -1.0 / 255.0)
                    nc.sync.dma_start(lhsT0[dw][dh * 3:dh * 3 + 3, 0:64], tmp0[:])
                    nc.sync.dma_start(lhsT0[dw][(dh + 2) * 3:(dh + 2) * 3 + 3, 64:128],
                                      tmp0[:])
            # final 1x1: lhsT_fin [128,6]
            fwsrc = cp.tile([3, 64], f32, name="fwsrc")
            nc.sync.dma_start(fwsrc[:], d["fw"][:])
            lhsT_fin = cp.tile([128, 6], bf, name="lhsT_fin")
            nc.vector.memset(lhsT_fin[:], 0.0)
            p = pw.tile([64, 64], f32, name="pwt")
            nc.tensor.transpose(p[:, 0:3], fwsrc[:], ident[0:3, 0:3])
            pbf = wev.tile([64, 3], bf, name="pbf")
            nc.vector.tensor_copy(pbf[:], p[:, 0:3])
            nc.sync.dma_start(lhsT_fin[0:64, 0:3], pbf[:])
            nc.sync.dma_start(lhsT_fin[64:128, 3:6], pbf[:])

            # watermark grid wmvT [kh, kw]: fk^T @ (pk * msg)
            msgc2 = cp.tile([MSG, 9], f32, name="msgc2")
            nc.vector.tensor_scalar(msgc2[:], pk_t[:], msg_t[:, 0:1], None,
                                    op0=ALU.mult)
            pwm = pw.tile([9, 9], f32, name="pwm", bufs=1)
            nc.tensor.matmul(pwm[:], fk_t[:], msgc2[:], start=True, stop=True)
            wmvT = cp.tile([9, 9], f32, name="wmvT")
            nc.vector.tensor_copy(wmvT[:], pwm[:])

            # ac aux band tiles (manual double-buffer) + mask columns.
            # Engine ops need 32-aligned partition bases; masks and odd-offset
            # zero fills go through base-0 staging tiles + DMA.
            ml_t = cp.tile([1, R + 2, WP], bf, name="ml_t")
            nc.vector.memset(ml_t[:].rearrange("p r c -> p (r c)"), 0.0)
            nc.vector.memset(ml_t[0:1, :, 1:2], 1.0)
            mr_t = cp.tile([1, R + 2, WP], bf, name="mr_t")
            nc.vector.memset(mr_t[:].rearrange("p r c -> p (r c)"), 0.0)
            nc.vector.memset(mr_t[0:1, :, 256:257], 1.0)
            zrow = cp.tile([3, 1, WP], bf, name="zrow")
            nc.vector.memset(zrow[:].rearrange("p r c -> p (r c)"), 0.0)
            aux_bufs = [cp.tile([17, R + 2, WP], bf, name=f"auxb{i}")
                        for i in range(2)]
            for t in aux_bufs:
                nc.vector.memset(t[:].rearrange("p r c -> p (r c)"), 0.0)
                nc.sync.dma_start(t[15:16, :, :], ml_t[:])
                nc.sync.dma_start(t[16:17, :, :], mr_t[:])

            self._consts = dict(DF=DF, ident=ident, maskt=maskt,
                                g_t=g_t, be_t=be_t, b128=b128, fb_t=fb_t,
                                lhsT_pair=lhsT_pair, lhsT_sing=lhsT_sing,
                                lhsT0=lhsT0, ac_pairs=ac_pairs, ac_sings=ac_sings,
                                lhsT_aux=lhsT_aux, lhsT_fin=lhsT_fin,
                                cE1=cE1, cE2=cE2, cE3=cE3, cE4=cE4, cE5=cE5,
                                kapT=kapT_t, wmvT=wmvT, ones1=ones1,
                                sb_top=sb_top, fixB=fixB, M9sb=M9sb,
                                eps64=eps64, zero128=zero128, cp=cp, zrow=zrow,
                                y=y, lpn_d=lpn_d, imgb_d=imgb_d, x0c_d=x0c_d,
                                aux_bufs=aux_bufs,
                                cl_in=cl_in, cl_out=cl_out)
            pwX.close()
            self._build_body(X, tc)
        self.nc.compile()

    # ------------------------------------------------------------------
    def _make_pairs64(self, pw, cp, wsrc, ident, tag, in_off=0):
        """Pair/cross lhsT tiles (bf16) for a 64->64 3x3 conv.
        wsrc [64, >=in_off+64, 9] f32 (o, i, tap)."""
        nc = self.nc
        pairs, sings = [], []
        for dw in range(3):
            pA = cp.tile([128, 128], bf, name=f"lTpA{tag}{dw}")
            pB = cp.tile([128, 128], bf, name=f"lTpB{tag}{dw}")
            cx = cp.tile([64, 128], bf, name=f"lTcx{tag}{dw}")
            nc.vector.memset(pA[:], 0.0)
            nc.vector.memset(pB[:], 0.0)
            tp3 = []
            for dh in range(3):
                p = pw.tile([64, 64], f32, name="pwt")
                nc.tensor.transpose(p[:], wsrc[:, in_off:in_off + 64, dh * 3 + dw],
                                    ident[0:64, 0:64])
                tp3.append(p)
            nc.vector.tensor_copy(pA[0:64, 0:64], tp3[0][:])
            nc.vector.tensor_copy(pA[64:128, 0:64], tp3[1][:])
            nc.vector.tensor_copy(pB[0:64, 64:128], tp3[1][:])
            nc.vector.tensor_copy(pB[64:128, 64:128], tp3[2][:])
            nc.vector.tensor_copy(cx[:, 0:64], tp3[2][:])
            nc.vector.tensor_copy(cx[:, 64:128], tp3[0][:])
            pairs.append((pA, pB)); sings.append(cx)
        return pairs, sings

    # ------------------------------------------------------------------
    def _stats_partial(self, pool, layer, part, ssum_cols, sqsum_cols, c0, c1):
        """Reduce stat columns [c0:c1), fold partition halves, AllReduce."""
        nc = self.nc
        C = self._consts
        red = pool.tile([128, 2], f32, name=f"red{layer}_{part}")
        nc.vector.tensor_reduce(red[:, 0:1], ssum_cols[:, c0:c1],
                                axis=mybir.AxisListType.X, op=ALU.add)
        nc.vector.tensor_reduce(red[:, 1:2], sqsum_cols[:, c0:c1],
                                axis=mybir.AxisListType.X, op=ALU.add)
        upper = pool.tile([64, 2], f32, name=f"up{layer}_{part}")
        nc.sync.dma_start(upper[:], red[64:128, :])
        stats = pool.tile([64, 2], f32, name=f"st{layer}_{part}")
        nc.vector.tensor_add(stats[:], red[0:64, :], upper[:])
        nc.sync.dma_start(C["cl_in"][layer][part][:], stats[:])
        nc.gpsimd.collective_compute(
            "AllReduce", ALU.add,
            replica_groups=[list(range(self.n_cores))],
            ins=[C["cl_in"][layer][part].opt()],
            outs=[C["cl_out"][layer][part].opt()])
        sr = pool.tile([64, 2], f32, name=f"sr{layer}_{part}")
        nc.sync.dma_start(sr[:], C["cl_out"][layer][part][:])
        return sr

    def _stats_combine(self, pool, layer, srA, srB):
        """sum partials -> fused scale/shift math."""
        nc = self.nc
        C = self._consts
        N = self.ntot
        s2 = pool.tile([64, 2], f32, name=f"s2{layer}")
        nc.vector.tensor_add(s2[:], srA[:], srB[:])
        msq = pool.tile([64, 1], f32, name=f"msq{layer}")
        nc.vector.tensor_scalar(msq[:], s2[:, 0:1], s2[:, 0:1], 1.0 / (N * N),
                                op0=ALU.mult, op1=ALU.mult)
        var = pool.tile([64, 1], f32, name=f"var{layer}")
        nc.vector.scalar_tensor_tensor(var[:], s2[:, 1:2], 1.0 / N, msq[:],
                                       op0=ALU.mult, op1=ALU.subtract)
        std = pool.tile([64, 1], f32, name=f"std{layer}")
        nc.scalar.activation(std[:], var[:], FT.Sqrt, bias=C["eps64"][:, 0:1],
                             scale=1.0)
        istd = pool.tile([64, 1], f32, name=f"istd{layer}")
        nc.vector.reciprocal(istd[:], std[:])
        scale = C["cp"].tile([64, 1], f32, name=f"scale{layer}")
        nc.vector.tensor_tensor(scale[:], C["g_t"][layer][:], istd[:], op=ALU.mult)
        prodneg = pool.tile([64, 1], f32, name=f"pn{layer}")
        nc.vector.scalar_tensor_tensor(prodneg[:], s2[:, 0:1], -1.0 / N, scale[:],
                                       op0=ALU.mult, op1=ALU.mult)
        shift = C["cp"].tile([64, 1], f32, name=f"shift{layer}")
        nc.vector.tensor_tensor(shift[:], prodneg[:], C["be_t"][layer][:],
                                op=ALU.add)
        ap = self.maybe_debug(f"sc{layer}", (64, 1))
        if ap is not None:
            nc.sync.dma_start(ap[:], scale[:])
        ap = self.maybe_debug(f"sh{layer}", (64, 1))
        if ap is not None:
            nc.sync.dma_start(ap[:], shift[:])
        return scale, shift

    # ------------------------------------------------------------------
    def _evict(self, pp_t, ep, sp_cols, layer, cidx, dst, q):
        """PSUM chunk -> +bias (scalar, ssum accum) -> bf16 -> DRAM;
        sqsum via vector tensor_tensor_reduce."""
        nc = self.nc
        C = self._consts
        ysb = ep.tile([128, 512], bf, name=f"ysb{layer}")
        nc.vector.tensor_scalar(ysb[:], pp_t[:], C["b128"][layer][:, 0:1],
                                0.0, op0=ALU.add, op1=ALU.add,
                                accum_out=sp_cols[0][:, cidx:cidx + 1])
        scr = ep.tile([128, 512], bf, name=f"sqscr{layer}")
        nc.scalar.activation(scr[:], ysb[:], FT.Square,
                             bias=C["zero128"][:, 0:1],
                             accum_out=sp_cols[1][:, cidx:cidx + 1])
        nc.sync.dma_start(dst[:, q:q + 2, :],
                          ysb[0:64, :].rearrange("p (r c) -> p r c", c=256))
        nc.sync.dma_start(dst[:, q + 2:q + 4, :],
                          ysb[64:128, :].rearrange("p (r c) -> p r c", c=256))

    # ------------------------------------------------------------------
    def _conv64(self, X, tc, layer, src, dst, scale, shift):
        """conv layers 1..3 (64->64) and 4 (ac: 64 enc + aux)."""
        nc = self.nc
        C = self._consts
        is_ac = (layer == 4)
        pairs = C["ac_pairs"] if is_ac else C["lhsT_pair"][layer - 1]
        sings = C["ac_sings"] if is_ac else C["lhsT_sing"][layer - 1]
        with ExitStack() as S:
            bp = S.enter_context(tc.tile_pool(name=f"band{layer}", bufs=4))
            pp = S.enter_context(tc.tile_pool(name=f"psum{layer}", bufs=4,
                                              space="PSUM"))
            ep = S.enter_context(tc.tile_pool(name=f"evict{layer}", bufs=3))
            sp = S.enter_context(tc.tile_pool(name=f"stat{layer}", bufs=1))
            ssum_cols = sp.tile([128, 64], f32, name=f"ssc{layer}")
            sqsum_cols = sp.tile([128, 64], f32, name=f"sqc{layer}")
            cidx = 0
            for bi in range(NBAND):
                r0 = bi * R
                band = bp.tile([128, (R + 2) * WP], bf, name=f"bandt{layer}")
                b3 = band[:].rearrange("p (r c) -> p r c", c=WP)
                i0 = 1 if bi == 0 else 0
                i1 = R + 1 if bi == NBAND - 1 else R + 2
                rl, rh = r0 - 1 + i0, r0 - 1 + i1
                nc.vector.memset(b3[0:64, :, 0:1], 0.0)
                nc.vector.memset(b3[0:64, :, 257:258], 0.0)
                if bi == 0:
                    nc.vector.memset(b3[0:64, 0:1, :], 0.0)
                if bi == NBAND - 1:
                    nc.vector.memset(b3[0:64, R + 1:R + 2, :], 0.0)
                nc.sync.dma_start(b3[0:64, i0:i1, 1:257], src[:, rl:rh, :])
                nc.scalar.activation(b3[0:64, i0:i1, 1:257],
                                     b3[0:64, i0:i1, 1:257],
                                     FT.Relu, bias=shift[:, 0:1],
                                     scale=scale[:, 0:1])
                if is_ac:
                    # ch0 = x0corr (already bn-relu'd + wm corr): overwrite
                    nc.sync.dma_start(b3[0:1, i0:i1, 1:257], C["x0c_d"][rl:rh, :])
                nc.sync.dma_start(b3[64:128, 0:R + 1, :], b3[0:64, 1:R + 2, :])
                if is_ac:
                    aux = C["aux_bufs"][bi % 2]
                    # img partitions (sigma,ch): band row i <- img row r0-1+i+sg
                    for sg in range(5):
                        a0 = 1 if (bi == 0 and sg == 0) else 0
                        a1 = min(30, 257 - r0 - sg)
                        if a1 <= a0:
                            continue
                        nc.sync.dma_start(
                            aux[sg * 3:sg * 3 + 3, a0:a1, 1:257],
                            C["imgb_d"][:, r0 - 1 + a0 + sg:r0 - 1 + a1 + sg, :])
                    if bi == NBAND - 1:
                        nc.sync.dma_start(aux[12:15, 29:30, :], C["zrow"][:])
                for t in range(0, R, 4):
                    pt = pp.tile([128, 512], f32, name=f"pchunk{layer}")
                    nmm = 12 if is_ac else 9
                    mm = 0
                    for dw in range(3):
                        pA, pB = pairs[dw]
                        nc.tensor.matmul(
                            pt[:], pA[:], b3[0:128, t:t + 2, dw:dw + 256],
                            start=(mm == 0), stop=(mm == nmm - 1)); mm += 1
                        nc.tensor.matmul(
                            pt[:], pB[:], b3[0:128, t + 3:t + 5, dw:dw + 256],
                            start=(mm == 0), stop=(mm == nmm - 1)); mm += 1
                        nc.tensor.matmul(
                            pt[:], sings[dw][:], b3[0:64, t + 2:t + 4, dw:dw + 256],
                            start=(mm == 0), stop=(mm == nmm - 1)); mm += 1
                        if is_ac:
                            np_aux = 17 if dw == 1 else 15
                            nc.tensor.matmul(
                                pt[:], C["lhsT_aux"][dw][0:np_aux, :],
                                aux[0:np_aux, t:t + 2, dw:dw + 256],
                                start=(mm == 0), stop=(mm == nmm - 1)); mm += 1
                    if is_ac and bi == 0 and t == 0:
                        # row 0 fixups: -S_top everywhere, +A_tl/+A_tr corners
                        nc.vector.tensor_scalar(pt[0:64, 0:256], pt[0:64, 0:256],
                                                C["sb_top"][:, 0:1], None,
                                                op0=ALU.subtract)
                        nc.vector.tensor_tensor(pt[0:64, 0:1], pt[0:64, 0:1],
                                                C["M9sb"][:, 0:1], op=ALU.add)
                        nc.vector.tensor_tensor(pt[0:64, 255:256], pt[0:64, 255:256],
                                                C["M9sb"][:, 2:3], op=ALU.add)
                    if is_ac and bi == NBAND - 1 and t == R - 4:
                        nc.vector.tensor_scalar(pt[64:128, 256:512],
                                                pt[64:128, 256:512],
                                                C["fixB"][64:128, 0:1], None,
                                                op0=ALU.subtract)
                        nc.vector.tensor_tensor(pt[64:128, 256:257],
                                                pt[64:128, 256:257],
                                                C["fixB"][64:128, 1:2], op=ALU.add)
                        nc.vector.tensor_tensor(pt[64:128, 511:512],
                                                pt[64:128, 511:512],
                                                C["fixB"][64:128, 2:3], op=ALU.add)
                    self._evict(pt, ep, (ssum_cols, sqsum_cols), layer, cidx,
                                dst, r0 + t)
                    cidx += 1
                if bi == NBAND - 2:
                    srA = self._stats_partial(sp, layer, 0, ssum_cols,
                                              sqsum_cols, 0, cidx)
                    csplit = cidx
            srB = self._stats_partial(sp, layer, 1, ssum_cols, sqsum_cols,
                                      csplit, cidx)
            return self._stats_combine(sp, layer, srA, srB)

    # ------------------------------------------------------------------
    def _conv0(self, X, tc):
        """conv0: lpn(3ch, negated/255) -> y0. (sigma,ch) 15-partition bands."""
        nc = self.nc
        C = self._consts
        with ExitStack() as S:
            bp = S.enter_context(tc.tile_pool(name="band0", bufs=3))
            pp = S.enter_context(tc.tile_pool(name="psum0", bufs=4, space="PSUM"))
            ep = S.enter_context(tc.tile_pool(name="evict0", bufs=3))
            sp = S.enter_context(tc.tile_pool(name="stat0", bufs=1))
            ssum_cols = sp.tile([128, 64], f32, name="ssc0")
            sqsum_cols = sp.tile([128, 64], f32, name="sqc0")
            cidx = 0
            for bi in range(NBAND):
                r0 = bi * R
                band = bp.tile([15, (R + 2) * WP], bf, name="bandt0")
                b3 = band[:].rearrange("p (r c) -> p r c", c=WP)
                nc.vector.memset(b3[:, :, 0:1], 0.0)
                nc.vector.memset(b3[:, :, 257:258], 0.0)
                if bi == 0:
                    nc.vector.memset(b3[0:3, 0:1, :], 0.0)
                if bi == NBAND - 1:
                    nc.sync.dma_start(b3[12:15, 29:30, :], C["zrow"][:])
                for sg in range(5):
                    a0 = 1 if (bi == 0 and sg == 0) else 0
                    a1 = min(30, 256 - r0 - sg + 1)
                    if a1 <= a0:
                        continue
                    nc.sync.dma_start(
                        b3[sg * 3:sg * 3 + 3, a0:a1, 1:257],
                        C["lpn_d"][:, r0 - 1 + a0 + sg:r0 - 1 + a1 + sg, :])
                for t in range(0, R, 4):
                    pt = pp.tile([128, 512], f32, name="pchunk0")
                    for dw in range(3):
                        nc.tensor.matmul(
                            pt[:], C["lhsT0"][dw][:],
                            b3[0:15, t:t + 2, dw:dw + 256],
                            start=(dw == 0), stop=(dw == 2))
                    self._evict(pt, ep, (ssum_cols, sqsum_cols), 0, cidx,
                                C["y"][0], r0 + t)
                    cidx += 1
                if bi == NBAND - 2:
                    srA = self._stats_partial(sp, 0, 0, ssum_cols,
                                              sqsum_cols, 0, cidx)
                    csplit = cidx
            srB = self._stats_partial(sp, 0, 1, ssum_cols, sqsum_cols,
                                      csplit, cidx)
            return self._stats_combine(sp, 0, srA, srB)

    # ------------------------------------------------------------------
    def _lowpass(self, tc):
        """fft2 -> disk mask -> ifft2, all 3 channels per pass (f32r),
        writing lpn_d (negated floor) and imgb_d (bf16 image cast)."""
        nc = self.nc
        C = self._consts
        d = self.d
        DF = C["DF"]

        def LT(nm, chunk):
            return DF[nm + ("_hi" if chunk == 0 else "_lo")]

        with ExitStack() as S:
            xp = S.enter_context(tc.tile_pool(name="lp_x", bufs=1))
            zp = S.enter_context(tc.tile_pool(name="lp_z", bufs=1))
            pp = S.enter_context(tc.tile_pool(name="lp_ps", bufs=1, space="PSUM"))
            ep = S.enter_context(tc.tile_pool(name="lp_ev", bufs=3))
            xt = {}
            for c in range(3):
                for hb in range(2):
                    t = xp.tile([128, 256], f32r, name=f"lpx{c}{hb}")
                    nc.sync.dma_start(
                        t[:], d["image"][c, hb * 128:(hb + 1) * 128, :].bitcast(f32r))
                    xb = ep.tile([128, 256], bf, name="lp_xb")
                    nc.vector.tensor_copy(xb[:], t[:].bitcast(f32))
                    nc.sync.dma_start(
                        C["imgb_d"][c, hb * 128:(hb + 1) * 128, :], xb[:])
                    xt[c, hb] = t
            Zt = {}
            for c in range(3):
                for wbl in range(2):
                    pZ = pp.tile([128, 512], f32, name="lp_ps1", bufs=2)
                    for ch in range(2):
                        nc.tensor.matmul(pZ[:], xt[c, ch][:, wbl * 128:(wbl + 1) * 128],
                                         LT("cCS", ch)[:], start=(ch == 0),
                                         stop=(ch == 1))
                    z = zp.tile([128, 512], f32r, name=f"lpz{c}{wbl}")
                    nc.vector.tensor_copy(z[:], pZ[:])
                    Zt[c, wbl] = z
            fsb = {}
            for c in range(3):
                for kwbl in range(2):
                    pf = pp.tile([128, 512], f32, name="lp_ps2", bufs=2)
                    sl = slice(kwbl * 128, (kwbl + 1) * 128)
                    for ch in range(2):
                        nc.tensor.matmul(pf[:], LT("cC", ch)[:, sl], Zt[c, ch][:],
                                         start=(ch == 0), stop=False)
                    for ch in range(2):
                        nc.tensor.matmul(pf[:, 0:256], LT("cNS", ch)[:, sl],
                                         Zt[c, ch][:, 256:512], start=False, stop=False)
                    for ch in range(2):
                        nc.tensor.matmul(pf[:, 256:512], LT("cS", ch)[:, sl],
                                         Zt[c, ch][:, 0:256], start=False,
                                         stop=(ch == 1))
                    ft = zp.tile([128, 512], f32r, name=f"lpf{c}{kwbl}")
                    nc.vector.tensor_tensor(ft[:, 0:256], pf[:, 0:256],
                                            C["maskt"][kwbl][:], op=ALU.mult)
                    nc.vector.tensor_tensor(ft[:, 256:512], pf[:, 256:512],
                                            C["maskt"][kwbl][:], op=ALU.mult)
                    fsb[c, kwbl] = ft
            Gt = {}
            for c in range(3):
                for khbl in range(2):
                    pG = pp.tile([128, 512], f32, name="lp_ps3", bufs=2)
                    sl = slice(khbl * 128, (khbl + 1) * 128)
                    sl2 = slice(256 + khbl * 128, 256 + (khbl + 1) * 128)
                    for ch in range(2):
                        nc.tensor.matmul(pG[:, 0:256], fsb[c, ch][:, sl],
                                         LT("cCi", ch)[:], start=(ch == 0), stop=False)
                    for ch in range(2):
                        nc.tensor.matmul(pG[:, 0:256], fsb[c, ch][:, sl2],
                                         LT("cSq", ch)[:], start=False, stop=False)
                    for ch in range(2):
                        nc.tensor.matmul(pG[:, 256:512], fsb[c, ch][:, sl2],
                                         LT("cCi", ch)[:], start=(ch == 0), stop=False)
                    for ch in range(2):
                        nc.tensor.matmul(pG[:, 256:512], fsb[c, ch][:, sl],
                                         LT("cNSq", ch)[:], start=False, stop=(ch == 1))
                    gt = zp.tile([128, 512], f32r, name=f"lpg{c}{khbl}")
                    nc.vector.tensor_copy(gt[:], pG[:])
                    Gt[c, khbl] = gt
            for c in range(3):
                for hbl in range(2):
                    pE = pp.tile([128, 256], f32, name="lp_pe", bufs=2)
                    sl = slice(hbl * 128, (hbl + 1) * 128)
                    for ch in range(2):
                        nc.tensor.matmul(pE[:], LT("cCi", ch)[:, sl],
                                         Gt[c, ch][:, 0:256], start=(ch == 0),
                                         stop=False)
                    for ch in range(2):
                        nc.tensor.matmul(pE[:], LT("cSq", ch)[:, sl],
                                         Gt[c, ch][:, 256:512], start=False,
                                         stop=(ch == 1))
                    t1 = ep.tile([128, 256], f32, name="lp_t1")
                    nc.vector.tensor_scalar(t1[:], pE[:], 0.0, 255.0,
                                            op0=ALU.max, op1=ALU.min)
                    xi = ep.tile([128, 256], mybir.dt.int32, name="lp_xi")
                    nc.vector.tensor_copy(xi[:], t1[:])
                    xr = ep.tile([128, 256], f32, name="lp_xr")
                    nc.vector.tensor_copy(xr[:], xi[:])
                    gt2 = ep.tile([128, 256], f32, name="lp_gt")
                    nc.vector.tensor_tensor(gt2[:], xr[:], t1[:], op=ALU.is_gt)
                    t2 = ep.tile([128, 256], bf, name="lp_t2")
                    nc.vector.tensor_tensor(t2[:], gt2[:], xr[:], op=ALU.subtract)
                    nc.sync.dma_start(C["lpn_d"][c, hbl * 128:(hbl + 1) * 128, :],
                                      t2[:])

    # ------------------------------------------------------------------
    def _wm_correction(self, tc, sc3, sh3):
        """x0 = relu(bn3(y3[0])); F0 = 9x9 DFT block of x0;
        delta = (wmvT - F0)*kapT; corr = Re(idft(delta))/N^2;
        x0c_d = x0 + corr."""
        nc = self.nc
        C = self._consts
        with ExitStack() as S:
            wp = S.enter_context(tc.tile_pool(name="wm", bufs=1))
            wpp = S.enter_context(tc.tile_pool(name="wm_ps", bufs=1, space="PSUM"))
            # broadcast sc3[0]/sh3[0] to 128 partitions via ones-matmul
            bc = []
            for vi, vec in enumerate((sc3, sh3)):
                prow = wpp.tile([1, 64], f32, name="wm_tr")
                nc.tensor.transpose(prow[:], vec[:], C["ident"][0:64, 0:64])
                row = wp.tile([1, 64], f32, name=f"wm_row{vi}")
                nc.vector.tensor_copy(row[:], prow[:])
                pbc = wpp.tile([128, 1], f32, name="wm_bc")
                nc.tensor.matmul(pbc[:], C["ones1"][:], row[:, 0:1],
                                 start=True, stop=True)
                sb = wp.tile([128, 1], f32, name=f"wm_bcs{vi}")
                nc.vector.tensor_copy(sb[:], pbc[:])
                bc.append(sb)
            sc_bc, sh_bc = bc
            # x0 tiles
            xt0 = []
            for hb in range(2):
                t = wp.tile([128, 256], bf, name=f"wm_x{hb}")
                nc.sync.dma_start(t[:], C["y"][3][0, hb * 128:(hb + 1) * 128, :])
                nc.scalar.activation(t[:], t[:], FT.Relu, bias=sh_bc[:, 0:1],
                                     scale=sc_bc[:, 0:1])
                xt0.append(t)
            # T[18, 256] = E1^T @ x0
            pT = wpp.tile([18, 256], f32, name="wm_pT")
            for hb in range(2):
                nc.tensor.matmul(pT[:], C["cE1"][hb][:], xt0[hb][:],
                                 start=(hb == 0), stop=(hb == 1))
            Tsb = wp.tile([18, 256], f32, name="wm_T")
            nc.vector.tensor_copy(Tsb[:], pT[:])
            # transpose T -> Tt [128,18] x2 (bf16)
            Ttsb = []
            for hc in range(2):
                pTt = wpp.tile([128, 18], f32, name="wm_pTt")
                nc.tensor.transpose(pTt[:], Tsb[:, hc * 128:(hc + 1) * 128],
                                    C["ident"][0:18, 0:18])
                t = wp.tile([128, 18], bf, name=f"wm_Tt{hc}")
                nc.vector.tensor_copy(t[:], pTt[:])
                Ttsb.append(t)
            # F0 combos [18, 18] = Tt^T @ E2
            pF = wpp.tile([18, 18], f32, name="wm_pF")
            for hc in range(2):
                nc.tensor.matmul(pF[:], Ttsb[hc][:], C["cE2"][hc][:],
                                 start=(hc == 0), stop=(hc == 1))
            Fsb = wp.tile([18, 18], f32, name="wm_F")
            nc.vector.tensor_copy(Fsb[:], pF[:])
            Fsh = wp.tile([9, 18], f32, name="wm_Fsh")
            nc.sync.dma_start(Fsh[:], Fsb[9:18, :])
            # F0_re = Fsb[0:9,0:9] + Fsh[:,9:18]; F0_im = Fsh[:,0:9] - Fsb[0:9,9:18]
            F0re = wp.tile([9, 9], f32, name="wm_F0re")
            nc.vector.tensor_tensor(F0re[:], Fsb[0:9, 0:9], Fsh[:, 9:18], op=ALU.add)
            F0im = wp.tile([9, 9], f32, name="wm_F0im")
            nc.vector.tensor_tensor(F0im[:], Fsh[:, 0:9], Fsb[0:9, 9:18],
                                    op=ALU.subtract)
            # delta = (wmvT - F0) * kapT  -> Dstack [18, 9] bf16
            Dstack = wp.tile([18, 9], bf, name="wm_D")
            dre = wp.tile([9, 9], f32, name="wm_dre")
            nc.vector.tensor_tensor(dre[:], C["wmvT"][:], F0re[:], op=ALU.subtract)
            nc.vector.tensor_tensor(Dstack[0:9, :], dre[:], C["kapT"][:], op=ALU.mult)
            dim = wp.tile([9, 9], f32, name="wm_dim")
            nc.vector.tensor_tensor(dim[:], C["wmvT"][:], F0im[:], op=ALU.subtract)
            dimk = wp.tile([9, 9], bf, name="wm_dimk")
            nc.vector.tensor_tensor(dimk[:], dim[:], C["kapT"][:], op=ALU.mult)
            nc.sync.dma_start(Dstack[9:18, :], dimk[:])
            # M rows: Mst [18, 256] bf16 = [M_re; M_im]
            Mst = wp.tile([18, 256], bf, name="wm_Mst")
            pMre = wpp.tile([9, 256], f32, name="wm_pM")
            nc.tensor.matmul(pMre[:], Dstack[:], C["cE3"][:], start=True, stop=True)
            nc.vector.tensor_copy(Mst[0:9, :], pMre[:])
            pMim = wpp.tile([9, 256], f32, name="wm_pM")
            nc.tensor.matmul(pMim[:], Dstack[:], C["cE4"][:], start=True, stop=True)
            mtmp = wp.tile([9, 256], bf, name="wm_mtmp")
            nc.vector.tensor_copy(mtmp[:], pMim[:])
            nc.sync.dma_start(Mst[9:18, :], mtmp[:])
            # corr chunks + add x0 -> x0c_d
            for hc in range(2):
                pC = wpp.tile([128, 256], f32, name="wm_pC")
                nc.tensor.matmul(pC[:], Mst[:, hc * 128:(hc + 1) * 128],
                                 C["cE5"][:], start=True, stop=True)
                cb = wp.tile([128, 256], bf, name="wm_cb")
                nc.vector.tensor_copy(cb[:], pC[:])
                xo = wp.tile([128, 256], bf, name="wm_xo")
                nc.vector.tensor_tensor(xo[:], xt0[hc][:], cb[:], op=ALU.add)
                nc.sync.dma_start(C["x0c_d"][hc * 128:(hc + 1) * 128, :], xo[:])
            ap = self.maybe_debug("x0c", (H, W))
            if ap is not None:
                xf = wp.tile([128, 256], f32, name="wm_xf")
                for hc in range(2):
                    t = wp.tile([128, 256], bf, name="wm_rb")
                    nc.sync.dma_start(t[:], C["x0c_d"][hc * 128:(hc + 1) * 128, :])
                    nc.vector.tensor_copy(xf[:], t[:])
                    nc.sync.dma_start(ap[hc * 128:(hc + 1) * 128, :], xf[:])

    # ------------------------------------------------------------------
    def _build_body(self, X, tc):
        nc = self.nc
        C = self._consts
        d = self.d

        # ============ low-pass filter + image bf16 cast ============
        self._lowpass(tc)

        # ============ conv0..3 ============
        sc, sh = self._conv0(X, tc)
        for k in range(1, 4):
            sc, sh = self._conv64(X, tc, k, C["y"][k - 1], C["y"][k], sc, sh)

        # ============ watermark correction (channel 0) ============
        self._wm_correction(tc, sc, sh)

        # ============ ac conv ============
        sc4, sh4 = self._conv64(X, tc, 4, C["y"][3], C["y"][4], sc, sh)

        # ============ final 1x1 conv ============
        with ExitStack() as S:
            bp = S.enter_context(tc.tile_pool(name="fin_b", bufs=3))
            pp = S.enter_context(tc.tile_pool(name="fin_ps", bufs=4, space="PSUM"))
            ep = S.enter_context(tc.tile_pool(name="fin_ev", bufs=3))
            fb6 = C["cp"].tile([6, 1], f32, name="fb6")
            nc.sync.dma_start(fb6[0:3, :], d["fb"][:])
            nc.sync.dma_start(fb6[3:6, :], d["fb"][:])
            sc128 = C["cp"].tile([128, 1], f32, name="fin_sc128")
            sh128 = C["cp"].tile([128, 1], f32, name="fin_sh128")
            for half in range(2):
                nc.sync.dma_start(sc128[64 * half:64 * half + 64, :], sc4[:])
                nc.sync.dma_start(sh128[64 * half:64 * half + 64, :], sh4[:])
            FR = 16
            for q in range(0, 128, FR):
                xf = bp.tile([128, FR, 256], bf, name="fin_x")
                nc.sync.dma_start(xf[0:64, :, :], C["y"][4][:, q:q + FR, :])
                nc.sync.dma_start(xf[64:128, :, :],
                                  C["y"][4][:, 128 + q:128 + q + FR, :])
                nc.scalar.activation(xf[:].rearrange("p r c -> p (r c)"),
                                     xf[:].rearrange("p r c -> p (r c)"),
                                     FT.Relu, bias=sh128[:, 0:1], scale=sc128[:, 0:1])
                osb = ep.tile([6, FR * 256], f32, name="fin_o")
                for rr in range(0, FR, 2):
                    pt = pp.tile([6, 512], f32, name="fin_p")
                    nc.tensor.matmul(pt[:], C["lhsT_fin"][:],
                                     xf[:, rr:rr + 2, :].rearrange("p r c -> p (r c)"),
                                     start=True, stop=True)
                    nc.vector.tensor_scalar(osb[:, rr * 256:(rr + 2) * 256], pt[:],
                                            fb6[:, 0:1], None, op0=ALU.add)
                nc.sync.dma_start(d["out"][:, q:q + FR, :],
                                  osb[0:3, :].rearrange("p (r c) -> p r c", c=256))
                nc.sync.dma_start(d["out"][:, 128 + q:128 + q + FR, :],
                                  osb[3:6, :].rearrange("p (r c) -> p r c", c=256))

        # debug outputs
        for nm, src, shp in [("lpn", C["lpn_d"], (3, H, W)),
                             ("y0", C["y"][0], (64, H, W)),
                             ("y1", C["y"][1], (64, H, W)),
                             ("y2", C["y"][2], (64, H, W)),
                             ("y3", C["y"][3], (64, H, W)),
                             ("y4", C["y"][4], (64, H, W))]:
            ap = self.maybe_debug(nm, shp, dt=bf)
            if ap is not None:
                nc.sync.dma_start(ap[:], src[:])


# ======================================================================
# harness entry point: full inputs in, full outputs out (8 cores SPMD)
# ======================================================================
from concourse.bass_utils import run_bass_kernel_spmd

_ENC = None

def _get_enc():
    global _ENC
    if _ENC is None:
        e = Enc(n_cores=8)
        e.build()
        _ENC = e
    return _ENC

def make_in_maps(inputs):
    consts = host_constants()
    g = lambda k: np.ascontiguousarray(np.asarray(inputs[k], dtype=np.float32))
    image, message = g("image"), g("message")
    shared = dict(
        w0=g("w0"), b0=g("b0").reshape(64, 1), g0=g("g0").reshape(64, 1),
        be0=g("be0").reshape(64, 1), ws=g("ws"), bs=g("bs").reshape(3, 64, 1),
        gs=g("gs").reshape(3, 64, 1), bes=g("bes").reshape(3, 64, 1),
        acw=g("acw"), acb=g("acb").reshape(64, 1), acg=g("acg").reshape(64, 1),
        acbe=g("acbe").reshape(64, 1), fw=np.ascontiguousarray(g("fw")[:, :, 0, 0]),
        fb=g("fb").reshape(3, 1), **consts)
    return [dict(image=np.ascontiguousarray(image[i]),
                 message=np.ascontiguousarray(message[i].reshape(MSG, 1)),
                 **shared) for i in range(8)]

def kernel(**inputs):
    e = _get_enc()
    in_maps = make_in_maps(inputs)
    res = run_bass_kernel_spmd(e.nc, in_maps, core_ids=list(range(8)))
    out = np.stack([res.results[i]["out"] for i in range(8)], axis=0)
    return np.ascontiguousarray(out.astype(np.float32))


# revision 25
# speedup vs baseline: 1.6871x; 1.2766x over previous
"""Encoder kernel: nn_Encoder (watermark encoder) on 8 TRN2 cores,
data-parallel over batch (1 image per core).

Key structure vs the straightforward version:
  - watermark: ifft2(fft2(x).at[pos].set(v)) == x for channels 1..63;
    channel 0 gets x0 + Re(idft(delta)) with delta nonzero on a 9x9
    frequency block -> tiny matmuls instead of a 64-channel FFT chain.
  - convs in bf16 (weights + moving operands) -> FWL weight loads.
  - ac conv: 30 constant message channels folded into the bias with
    edge-mask corrections; image channels packed conv0-style.
Layouts per core:
  y_k DRAM [64, 256, 256] bf16 (pre-BN, conv bias added)
  lpn DRAM [3, 256, 256] bf16  (holds -floor(clip(low,0,255)); conv0
    weights negated/255)
  imgb DRAM [3, 256, 256] bf16 (image cast)
  x0c DRAM [256, 256] bf16     (bn-relu'd x0 + watermark correction)
"""
import numpy as np
import concourse.bass as bass
import concourse.tile as tile
from concourse import bacc, mybir
from contextlib import ExitStack

f32 = mybir.dt.float32
f32r = mybir.dt.float32r
bf = mybir.dt.bfloat16
FT = mybir.ActivationFunctionType
ALU = mybir.AluOpType

H = W = 256
HW = H * W
CH = 64
MSG = 30
WP = W + 2          # padded row length 258
R = 32              # conv band rows
NBAND = H // R      # 8

# ---------------------------------------------------------------- host consts
def host_constants():
    j = np.arange(256)
    ang = 2.0 * np.pi * np.outer(j, j) / 256.0
    C = np.cos(ang).astype(np.float32)
    S = (-np.sin(ang)).astype(np.float32)      # F = C + iS
    Ci = (C / 256.0).astype(np.float32)
    Sq = (S / 256.0).astype(np.float32)
    consts = {
        "cC": C, "cS": S, "cNS": -S, "cCi": Ci, "cSq": Sq, "cNSq": -Sq,
        "cCS": np.hstack([C, S]).astype(np.float32),
        "cIdent": np.eye(128, dtype=np.float32),
    }
    yy = np.arange(H, dtype=np.float32) - H // 2
    xx = np.arange(W, dtype=np.float32) - W // 2
    m = ((yy[:, None] ** 2 + xx[None, :] ** 2) <= float(50 * 50)).astype(np.float32)
    consts["cMask"] = np.fft.ifftshift(m).astype(np.float32)
    # watermark positions (all in channel 0; block kh,kw in 124..132)
    cy = cx = 128
    rr, cc = [], []
    idx = 0
    for i in range(-4, 5):
        for j2 in range(-4, 5):
            if idx >= MSG:
                break
            if (i * i + j2 * j2) ** 0.5 <= 4:
                rr.append(cy + i); cc.append(cx + j2); idx += 1
    rr = np.array(rr); cc = np.array(cc)   # rr = kh (axis -2), cc = kw (axis -1)
    pk = np.zeros((MSG, 9), np.float32)    # kw indicator
    fk = np.zeros((MSG, 9), np.float32)    # kh indicator
    kapT = np.zeros((9, 9), np.float32)    # [kh, kw] position mask
    for k in range(MSG):
        pk[k, cc[k] - 124] = 1.0
        fk[k, rr[k] - 124] = 1.0
        kapT[rr[k] - 124, cc[k] - 124] = 1.0
    consts["cPk"] = pk
    consts["cFk"] = fk
    consts["cKapT"] = kapT
    # wm DFT block matrices (bf16 on host; kh,kw in 124..132)
    import ml_dtypes
    th = 2.0 * np.pi / 256.0
    K9 = np.arange(124, 133, dtype=np.float64)
    hh = np.arange(256, dtype=np.float64)
    A = th * np.outer(hh, K9)            # [256, 9] angle(h, k)
    cosA, sinA = np.cos(A), np.sin(A)
    bfc = lambda x: np.ascontiguousarray(x).astype(ml_dtypes.bfloat16)
    consts["cE1"] = bfc(np.hstack([cosA, -sinA]))            # [256,18] fwd rows
    consts["cE2"] = bfc(np.hstack([cosA, sinA]))             # [256,18] fwd cols
    consts["cE3"] = bfc(np.hstack([cosA, -sinA]).T)          # [18,256] inv M_re
    consts["cE4"] = bfc(np.hstack([sinA, cosA]).T)           # [18,256] inv M_im
    consts["cE5"] = bfc(np.hstack([cosA / 65536.0, -sinA / 65536.0]).T)
    return consts


# ---------------------------------------------------------------- builders
class Enc:
    def __init__(self, n_cores=8, debug_outs=()):
        self.n_cores = n_cores
        self.ntot = float(n_cores * HW)
        self.debug_outs = debug_outs
        nc = bacc.Bacc("TRN2", target_bir_lowering=False, debug=False,
                       num_devices=n_cores)
        self.nc = nc
        d = {}
        d["image"] = nc.dram_tensor("image", (3, H, W), f32, kind="ExternalInput").ap()
        d["message"] = nc.dram_tensor("message", (MSG, 1), f32, kind="ExternalInput").ap()
        d["w0"] = nc.dram_tensor("w0", (64, 3, 3, 3), f32, kind="ExternalInput").ap()
        d["b0"] = nc.dram_tensor("b0", (64, 1), f32, kind="ExternalInput").ap()
        d["g0"] = nc.dram_tensor("g0", (64, 1), f32, kind="ExternalInput").ap()
        d["be0"] = nc.dram_tensor("be0", (64, 1), f32, kind="ExternalInput").ap()
        d["ws"] = nc.dram_tensor("ws", (3, 64, 64, 3, 3), f32, kind="ExternalInput").ap()
        d["bs"] = nc.dram_tensor("bs", (3, 64, 1), f32, kind="ExternalInput").ap()
        d["gs"] = nc.dram_tensor("gs", (3, 64, 1), f32, kind="ExternalInput").ap()
        d["bes"] = nc.dram_tensor("bes", (3, 64, 1), f32, kind="ExternalInput").ap()
        d["acw"] = nc.dram_tensor("acw", (64, 97, 3, 3), f32, kind="ExternalInput").ap()
        d["acb"] = nc.dram_tensor("acb", (64, 1), f32, kind="ExternalInput").ap()
        d["acg"] = nc.dram_tensor("acg", (64, 1), f32, kind="ExternalInput").ap()
        d["acbe"] = nc.dram_tensor("acbe", (64, 1), f32, kind="ExternalInput").ap()
        d["fw"] = nc.dram_tensor("fw", (3, 64), f32, kind="ExternalInput").ap()
        d["fb"] = nc.dram_tensor("fb", (3, 1), f32, kind="ExternalInput").ap()
        for k, shp, dt in [("cC", (256, 256), f32), ("cS", (256, 256), f32),
                           ("cNS", (256, 256), f32), ("cCi", (256, 256), f32),
                           ("cSq", (256, 256), f32), ("cNSq", (256, 256), f32),
                           ("cCS", (256, 512), f32), ("cIdent", (128, 128), f32),
                           ("cMask", (256, 256), f32),
                           ("cPk", (MSG, 9), f32), ("cFk", (MSG, 9), f32),
                           ("cKapT", (9, 9), f32),
                           ("cE1", (256, 18), bf), ("cE2", (256, 18), bf),
                           ("cE3", (18, 256), bf), ("cE4", (18, 256), bf),
                           ("cE5", (18, 256), bf)]:
            d[k] = nc.dram_tensor(k, shp, dt, kind="ExternalInput").ap()
        d["out"] = nc.dram_tensor("out", (3, H, W), f32, kind="ExternalOutput").ap()
        self.d = d
        self.dbg = {}

    def maybe_debug(self, name, shape, dt=f32):
        if name in self.debug_outs:
            self.dbg[name] = self.nc.dram_tensor(
                "dbg_" + name, shape, dt, kind="ExternalOutput").ap()
            return self.dbg[name]
        return None

    # ------------------------------------------------------------------
    def build(self):
        nc, d = self.nc, self.d
        with tile.TileContext(nc) as tc, ExitStack() as X:
            cp = X.enter_context(tc.tile_pool(name="consts", bufs=1))
            dp = X.enter_context(tc.tile_pool(name="dram", bufs=1, space="DRAM"))
            pwX = ExitStack()
            pw = pwX.enter_context(tc.tile_pool(name="pw", bufs=2, space="PSUM"))
            wev = pwX.enter_context(tc.tile_pool(name="wev", bufs=2))

            # ---------------- DRAM scratch
            y = [dp.tile([64, H, W], bf, name=f"yact{k}") for k in range(5)]
            lpn_d = dp.tile([3, H, W], bf, name="lpn_d")
            imgb_d = dp.tile([3, H, W], bf, name="imgb_d")
            x0c_d = dp.tile([H, W], bf, name="x0c_d")
            cl_in = [[dp.tile([64, 2], f32, name=f"clin{k}_{p}") for p in range(2)]
                     for k in range(5)]
            cl_out = [[dp.tile([64, 2], f32, name=f"clout{k}_{p}",
                               addr_space="Shared") for p in range(2)]
                      for k in range(5)]

            # ---------------- constants into SBUF
            def cload(name, src, shape, dtype):
                t = cp.tile(shape, dtype, name=name)
                nc.sync.dma_start(t[:], src)
                return t
            DF = {}
            for nm, wdt in [("cC", 256), ("cS", 256), ("cNS", 256), ("cCi", 256),
                            ("cSq", 256), ("cNSq", 256), ("cCS", 512)]:
                DF[nm + "_hi"] = cload(nm + "_hi", d[nm][0:128, :].bitcast(f32r),
                                       [128, wdt], f32r)
                DF[nm + "_lo"] = cload(nm + "_lo", d[nm][128:256, :].bitcast(f32r),
                                       [128, wdt], f32r)
            ident = cload("ident", d["cIdent"][:], [128, 128], f32)
            maskt = [cload(f"maskt{i}", d["cMask"][i * 128:(i + 1) * 128, :],
                           [128, 256], f32) for i in range(2)]
            pk_t = cload("pk_t", d["cPk"][:], [MSG, 9], f32)
            fk_t = cload("fk_t", d["cFk"][:], [MSG, 9], f32)
            kapT_t = cload("kapT_t", d["cKapT"][:], [9, 9], f32)
            msg_t = cload("msg_t", d["message"][:], [MSG, 1], f32)
            cE1 = [cload(f"cE1_{i}", d["cE1"][i * 128:(i + 1) * 128, :], [128, 18], bf)
                   for i in range(2)]
            cE2 = [cload(f"cE2_{i}", d["cE2"][i * 128:(i + 1) * 128, :], [128, 18], bf)
                   for i in range(2)]
            cE3 = cload("cE3", d["cE3"][:], [18, 256], bf)
            cE4 = cload("cE4", d["cE4"][:], [18, 256], bf)
            cE5 = cload("cE5", d["cE5"][:], [18, 256], bf)
            eps64 = cp.tile([64, 1], f32, name="eps64")
            nc.vector.memset(eps64[:], 1e-5)
            zero128 = cp.tile([128, 1], f32, name="zero128")
            nc.vector.memset(zero128[:], 0.0)
            ones1 = cp.tile([1, 128], f32, name="ones1")
            nc.vector.memset(ones1[:], 1.0)
            msg_b = cp.tile([MSG, 1], bf, name="msg_b")
            nc.vector.tensor_copy(msg_b[:], msg_t[:])

            # per-layer bn param tiles
            def vload(name, src):
                t = cp.tile([64, 1], f32, name=name)
                nc.sync.dma_start(t[:], src)
                return t
            g_t = [vload("g_t0", d["g0"][:])] + \
                  [vload(f"g_t{k+1}", d["gs"][k]) for k in range(3)] + \
                  [vload("g_t4", d["acg"][:])]
            be_t = [vload("be_t0", d["be0"][:])] + \
                   [vload(f"be_t{k+1}", d["bes"][k]) for k in range(3)] + \
                   [vload("be_t4", d["acbe"][:])]
            acb_t = vload("acb_t", d["acb"][:])
            # conv bias replicated to 128 partitions; b128[4] (ac) filled later
            b128 = []
            for k, src in enumerate([d["b0"], d["bs"][0], d["bs"][1], d["bs"][2]]):
                t = cp.tile([128, 1], f32, name=f"b128_{k}")
                nc.sync.dma_start(t[0:64, :], src)
                nc.sync.dma_start(t[64:128, :], src)
                b128.append(t)
            b128.append(cp.tile([128, 1], f32, name="b128_4"))
            fb_t = cp.tile([3, 1], f32, name="fb_t")
            nc.sync.dma_start(fb_t[:], d["fb"][:])

            # ---------------- weight transposes (lhsT prep), all bf16
            # conv1..3: pair lhsT [128,128] x3(dw), cross lhsT [64,128] x3
            lhsT_pair, lhsT_sing = [], []
            for k in range(3):
                wsrc = cp.tile([64, 64, 9], f32, name=f"wsrc{k}")
                nc.sync.dma_start(wsrc[:], d["ws"][k].rearrange("o i a b -> o i (a b)"))
                pairs, sings = self._make_pairs64(pw, cp, wsrc, ident, f"c{k}")
                lhsT_pair.append(pairs); lhsT_sing.append(sings)
            # ac conv enc part (input ch 30:94) -> same pair structure
            acsrc = cp.tile([64, 97, 9], f32, name="acsrc")
            nc.sync.dma_start(acsrc[:], d["acw"][:].rearrange("o i a b -> o i (a b)"))
            ac_pairs, ac_sings = self._make_pairs64(
                pw, cp, acsrc, ident, "ac", in_off=30)
            # full acw transposes for aux/msg handling: acT_sb[tap] [97,64] bf16
            acT_sb = []
            for tap in range(9):
                p = pw.tile([97, 64], f32, name="pwa", bufs=2)
                nc.tensor.transpose(p[:], acsrc[:, :, tap], ident[0:64, 0:64])
                t = cp.tile([97, 64], bf, name=f"acT{tap}")
                nc.vector.tensor_copy(t[:], p[:])
                acT_sb.append(t)
            # msg-fold: M9[o, tap] = sum_c acw[o, c<30, tap] * msg[c]
            pM9 = pw.tile([64, 9], f32, name="pM9", bufs=1)
            for tap in range(9):
                nc.tensor.matmul(pM9[:, tap:tap + 1], acT_sb[tap][0:30, :],
                                 msg_b[:], start=True, stop=True)
            M9sb = cp.tile([64, 9], f32, name="M9sb")
            nc.vector.tensor_copy(M9sb[:], pM9[:])
            # bias_eff = acb + sum_j M9[:, j]  -> b128[4]
            bsum = wev.tile([64, 1], f32, name="bsum")
            nc.vector.tensor_reduce(bsum[:], M9sb[:], axis=mybir.AxisListType.X,
                                    op=ALU.add)
            beff = cp.tile([64, 1], f32, name="beff")
            nc.vector.tensor_tensor(beff[:], acb_t[:], bsum[:], op=ALU.add)
            nc.sync.dma_start(b128[4][0:64, :], beff[:])
            nc.sync.dma_start(b128[4][64:128, :], beff[:])
            # edge sums: S_top=j0:3, S_bot=j6:9, S_left=j{0,3,6}, S_right=j{2,5,8}
            sb_top = cp.tile([64, 1], f32, name="sb_top")
            nc.vector.tensor_reduce(sb_top[:], M9sb[:, 0:3], axis=mybir.AxisListType.X,
                                    op=ALU.add)
            sb_bot = cp.tile([64, 1], f32, name="sb_bot")
            nc.vector.tensor_reduce(sb_bot[:], M9sb[:, 6:9], axis=mybir.AxisListType.X,
                                    op=ALU.add)
            s_lr = []
            for nm, js in [("sl", (0, 3, 6)), ("sr", (2, 5, 8))]:
                t0 = wev.tile([64, 1], f32, name=nm + "a")
                nc.vector.tensor_tensor(t0[:], M9sb[:, js[0]:js[0] + 1],
                                        M9sb[:, js[1]:js[1] + 1], op=ALU.add)
                t1 = cp.tile([64, 1], f32, name=nm)
                nc.vector.tensor_tensor(t1[:], t0[:], M9sb[:, js[2]:js[2] + 1],
                                        op=ALU.add)
                s_lr.append(t1)
            # bottom fixups need partition base 64: fixB [128,3]
            fixB = cp.tile([128, 3], f32, name="fixB")
            nc.sync.dma_start(fixB[64:128, 0:1], sb_bot[:])
            nc.sync.dma_start(fixB[64:128, 1:2], M9sb[:, 6:7])
            nc.sync.dma_start(fixB[64:128, 2:3], M9sb[:, 8:9])
            # aux lhsT [17,128] x3(dw): rows 0:15 img (sigma,ch), 15:17 -S_l/-S_r
            lhsT_aux = []
            for dw in range(3):
                la = cp.tile([17, 128], bf, name=f"lhsTaux{dw}")
                nc.vector.memset(la[:], 0.0)
                lhsT_aux.append(la)
            for dw in range(3):
                for sg in range(3):       # g0: tap dh=sg
                    nc.sync.dma_start(lhsT_aux[dw][sg * 3:sg * 3 + 3, 0:64],
                                      acT_sb[sg * 3 + dw][94:97, :])
                for sg in range(2, 5):    # g1: tap dh=sg-2
                    nc.sync.dma_start(lhsT_aux[dw][sg * 3:sg * 3 + 3, 64:128],
                                      acT_sb[(sg - 2) * 3 + dw][94:97, :])
            # mask rows: transpose S vectors to rows, negate, bf16
            for i, sv in enumerate(s_lr):
                prow = pw.tile([1, 64], f32, name="prow", bufs=1)
                nc.tensor.transpose(prow[:], sv[:], ident[0:64, 0:64])
                rowneg = wev.tile([1, 64], bf, name="rowneg")
                nc.vector.tensor_scalar_mul(rowneg[:], prow[:], -1.0)
                nc.sync.dma_start(lhsT_aux[1][15 + i:16 + i, 0:64], rowneg[:])
                nc.sync.dma_start(lhsT_aux[1][15 + i:16 + i, 64:128], rowneg[:])
            # conv0: lhsT0x[dw] [15,128] = -w0^T/255, (sigma,ch) rows
            w0src = cp.tile([64, 3, 9], f32, name="w0src")
            nc.sync.dma_start(w0src[:], d["w0"][:].rearrange("o i a b -> o i (a b)"))
            lhsT0 = []
            for dw in range(3):
                l0 = cp.tile([15, 128], bf, name=f"lhsT0{dw}")
                nc.vector.memset(l0[:], 0.0)
                lhsT0.append(l0)
            for dw in range(3):
                for dh in range(3):
                    p = pw.tile([64, 64], f32, name="pwt")
                    nc.tensor.transpose(p[0:3, :], w0src[:, :, dh * 3 + dw],
                                        ident[0:64, 0:64])
                    tmp0 = wev.tile([3, 64], bf, name="w0tmp")
                    nc.vector.tensor_scalar_mul(tmp0[:], p[0:3, :], -1.0 / 255.0)
                    nc.sync.dma_start(lhsT0[dw][dh * 3:dh * 3 + 3, 0:64], tmp0[:])
                    nc.sync.dma_start(lhsT0[dw][(dh + 2) * 3:(dh + 2) * 3 + 3, 64:128],
                                      tmp0[:])
            # final 1x1: lhsT_fin [128,6]
            fwsrc = cp.tile([3, 64], f32, name="fwsrc")
            nc.sync.dma_start(fwsrc[:], d["fw"][:])
            lhsT_fin = cp.tile([128, 6], bf, name="lhsT_fin")
            nc.vector.memset(lhsT_fin[:], 0.0)
            p = pw.tile([64, 64], f32, name="pwt")
            nc.tensor.transpose(p[:, 0:3], fwsrc[:], ident[0:3, 0:3])
            pbf = wev.tile([64, 3], bf, name="pbf")
            nc.vector.tensor_copy(pbf[:], p[:, 0:3])
            nc.sync.dma_start(lhsT_fin[0:64, 0:3], pbf[:])
            nc.sync.dma_start(lhsT_fin[64:128, 3:6], pbf[:])

            # watermark grid wmvT [kh, kw]: fk^T @ (pk * msg)
            msgc2 = cp.tile([MSG, 9], f32, name="msgc2")
            nc.vector.tensor_scalar(msgc2[:], pk_t[:], msg_t[:, 0:1], None,
                                    op0=ALU.mult)
            pwm = pw.tile([9, 9], f32, name="pwm", bufs=1)
            nc.tensor.matmul(pwm[:], fk_t[:], msgc2[:], start=True, stop=True)
            wmvT = cp.tile([9, 9], f32, name="wmvT")
            nc.vector.tensor_copy(wmvT[:], pwm[:])

            # ac aux band tiles (manual double-buffer) + mask columns.
            # Engine ops need 32-aligned partition bases; masks and odd-offset
            # zero fills go through base-0 staging tiles + DMA.
            ml_t = cp.tile([1, R + 2, WP], bf, name="ml_t")
            nc.vector.memset(ml_t[:].rearrange("p r c -> p (r c)"), 0.0)
            nc.vector.memset(ml_t[0:1, :, 1:2], 1.0)
            mr_t = cp.tile([1, R + 2, WP], bf, name="mr_t")
            nc.vector.memset(mr_t[:].rearrange("p r c -> p (r c)"), 0.0)
            nc.vector.memset(mr_t[0:1, :, 256:257], 1.0)
            zrow = cp.tile([3, 1, WP], bf, name="zrow")
            nc.vector.memset(zrow[:].rearrange("p r c -> p (r c)"), 0.0)
            aux_bufs = [cp.tile([17, R + 2, WP], bf, name=f"auxb{i}")
                        for i in range(2)]
            for t in aux_bufs:
                nc.vector.memset(t[:].rearrange("p r c -> p (r c)"), 0.0)
                nc.sync.dma_start(t[15:16, :, :], ml_t[:])
                nc.sync.dma_start(t[16:17, :, :], mr_t[:])

            self._consts = dict(DF=DF, ident=ident, maskt=maskt,
                                g_t=g_t, be_t=be_t, b128=b128, fb_t=fb_t,
                                lhsT_pair=lhsT_pair, lhsT_sing=lhsT_sing,
                                lhsT0=lhsT0, ac_pairs=ac_pairs, ac_sings=ac_sings,
                                lhsT_aux=lhsT_aux, lhsT_fin=lhsT_fin,
                                cE1=cE1, cE2=cE2, cE3=cE3, cE4=cE4, cE5=cE5,
                                kapT=kapT_t, wmvT=wmvT, ones1=ones1,
                                sb_top=sb_top, fixB=fixB, M9sb=M9sb,
                                eps64=eps64, zero128=zero128, cp=cp, zrow=zrow,
                                y=y, lpn_d=lpn_d, imgb_d=imgb_d, x0c_d=x0c_d,
                                aux_bufs=aux_bufs,
                                cl_in=cl_in, cl_out=cl_out)
            pwX.close()
            self._build_body(X, tc)
        self._dedupe_ldweights()
        self.nc.compile()

    def _dedupe_ldweights(self):
        """Drop InstLdweights that reload the exact weights already resident
        in the PE array (consecutive in the PE stream, no sync effects).
        All lhsT tiles in this kernel are write-once, so content is stable."""
        ndrop = 0
        for f in self.nc.m.functions:
            for b in f.blocks:
                keep = []
                last_key = None
                for ins in b.instructions:
                    nm = type(ins).__name__
                    eng = str(getattr(ins, "engine", None))
                    if eng == "EngineType.PE":
                        if nm == "InstLdweights":
                            si = ins.sync_info
                            clean = (si is None) or (
                                not si.on_wait and not si.on_update)
                            key = (str(ins.ins[0]), str(ins.tile_position),
                                   str(ins.perf_mode), str(ins.is_transpose))
                            if clean and key == last_key:
                                ndrop += 1
                                continue
                            last_key = key
                        elif nm in ("InstMatmult", "InstEventSemaphore",
                                    "InstNop", "InstNotify"):
                            pass
                        else:
                            last_key = None
                    keep.append(ins)
                b.instructions = keep
        self._ldw_dropped = ndrop

    # ------------------------------------------------------------------
    def _make_pairs64(self, pw, cp, wsrc, ident, tag, in_off=0):
        """Pair/cross lhsT tiles (bf16) for a 64->64 3x3 conv.
        wsrc [64, >=in_off+64, 9] f32 (o, i, tap)."""
        nc = self.nc
        pairs, sings = [], []
        for dw in range(3):
            pA = cp.tile([128, 128], bf, name=f"lTpA{tag}{dw}")
            pB = cp.tile([128, 128], bf, name=f"lTpB{tag}{dw}")
            cx = cp.tile([64, 128], bf, name=f"lTcx{tag}{dw}")
            nc.vector.memset(pA[:], 0.0)
            nc.vector.memset(pB[:], 0.0)
            tp3 = []
            for dh in range(3):
                p = pw.tile([64, 64], f32, name="pwt")
                nc.tensor.transpose(p[:], wsrc[:, in_off:in_off + 64, dh * 3 + dw],
                                    ident[0:64, 0:64])
                tp3.append(p)
            nc.vector.tensor_copy(pA[0:64, 0:64], tp3[0][:])
            nc.vector.tensor_copy(pA[64:128, 0:64], tp3[1][:])
            nc.vector.tensor_copy(pB[0:64, 64:128], tp3[1][:])
            nc.vector.tensor_copy(pB[64:128, 64:128], tp3[2][:])
            nc.vector.tensor_copy(cx[:, 0:64], tp3[2][:])
            nc.vector.tensor_copy(cx[:, 64:128], tp3[0][:])
            pairs.append((pA, pB)); sings.append(cx)
        return pairs, sings

    # ------------------------------------------------------------------
    def _stats_partial(self, pool, layer, part, ssum_cols, sqsum_cols, c0, c1):
        """Reduce stat columns [c0:c1), fold partition halves, AllReduce."""
        nc = self.nc
        C = self._consts
        red = pool.tile([128, 2], f32, name=f"red{layer}_{part}")
        nc.vector.tensor_reduce(red[:, 0:1], ssum_cols[:, c0:c1],
                                axis=mybir.AxisListType.X, op=ALU.add)
        nc.vector.tensor_reduce(red[:, 1:2], sqsum_cols[:, c0:c1],
                                axis=mybir.AxisListType.X, op=ALU.add)
        upper = pool.tile([64, 2], f32, name=f"up{layer}_{part}")
        nc.sync.dma_start(upper[:], red[64:128, :])
        stats = pool.tile([64, 2], f32, name=f"st{layer}_{part}")
        nc.vector.tensor_add(stats[:], red[0:64, :], upper[:])
        nc.sync.dma_start(C["cl_in"][layer][part][:], stats[:])
        nc.gpsimd.collective_compute(
            "AllReduce", ALU.add,
            replica_groups=[list(range(self.n_cores))],
            ins=[C["cl_in"][layer][part].opt()],
            outs=[C["cl_out"][layer][part].opt()])
        sr = pool.tile([64, 2], f32, name=f"sr{layer}_{part}")
        nc.sync.dma_start(sr[:], C["cl_out"][layer][part][:])
        return sr

    def _stats_combine(self, pool, layer, srA, srB):
        """sum partials -> fused scale/shift math."""
        nc = self.nc
        C = self._consts
        N = self.ntot
        s2 = pool.tile([64, 2], f32, name=f"s2{layer}")
        nc.vector.tensor_add(s2[:], srA[:], srB[:])
        msq = pool.tile([64, 1], f32, name=f"msq{layer}")
        nc.vector.tensor_scalar(msq[:], s2[:, 0:1], s2[:, 0:1], 1.0 / (N * N),
                                op0=ALU.mult, op1=ALU.mult)
        var = pool.tile([64, 1], f32, name=f"var{layer}")
        nc.vector.scalar_tensor_tensor(var[:], s2[:, 1:2], 1.0 / N, msq[:],
                                       op0=ALU.mult, op1=ALU.subtract)
        std = pool.tile([64, 1], f32, name=f"std{layer}")
        nc.scalar.activation(std[:], var[:], FT.Sqrt, bias=C["eps64"][:, 0:1],
                             scale=1.0)
        istd = pool.tile([64, 1], f32, name=f"istd{layer}")
        nc.vector.reciprocal(istd[:], std[:])
        scale = C["cp"].tile([64, 1], f32, name=f"scale{layer}")
        nc.vector.tensor_tensor(scale[:], C["g_t"][layer][:], istd[:], op=ALU.mult)
        prodneg = pool.tile([64, 1], f32, name=f"pn{layer}")
        nc.vector.scalar_tensor_tensor(prodneg[:], s2[:, 0:1], -1.0 / N, scale[:],
                                       op0=ALU.mult, op1=ALU.mult)
        shift = C["cp"].tile([64, 1], f32, name=f"shift{layer}")
        nc.vector.tensor_tensor(shift[:], prodneg[:], C["be_t"][layer][:],
                                op=ALU.add)
        ap = self.maybe_debug(f"sc{layer}", (64, 1))
        if ap is not None:
            nc.sync.dma_start(ap[:], scale[:])
        ap = self.maybe_debug(f"sh{layer}", (64, 1))
        if ap is not None:
            nc.sync.dma_start(ap[:], shift[:])
        return scale, shift

    # ------------------------------------------------------------------
    def _evict(self, pp_t, ep, sp_cols, layer, cidx, dst, q):
        """PSUM chunk -> +bias (scalar, ssum accum) -> bf16 -> DRAM;
        sqsum via vector tensor_tensor_reduce."""
        nc = self.nc
        C = self._consts
        ysb = ep.tile([128, 512], bf, name=f"ysb{layer}")
        nc.vector.tensor_scalar(ysb[:], pp_t[:], C["b128"][layer][:, 0:1],
                                0.0, op0=ALU.add, op1=ALU.add,
                                accum_out=sp_cols[0][:, cidx:cidx + 1])
        scr = ep.tile([128, 512], bf, name=f"sqscr{layer}")
        nc.scalar.activation(scr[:], ysb[:], FT.Square,
                             bias=C["zero128"][:, 0:1],
                             accum_out=sp_cols[1][:, cidx:cidx + 1])
        nc.sync.dma_start(dst[:, q:q + 2, :],
                          ysb[0:64, :].rearrange("p (r c) -> p r c", c=256))
        nc.sync.dma_start(dst[:, q + 2:q + 4, :],
                          ysb[64:128, :].rearrange("p (r c) -> p r c", c=256))

    # ------------------------------------------------------------------
    def _conv64(self, X, tc, layer, src, dst, scale, shift):
        """conv layers 1..3 (64->64) and 4 (ac: 64 enc + aux)."""
        nc = self.nc
        C = self._consts
        is_ac = (layer == 4)
        pairs = C["ac_pairs"] if is_ac else C["lhsT_pair"][layer - 1]
        sings = C["ac_sings"] if is_ac else C["lhsT_sing"][layer - 1]
        with ExitStack() as S:
            bp = S.enter_context(tc.tile_pool(name=f"band{layer}", bufs=4))
            pp = S.enter_context(tc.tile_pool(name=f"psum{layer}", bufs=8,
                                              space="PSUM"))
            ep = S.enter_context(tc.tile_pool(name=f"evict{layer}", bufs=3))
            sp = S.enter_context(tc.tile_pool(name=f"stat{layer}", bufs=1))
            ssum_cols = sp.tile([128, 64], f32, name=f"ssc{layer}")
            sqsum_cols = sp.tile([128, 64], f32, name=f"sqc{layer}")
            cidx = 0
            for bi in range(NBAND):
                r0 = bi * R
                band = bp.tile([128, (R + 2) * WP], bf, name=f"bandt{layer}")
                b3 = band[:].rearrange("p (r c) -> p r c", c=WP)
                i0 = 1 if bi == 0 else 0
                i1 = R + 1 if bi == NBAND - 1 else R + 2
                rl, rh = r0 - 1 + i0, r0 - 1 + i1
                nc.vector.memset(b3[0:64, :, 0:1], 0.0)
                nc.vector.memset(b3[0:64, :, 257:258], 0.0)
                if bi == 0:
                    nc.vector.memset(b3[0:64, 0:1, :], 0.0)
                if bi == NBAND - 1:
                    nc.vector.memset(b3[0:64, R + 1:R + 2, :], 0.0)
                nc.sync.dma_start(b3[0:64, i0:i1, 1:257], src[:, rl:rh, :])
                nc.scalar.activation(b3[0:64, i0:i1, 1:257],
                                     b3[0:64, i0:i1, 1:257],
                                     FT.Relu, bias=shift[:, 0:1],
                                     scale=scale[:, 0:1])
                if is_ac:
                    # ch0 = x0corr (already bn-relu'd + wm corr): overwrite
                    nc.sync.dma_start(b3[0:1, i0:i1, 1:257], C["x0c_d"][rl:rh, :])
                nc.sync.dma_start(b3[64:128, 0:R + 1, :], b3[0:64, 1:R + 2, :])
                if is_ac:
                    aux = C["aux_bufs"][bi % 2]
                    # img partitions (sigma,ch): band row i <- img row r0-1+i+sg
                    for sg in range(5):
                        a0 = 1 if (bi == 0 and sg == 0) else 0
                        a1 = min(30, 257 - r0 - sg)
                        if a1 <= a0:
                            continue
                        nc.sync.dma_start(
                            aux[sg * 3:sg * 3 + 3, a0:a1, 1:257],
                            C["imgb_d"][:, r0 - 1 + a0 + sg:r0 - 1 + a1 + sg, :])
                    if bi == NBAND - 1:
                        nc.sync.dma_start(aux[12:15, 29:30, :], C["zrow"][:])
                wspecs = []
                for dw in range(3):
                    pA, pB = pairs[dw]
                    wspecs.append((pA[:],
                                   lambda t, dw=dw: b3[0:128, t:t + 2, dw:dw + 256]))
                    wspecs.append((pB[:],
                                   lambda t, dw=dw: b3[0:128, t + 3:t + 5, dw:dw + 256]))
                    wspecs.append((sings[dw][:],
                                   lambda t, dw=dw: b3[0:64, t + 2:t + 4, dw:dw + 256]))
                    if is_ac:
                        npa = 17 if dw == 1 else 15
                        wspecs.append((C["lhsT_aux"][dw][0:npa, :],
                                       lambda t, dw=dw, npa=npa:
                                       aux[0:npa, t:t + 2, dw:dw + 256]))
                nW = len(wspecs)
                for half in range(2):
                    tlist = list(range(half * 16, half * 16 + 16, 4))
                    pts = [pp.tile([128, 512], f32, name=f"pchunk{layer}")
                           for _ in tlist]
                    for wi, (lh, mf) in enumerate(wspecs):
                        for ti, t in enumerate(tlist):
                            nc.tensor.matmul(pts[ti][:], lh, mf(t),
                                             start=(wi == 0), stop=(wi == nW - 1))
                    for ti, t in enumerate(tlist):
                        pt = pts[ti]
                        self._conv_fixups(pt, is_ac, bi, t)
                        self._evict(pt, ep, (ssum_cols, sqsum_cols), layer, cidx,
                                    dst, r0 + t)
                        cidx += 1
                if bi == NBAND - 2:
                    srA = self._stats_partial(sp, layer, 0, ssum_cols,
                                              sqsum_cols, 0, cidx)
                    csplit = cidx
            srB = self._stats_partial(sp, layer, 1, ssum_cols, sqsum_cols,
                                      csplit, cidx)
            return self._stats_combine(sp, layer, srA, srB)

    # ------------------------------------------------------------------
    def _conv_fixups(self, pt, is_ac, bi, t):
        """ac-layer msg-fold border fixups on the PSUM chunk."""
        nc = self.nc
        C = self._consts
        if is_ac and bi == 0 and t == 0:
            # row 0 fixups: -S_top everywhere, +A_tl/+A_tr corners
            nc.vector.tensor_scalar(pt[0:64, 0:256], pt[0:64, 0:256],
                                    C["sb_top"][:, 0:1], None, op0=ALU.subtract)
            nc.vector.tensor_tensor(pt[0:64, 0:1], pt[0:64, 0:1],
                                    C["M9sb"][:, 0:1], op=ALU.add)
            nc.vector.tensor_tensor(pt[0:64, 255:256], pt[0:64, 255:256],
                                    C["M9sb"][:, 2:3], op=ALU.add)
        if is_ac and bi == NBAND - 1 and t == R - 4:
            nc.vector.tensor_scalar(pt[64:128, 256:512], pt[64:128, 256:512],
                                    C["fixB"][64:128, 0:1], None, op0=ALU.subtract)
            nc.vector.tensor_tensor(pt[64:128, 256:257], pt[64:128, 256:257],
                                    C["fixB"][64:128, 1:2], op=ALU.add)
            nc.vector.tensor_tensor(pt[64:128, 511:512], pt[64:128, 511:512],
                                    C["fixB"][64:128, 2:3], op=ALU.add)

    # ------------------------------------------------------------------
    def _conv0(self, X, tc):
        """conv0: lpn(3ch, negated/255) -> y0. (sigma,ch) 15-partition bands."""
        nc = self.nc
        C = self._consts
        with ExitStack() as S:
            bp = S.enter_context(tc.tile_pool(name="band0", bufs=3))
            pp = S.enter_context(tc.tile_pool(name="psum0", bufs=8, space="PSUM"))
            ep = S.enter_context(tc.tile_pool(name="evict0", bufs=3))
            sp = S.enter_context(tc.tile_pool(name="stat0", bufs=1))
            ssum_cols = sp.tile([128, 64], f32, name="ssc0")
            sqsum_cols = sp.tile([128, 64], f32, name="sqc0")
            cidx = 0
            for bi in range(NBAND):
                r0 = bi * R
                band = bp.tile([15, (R + 2) * WP], bf, name="bandt0")
                b3 = band[:].rearrange("p (r c) -> p r c", c=WP)
                nc.vector.memset(b3[:, :, 0:1], 0.0)
                nc.vector.memset(b3[:, :, 257:258], 0.0)
                if bi == 0:
                    nc.vector.memset(b3[0:3, 0:1, :], 0.0)
                if bi == NBAND - 1:
                    nc.sync.dma_start(b3[12:15, 29:30, :], C["zrow"][:])
                for sg in range(5):
                    a0 = 1 if (bi == 0 and sg == 0) else 0
                    a1 = min(30, 256 - r0 - sg + 1)
                    if a1 <= a0:
                        continue
                    nc.sync.dma_start(
                        b3[sg * 3:sg * 3 + 3, a0:a1, 1:257],
                        C["lpn_d"][:, r0 - 1 + a0 + sg:r0 - 1 + a1 + sg, :])
                for half in range(2):
                    tlist = list(range(half * 16, half * 16 + 16, 4))
                    pts = [pp.tile([128, 512], f32, name="pchunk0")
                           for _ in tlist]
                    for dw in range(3):
                        for ti, t in enumerate(tlist):
                            nc.tensor.matmul(
                                pts[ti][:], C["lhsT0"][dw][:],
                                b3[0:15, t:t + 2, dw:dw + 256],
                                start=(dw == 0), stop=(dw == 2))
                    for ti, t in enumerate(tlist):
                        self._evict(pts[ti], ep, (ssum_cols, sqsum_cols), 0, cidx,
                                    C["y"][0], r0 + t)
                        cidx += 1
                if bi == NBAND - 2:
                    srA = self._stats_partial(sp, 0, 0, ssum_cols,
                                              sqsum_cols, 0, cidx)
                    csplit = cidx
            srB = self._stats_partial(sp, 0, 1, ssum_cols, sqsum_cols,
                                      csplit, cidx)
            return self._stats_combine(sp, 0, srA, srB)

    # ------------------------------------------------------------------
    def _lowpass(self, tc):
        """fft2 -> disk mask -> ifft2, all 3 channels per pass (f32r),
        writing lpn_d (negated floor) and imgb_d (bf16 image cast)."""
        nc = self.nc
        C = self._consts
        d = self.d
        DF = C["DF"]

        def LT(nm, chunk):
            return DF[nm + ("_hi" if chunk == 0 else "_lo")]

        with ExitStack() as S:
            xp = S.enter_context(tc.tile_pool(name="lp_x", bufs=1))
            zp = S.enter_context(tc.tile_pool(name="lp_z", bufs=1))
            pp = S.enter_context(tc.tile_pool(name="lp_ps", bufs=1, space="PSUM"))
            ep = S.enter_context(tc.tile_pool(name="lp_ev", bufs=3))
            xt = {}
            for c in range(3):
                for hb in range(2):
                    t = xp.tile([128, 256], f32r, name=f"lpx{c}{hb}")
                    nc.sync.dma_start(
                        t[:], d["image"][c, hb * 128:(hb + 1) * 128, :].bitcast(f32r))
                    xb = ep.tile([128, 256], bf, name="lp_xb")
                    nc.vector.tensor_copy(xb[:], t[:].bitcast(f32))
                    nc.sync.dma_start(
                        C["imgb_d"][c, hb * 128:(hb + 1) * 128, :], xb[:])
                    xt[c, hb] = t
            Zt = {}
            for c in range(3):
                for wbl in range(2):
                    pZ = pp.tile([128, 512], f32, name="lp_ps1", bufs=2)
                    for ch in range(2):
                        nc.tensor.matmul(pZ[:], xt[c, ch][:, wbl * 128:(wbl + 1) * 128],
                                         LT("cCS", ch)[:], start=(ch == 0),
                                         stop=(ch == 1))
                    z = zp.tile([128, 512], f32r, name=f"lpz{c}{wbl}")
                    nc.vector.tensor_copy(z[:], pZ[:])
                    Zt[c, wbl] = z
            fsb = {}
            for c in range(3):
                for kwbl in range(2):
                    pf = pp.tile([128, 512], f32, name="lp_ps2", bufs=2)
                    sl = slice(kwbl * 128, (kwbl + 1) * 128)
                    for ch in range(2):
                        nc.tensor.matmul(pf[:], LT("cC", ch)[:, sl], Zt[c, ch][:],
                                         start=(ch == 0), stop=False)
                    for ch in range(2):
                        nc.tensor.matmul(pf[:, 0:256], LT("cNS", ch)[:, sl],
                                         Zt[c, ch][:, 256:512], start=False, stop=False)
                    for ch in range(2):
                        nc.tensor.matmul(pf[:, 256:512], LT("cS", ch)[:, sl],
                                         Zt[c, ch][:, 0:256], start=False,
                                         stop=(ch == 1))
                    ft = zp.tile([128, 512], f32r, name=f"lpf{c}{kwbl}")
                    nc.vector.tensor_tensor(ft[:, 0:256], pf[:, 0:256],
                                            C["maskt"][kwbl][:], op=ALU.mult)
                    nc.vector.tensor_tensor(ft[:, 256:512], pf[:, 256:512],
                                            C["maskt"][kwbl][:], op=ALU.mult)
                    fsb[c, kwbl] = ft
            Gt = {}
            for c in range(3):
                for khbl in range(2):
                    pG = pp.tile([128, 512], f32, name="lp_ps3", bufs=2)
                    sl = slice(khbl * 128, (khbl + 1) * 128)
                    sl2 = slice(256 + khbl * 128, 256 + (khbl + 1) * 128)
                    for ch in range(2):
                        nc.tensor.matmul(pG[:, 0:256], fsb[c, ch][:, sl],
                                         LT("cCi", ch)[:], start=(ch == 0), stop=False)
                    for ch in range(2):
                        nc.tensor.matmul(pG[:, 0:256], fsb[c, ch][:, sl2],
                                         LT("cSq", ch)[:], start=False, stop=False)
                    for ch in range(2):
                        nc.tensor.matmul(pG[:, 256:512], fsb[c, ch][:, sl2],
                                         LT("cCi", ch)[:], start=(ch == 0), stop=False)
                    for ch in range(2):
                        nc.tensor.matmul(pG[:, 256:512], fsb[c, ch][:, sl],
                                         LT("cNSq", ch)[:], start=False, stop=(ch == 1))
                    gt = zp.tile([128, 512], f32r, name=f"lpg{c}{khbl}")
                    nc.vector.tensor_copy(gt[:], pG[:])
                    Gt[c, khbl] = gt
            for c in range(3):
                for hbl in range(2):
                    pE = pp.tile([128, 256], f32, name="lp_pe", bufs=2)
                    sl = slice(hbl * 128, (hbl + 1) * 128)
                    for ch in range(2):
                        nc.tensor.matmul(pE[:], LT("cCi", ch)[:, sl],
                                         Gt[c, ch][:, 0:256], start=(ch == 0),
                                         stop=False)
                    for ch in range(2):
                        nc.tensor.matmul(pE[:], LT("cSq", ch)[:, sl],
                                         Gt[c, ch][:, 256:512], start=False,
                                         stop=(ch == 1))
                    t1 = ep.tile([128, 256], f32, name="lp_t1")
                    nc.vector.tensor_scalar(t1[:], pE[:], 0.0, 255.0,
                                            op0=ALU.max, op1=ALU.min)
                    xi = ep.tile([128, 256], mybir.dt.int32, name="lp_xi")
                    nc.vector.tensor_copy(xi[:], t1[:])
                    xr = ep.tile([128, 256], f32, name="lp_xr")
                    nc.vector.tensor_copy(xr[:], xi[:])
                    gt2 = ep.tile([128, 256], f32, name="lp_gt")
                    nc.vector.tensor_tensor(gt2[:], xr[:], t1[:], op=ALU.is_gt)
                    t2 = ep.tile([128, 256], bf, name="lp_t2")
                    nc.vector.tensor_tensor(t2[:], gt2[:], xr[:], op=ALU.subtract)
                    nc.sync.dma_start(C["lpn_d"][c, hbl * 128:(hbl + 1) * 128, :],
                                      t2[:])

    # ------------------------------------------------------------------
    def _wm_correction(self, tc, sc3, sh3):
        """x0 = relu(bn3(y3[0])); F0 = 9x9 DFT block of x0;
        delta = (wmvT - F0)*kapT; corr = Re(idft(delta))/N^2;
        x0c_d = x0 + corr."""
        nc = self.nc
        C = self._consts
        with ExitStack() as S:
            wp = S.enter_context(tc.tile_pool(name="wm", bufs=1))
            wpp = S.enter_context(tc.tile_pool(name="wm_ps", bufs=1, space="PSUM"))
            # broadcast sc3[0]/sh3[0] to 128 partitions via ones-matmul
            bc = []
            for vi, vec in enumerate((sc3, sh3)):
                prow = wpp.tile([1, 64], f32, name="wm_tr")
                nc.tensor.transpose(prow[:], vec[:], C["ident"][0:64, 0:64])
                row = wp.tile([1, 64], f32, name=f"wm_row{vi}")
                nc.vector.tensor_copy(row[:], prow[:])
                pbc = wpp.tile([128, 1], f32, name="wm_bc")
                nc.tensor.matmul(pbc[:], C["ones1"][:], row[:, 0:1],
                                 start=True, stop=True)
                sb = wp.tile([128, 1], f32, name=f"wm_bcs{vi}")
                nc.vector.tensor_copy(sb[:], pbc[:])
                bc.append(sb)
            sc_bc, sh_bc = bc
            # x0 tiles
            xt0 = []
            for hb in range(2):
                t = wp.tile([128, 256], bf, name=f"wm_x{hb}")
                nc.sync.dma_start(t[:], C["y"][3][0, hb * 128:(hb + 1) * 128, :])
                nc.scalar.activation(t[:], t[:], FT.Relu, bias=sh_bc[:, 0:1],
                                     scale=sc_bc[:, 0:1])
                xt0.append(t)
            # T[18, 256] = E1^T @ x0
            pT = wpp.tile([18, 256], f32, name="wm_pT")
            for hb in range(2):
                nc.tensor.matmul(pT[:], C["cE1"][hb][:], xt0[hb][:],
                                 start=(hb == 0), stop=(hb == 1))
            Tsb = wp.tile([18, 256], f32, name="wm_T")
            nc.vector.tensor_copy(Tsb[:], pT[:])
            # transpose T -> Tt [128,18] x2 (bf16)
            Ttsb = []
            for hc in range(2):
                pTt = wpp.tile([128, 18], f32, name="wm_pTt")
                nc.tensor.transpose(pTt[:], Tsb[:, hc * 128:(hc + 1) * 128],
                                    C["ident"][0:18, 0:18])
                t = wp.tile([128, 18], bf, name=f"wm_Tt{hc}")
                nc.vector.tensor_copy(t[:], pTt[:])
                Ttsb.append(t)
            # F0 combos [18, 18] = Tt^T @ E2
            pF = wpp.tile([18, 18], f32, name="wm_pF")
            for hc in range(2):
                nc.tensor.matmul(pF[:], Ttsb[hc][:], C["cE2"][hc][:],
                                 start=(hc == 0), stop=(hc == 1))
            Fsb = wp.tile([18, 18], f32, name="wm_F")
            nc.vector.tensor_copy(Fsb[:], pF[:])
            Fsh = wp.tile([9, 18], f32, name="wm_Fsh")
            nc.sync.dma_start(Fsh[:], Fsb[9:18, :])
            # F0_re = Fsb[0:9,0:9] + Fsh[:,9:18]; F0_im = Fsh[:,0:9] - Fsb[0:9,9:18]
            F0re = wp.tile([9, 9], f32, name="wm_F0re")
            nc.vector.tensor_tensor(F0re[:], Fsb[0:9, 0:9], Fsh[:, 9:18], op=ALU.add)
            F0im = wp.tile([9, 9], f32, name="wm_F0im")
            nc.vector.tensor_tensor(F0im[:], Fsh[:, 0:9], Fsb[0:9, 9:18],
                                    op=ALU.subtract)
            # delta = (wmvT - F0) * kapT  -> Dstack [18, 9] bf16
            Dstack = wp.tile([18, 9], bf, name="wm_D")
            dre = wp.tile([9, 9], f32, name="wm_dre")
            nc.vector.tensor_tensor(dre[:], C["wmvT"][:], F0re[:], op=ALU.subtract)
            nc.vector.tensor_tensor(Dstack[0:9, :], dre[:], C["kapT"][:], op=ALU.mult)
            dim = wp.tile([9, 9], f32, name="wm_dim")
            nc.vector.tensor_tensor(dim[:], C["wmvT"][:], F0im[:], op=ALU.subtract)
            dimk = wp.tile([9, 9], bf, name="wm_dimk")
            nc.vector.tensor_tensor(dimk[:], dim[:], C["kapT"][:], op=ALU.mult)
            nc.sync.dma_start(Dstack[9:18, :], dimk[:])
            # M rows: Mst [18, 256] bf16 = [M_re; M_im]
            Mst = wp.tile([18, 256], bf, name="wm_Mst")
            pMre = wpp.tile([9, 256], f32, name="wm_pM")
            nc.tensor.matmul(pMre[:], Dstack[:], C["cE3"][:], start=True, stop=True)
            nc.vector.tensor_copy(Mst[0:9, :], pMre[:])
            pMim = wpp.tile([9, 256], f32, name="wm_pM")
            nc.tensor.matmul(pMim[:], Dstack[:], C["cE4"][:], start=True, stop=True)
            mtmp = wp.tile([9, 256], bf, name="wm_mtmp")
            nc.vector.tensor_copy(mtmp[:], pMim[:])
            nc.sync.dma_start(Mst[9:18, :], mtmp[:])
            # corr chunks + add x0 -> x0c_d
            for hc in range(2):
                pC = wpp.tile([128, 256], f32, name="wm_pC")
                nc.tensor.matmul(pC[:], Mst[:, hc * 128:(hc + 1) * 128],
                                 C["cE5"][:], start=True, stop=True)
                cb = wp.tile([128, 256], bf, name="wm_cb")
                nc.vector.tensor_copy(cb[:], pC[:])
                xo = wp.tile([128, 256], bf, name="wm_xo")
                nc.vector.tensor_tensor(xo[:], xt0[hc][:], cb[:], op=ALU.add)
                nc.sync.dma_start(C["x0c_d"][hc * 128:(hc + 1) * 128, :], xo[:])
            ap = self.maybe_debug("x0c", (H, W))
            if ap is not None:
                xf = wp.tile([128, 256], f32, name="wm_xf")
                for hc in range(2):
                    t = wp.tile([128, 256], bf, name="wm_rb")
                    nc.sync.dma_start(t[:], C["x0c_d"][hc * 128:(hc + 1) * 128, :])
                    nc.vector.tensor_copy(xf[:], t[:])
                    nc.sync.dma_start(ap[hc * 128:(hc + 1) * 128, :], xf[:])

    # ------------------------------------------------------------------
    def _build_body(self, X, tc):
        nc = self.nc
        C = self._consts
        d = self.d

        # ============ low-pass filter + image bf16 cast ============
        self._lowpass(tc)

        # ============ conv0..3 ============
        sc, sh = self._conv0(X, tc)
        for k in range(1, 4):
            sc, sh = self._conv64(X, tc, k, C["y"][k - 1], C["y"][k], sc, sh)

        # ============ watermark correction (channel 0) ============
        self._wm_correction(tc, sc, sh)

        # ============ ac conv ============
        sc4, sh4 = self._conv64(X, tc, 4, C["y"][3], C["y"][4], sc, sh)

        # ============ final 1x1 conv ============
        with ExitStack() as S:
            bp = S.enter_context(tc.tile_pool(name="fin_b", bufs=3))
            pp = S.enter_context(tc.tile_pool(name="fin_ps", bufs=4, space="PSUM"))
            ep = S.enter_context(tc.tile_pool(name="fin_ev", bufs=3))
            fb6 = C["cp"].tile([6, 1], f32, name="fb6")
            nc.sync.dma_start(fb6[0:3, :], d["fb"][:])
            nc.sync.dma_start(fb6[3:6, :], d["fb"][:])
            sc128 = C["cp"].tile([128, 1], f32, name="fin_sc128")
            sh128 = C["cp"].tile([128, 1], f32, name="fin_sh128")
            for half in range(2):
                nc.sync.dma_start(sc128[64 * half:64 * half + 64, :], sc4[:])
                nc.sync.dma_start(sh128[64 * half:64 * half + 64, :], sh4[:])
            FR = 16
            for q in range(0, 128, FR):
                xf = bp.tile([128, FR, 256], bf, name="fin_x")
                nc.sync.dma_start(xf[0:64, :, :], C["y"][4][:, q:q + FR, :])
                nc.sync.dma_start(xf[64:128, :, :],
                                  C["y"][4][:, 128 + q:128 + q + FR, :])
                nc.scalar.activation(xf[:].rearrange("p r c -> p (r c)"),
                                     xf[:].rearrange("p r c -> p (r c)"),
                                     FT.Relu, bias=sh128[:, 0:1], scale=sc128[:, 0:1])
                osb = ep.tile([6, FR * 256], f32, name="fin_o")
                for rr in range(0, FR, 2):
                    pt = pp.tile([6, 512], f32, name="fin_p")
                    nc.tensor.matmul(pt[:], C["lhsT_fin"][:],
                                     xf[:, rr:rr + 2, :].rearrange("p r c -> p (r c)"),
                                     start=True, stop=True)
                    nc.vector.tensor_scalar(osb[:, rr * 256:(rr + 2) * 256], pt[:],
                                            fb6[:, 0:1], None, op0=ALU.add)
                nc.sync.dma_start(d["out"][:, q:q + FR, :],
                                  osb[0:3, :].rearrange("p (r c) -> p r c", c=256))
                nc.sync.dma_start(d["out"][:, 128 + q:128 + q + FR, :],
                                  osb[3:6, :].rearrange("p (r c) -> p r c", c=256))

        # debug outputs
        for nm, src, shp in [("lpn", C["lpn_d"], (3, H, W)),
                             ("y0", C["y"][0], (64, H, W)),
                             ("y1", C["y"][1], (64, H, W)),
                             ("y2", C["y"][2], (64, H, W)),
                             ("y3", C["y"][3], (64, H, W)),
                             ("y4", C["y"][4], (64, H, W))]:
            ap = self.maybe_debug(nm, shp, dt=bf)
            if ap is not None:
                nc.sync.dma_start(ap[:], src[:])


# ======================================================================
# harness entry point: full inputs in, full outputs out (8 cores SPMD)
# ======================================================================
from concourse.bass_utils import run_bass_kernel_spmd

_ENC = None

def _get_enc():
    global _ENC
    if _ENC is None:
        e = Enc(n_cores=8)
        e.build()
        _ENC = e
    return _ENC

def make_in_maps(inputs):
    consts = host_constants()
    g = lambda k: np.ascontiguousarray(np.asarray(inputs[k], dtype=np.float32))
    image, message = g("image"), g("message")
    shared = dict(
        w0=g("w0"), b0=g("b0").reshape(64, 1), g0=g("g0").reshape(64, 1),
        be0=g("be0").reshape(64, 1), ws=g("ws"), bs=g("bs").reshape(3, 64, 1),
        gs=g("gs").reshape(3, 64, 1), bes=g("bes").reshape(3, 64, 1),
        acw=g("acw"), acb=g("acb").reshape(64, 1), acg=g("acg").reshape(64, 1),
        acbe=g("acbe").reshape(64, 1), fw=np.ascontiguousarray(g("fw")[:, :, 0, 0]),
        fb=g("fb").reshape(3, 1), **consts)
    return [dict(image=np.ascontiguousarray(image[i]),
                 message=np.ascontiguousarray(message[i].reshape(MSG, 1)),
                 **shared) for i in range(8)]

def kernel(**inputs):
    e = _get_enc()
    in_maps = make_in_maps(inputs)
    res = run_bass_kernel_spmd(e.nc, in_maps, core_ids=list(range(8)))
    out = np.stack([res.results[i]["out"] for i in range(8)], axis=0)
    return np.ascontiguousarray(out.astype(np.float32))
